# revision 1
# baseline (speedup 1.0000x reference)
"""GAT (2-layer graph attention network) Trainium2 Bass kernel.

N=4096 nodes, F=512 feats; layer1: 8 heads x 16 (ELU, concat); layer2:
1 head 128->16; log_softmax. Dense masked attention, row-parallel over
8 cores (core k owns rows [512k, 512k+512)).

Score restructure (vs direct exp pipeline): leaky(s) = max(s, 0.2s) with
s = f1[i]+f2[j] splits softmax numerators into two rank-separable
branches selected by S = [s >= 0]:
  P = exp(leaky(s)-M)*adj = S*adj*A[i]*B[j] + (1-S)*adj*C[i]*D[j]
with A=exp(f1-f1max), B=exp(f2+f1max-M), C=exp(.2(f1-f1max)),
D=exp(.2 f2+.2 f1max-M). Per N^2 tile only the 0/1 mask m1 = S*adj is
materialized (one 4x-mode tensor_scalar is_ge OR one steep ACT Sigmoid, plus one
2x tensor_tensor mult against adj). The exp factors ride in
the matmul lhsT [B*Whaug | D*Whaug] (49-wide with a zero gap so both
PSUM halves start at legal partitions 0/32), and the complement branch
uses Q = sum(adj*D*Wh), a batched head-parallel matmul against the same
D-scaled lhsT copy
(bf16 rounding cancels coherently; Q'/r0 copies kept fp32).
Epilogue per head (i free dim): t = stB + w[i]*(Q - stD),
w = C/A = exp(.8(f1max-f1)); h = t[0:16]/t[16].
"""

import os
import sys
import contextlib

for _p in ("/opt/trn_rl_repo",):
    if _p not in sys.path and os.path.isdir(_p):
        sys.path.insert(0, _p)

import numpy as np
import ml_dtypes

import concourse.bass as bass
import concourse.bacc as bacc
import concourse.tile as tile
from concourse import mybir
from concourse.bass import broadcast_tensor_aps
from concourse.bass_utils import run_bass_kernel_spmd

BF16 = ml_dtypes.bfloat16
ALPHA = 0.2

F = 512      # input features
H = 8        # heads (layer 1)
D = 16       # per-head dim
C = 16       # classes
P = 128      # partitions
NCORES = 8
E = D + 1    # Wh columns + ones column
U = H + 1    # units: 8 layer-1 heads + 1 layer-2 head
W2 = 32      # whBD half-stride (D-half starts at psum partition 32)

BB, BD, BW = 0, 1, 2   # bias columns: f1max-M, .2f1max-M, .8f1max
NBIAS = 3
KSTEEP = 1024.0        # sigmoid steepness for the ACT mask variant


def build_gat(n=4096, ncores=NCORES, dbg=False, no_collective=False,
              act_frac=0.42, pool_frac=0.42, act_frac2=0.3, pool_frac2=0.25,
              ttbufs=4, attbufs=2, jb=4):
    """Build the SPMD Bass program for one core (row-parallel)."""
    R = n // ncores          # rows per core
    IC = R // P              # i-blocks per core
    JT = n // P              # j-tiles (partition tiles of full node dim)
    FC = F // P              # f chunks
    HD = H * D               # 128
    JB = jb                  # j-tiles per mask batch
    NB = JT // JB
    WCAT = HD + H            # fused Wh|w2c matmul width
    assert R % P == 0 and JT % JB == 0

    fp32 = mybir.dt.float32
    f32r = mybir.dt.float32r
    bf16 = mybir.dt.bfloat16

    nc = bacc.Bacc("TRN2", target_bir_lowering=False, debug=dbg,
                   num_devices=ncores)

    XWC = R + WCAT + H       # combined xTm|Wcat|w1c load width
    xT = nc.dram_tensor("xT", [F, n], bf16, kind="ExternalInput").ap()
    xw = nc.dram_tensor("xw", [F, XWC], bf16, kind="ExternalInput").ap()
    adjm = nc.dram_tensor("adjm", [R, n], bf16, kind="ExternalInput").ap()
    Wo2 = nc.dram_tensor("Wo2", [HD, C + 2], bf16, kind="ExternalInput").ap()
    identf = nc.dram_tensor("identf", [P, P], fp32, kind="ExternalInput").ap()
    out = nc.dram_tensor("out", [R, C], fp32, kind="ExternalOutput").ap()

    AF = mybir.ActivationFunctionType
    ALU = mybir.AluOpType
    AX = mybir.AxisListType

    # per-tile variant assignment (ACT Sigmoid vs DVE is_ge) and per-batch
    # mask engine (Pool vs DVE), spread evenly by fractional accumulator.
    # Layer 2 (unit U-1) is latency-critical with an otherwise-idle DVE, so
    # it uses its own (lower) fractions.
    def spread(total, frac):
        picked = set()
        acc = 0.0
        for i in range(total):
            acc += frac
            if acc >= 1.0:
                acc -= 1.0
                picked.add(i)
        return picked

    act_tiles = spread(H * JT, act_frac)
    act_tiles |= {H * JT + i for i in spread(JT, act_frac2)}
    pool_batches = spread(H * NB, pool_frac)
    pool_batches |= {H * NB + i for i in spread(NB, pool_frac2)}

    with tile.TileContext(nc) as tc, contextlib.ExitStack() as ctx:
        big = ctx.enter_context(tc.tile_pool(name="big", bufs=1))
        consts = ctx.enter_context(tc.tile_pool(name="consts", bufs=1))
        work = ctx.enter_context(tc.tile_pool(name="work", bufs=2))
        wk1 = ctx.enter_context(tc.tile_pool(name="wk1", bufs=1))
        sc_t = ctx.enter_context(tc.tile_pool(name="sc_t", bufs=ttbufs))
        psA = ctx.enter_context(tc.tile_pool(name="psA", bufs=2, space="PSUM"))
        psQ = ctx.enter_context(tc.tile_pool(name="psQ", bufs=1, space="PSUM"))
        psATT = ctx.enter_context(
            tc.tile_pool(name="psATT", bufs=attbufs, space="PSUM"))
        psB = ctx.enter_context(tc.tile_pool(name="psB", bufs=2, space="PSUM"))
        dram = ctx.enter_context(tc.tile_pool(name="dram", bufs=1,
                                              space="DRAM"))

        # ---- const / persistent loads ----
        xT_sb = big.tile([P, FC, n], bf16, tag="bigslot")
        for fc in range(FC):
            nc.sync.dma_start(
                xT_sb[:, fc, :],
                xT.rearrange("(c p) n -> p c n", p=P)[:, fc, :])
        xw_sb = consts.tile([P, FC, XWC], bf16)
        nc.sync.dma_start(xw_sb[:], xw.rearrange("(c p) n -> p c n", p=P))
        xTm_sb = xw_sb[:, :, 0:R]
        W1a_sb = xw_sb[:, :, R:R + HD]
        w2c_sb = xw_sb[:, :, R + HD:R + WCAT]
        w1c_sb = xw_sb[:, :, R + WCAT:XWC]
        Wo2_sb = consts.tile([P, C + 2], bf16)
        nc.sync.dma_start(Wo2_sb[:], Wo2)
        WoA_sb = Wo2_sb[:, 0:C + 1]
        w1o_sb = Wo2_sb[:, C + 1:C + 2]
        identf_sb = consts.tile([P, P], fp32)
        nc.sync.dma_start(identf_sb[:], identf)
        # adjacency adjT[j%P, jt, i] = adj[i,j], transposed via the DMA
        # xbar. Own slot + issued right after the const loads so they
        # stream during phase 1 instead of waiting for xT to die.
        adjT = big.tile([P, JT, R], bf16)
        for jt in range(JT):
            nc.sync.dma_start_transpose(adjT[:, jt, :],
                                        adjm[:, jt * P:(jt + 1) * P])

        # persistent intermediates
        whaug = big.tile([P, E, JT, U], bf16)     # [j%P, e, jt, unit]
        whBD = big.tile([P, W2 + E, JT, U], bf16)  # [B*Wh|0|D*Wh] lhsT
        f1b_all = big.tile([P, U, R], bf16)       # f1[i] bcast on partitions
        w_bc = big.tile([P, U, R], bf16)          # w = exp(.8(f1max-f1[i]))
        f2col = big.tile([P, JT, U], fp32)        # f2[j]
        f2colK = big.tile([P, JT, U], fp32)       # KSTEEP*f2[j] (sigmoid)
        bcol = big.tile([P, 1, JT, U], bf16)      # B[j]
        dcol = big.tile([P, 1, JT, U], bf16)      # D[j]
        hT = big.tile([P, R], bf16)               # layer-1 out (elu,cat)^T
        hpre = big.tile([P, R], fp32)
        whD2 = big.tile([P, JT, U * E], bf16)     # D*Whaug, (u,e)-contiguous
        qsb = []                                  # Q' fp32 copies per group
        for g, gp in enumerate((4 * E, 4 * E, E)):
            qt = consts.tile([gp, R], fp32, name=f"qsb{g}")
            qsb.append(qt)
        onesb = consts.tile([1, P], bf16)
        nc.vector.memset(onesb[:], 1.0)
        onesf = consts.tile([1, P], fp32)
        nc.vector.memset(onesf[:], 1.0)

        nc.gpsimd.memset(whaug[:, D:E, :, :], 1.0)      # ones row (all units)
        nc.gpsimd.memset(whBD[:, E:W2, :, :], 0.0)      # psum-gap zeros

        # ---- phase 1: f1 row, fused Wh+f2 matmuls ----
        pf1 = psA.tile([H, R], fp32, tag="ps")
        for fc in range(FC):
            nc.tensor.matmul(pf1[:], lhsT=w1c_sb[:, fc, :],
                             rhs=xTm_sb[:, fc, :],
                             start=(fc == 0), stop=(fc == FC - 1))
        f1row_sb = consts.tile([H, R], fp32)
        nc.vector.tensor_copy(f1row_sb[:], pf1[:])
        f1row_bf = consts.tile([H, R], bf16)
        nc.vector.tensor_copy(f1row_bf[:], f1row_sb[:])
        f1row_1 = consts.tile([1, H, R], bf16)
        nc.scalar.dma_start(f1row_1[:], f1row_bf[:])
        f1max = consts.tile([H, 1], fp32)
        nc.vector.tensor_reduce(f1max[:], f1row_sb[:], axis=AX.X, op=ALU.max)

        # f2 first (cheap stream-8 matmuls) so the f2max/bias chain overlaps
        # the Wh matmuls; both loops copy 4 j-tiles per ACT op.
        for g4 in range(JT // 4):
            pf2 = psA.tile([P, 4, H], fp32, tag="ps")
            for q in range(4):
                jt = g4 * 4 + q
                for fc in range(FC):
                    nc.tensor.matmul(
                        pf2[:, q, :],
                        lhsT=xT_sb[:, fc, jt * P:(jt + 1) * P],
                        rhs=w2c_sb[:, fc, :],
                        start=(fc == 0), stop=(fc == FC - 1))
            nc.scalar.copy(f2col[:, g4 * 4:(g4 + 1) * 4, 0:H], pf2[:])
            nc.vector.tensor_scalar_mul(
                f2colK[:, g4 * 4:(g4 + 1) * 4, 0:H],
                f2col[:, g4 * 4:(g4 + 1) * 4, 0:H], KSTEEP)
        for g4 in range(JT // 4):
            pw4 = psA.tile([P, 4, HD], fp32, tag="ps")
            for q in range(4):
                jt = g4 * 4 + q
                for fc in range(FC):
                    nc.tensor.matmul(
                        pw4[:, q, :],
                        lhsT=xT_sb[:, fc, jt * P:(jt + 1) * P],
                        rhs=W1a_sb[:, fc, :],
                        start=(fc == 0), stop=(fc == FC - 1))
            nc.scalar.copy(
                whaug[:, 0:D, g4 * 4:(g4 + 1) * 4, 0:H],
                pw4[:].rearrange("p t (h d) -> p d t h", d=D))

        # f2max per head: free-dim partial max, transpose, reduce
        f2m_p = consts.tile([P, H], fp32)
        for h in range(H):
            nc.vector.tensor_reduce(f2m_p[:, h:h + 1], f2col[:, :, h],
                                    axis=AX.X, op=ALU.max)
        ptm = psB.tile([H, P], fp32, tag="ep")
        nc.tensor.transpose(ptm[:], f2m_p[:], identf_sb[:])
        f2max = consts.tile([H, 1], fp32)
        nc.vector.tensor_reduce(f2max[:], ptm[:], axis=AX.X, op=ALU.max)

        # biases: M = max(m0, .2 m0), m0 = f1max+f2max
        m0 = consts.tile([H, 1], fp32)
        nc.vector.tensor_tensor(m0[:], f1max[:], f2max[:], op=ALU.add)
        Mh = consts.tile([H, 1], fp32)
        nc.vector.scalar_tensor_tensor(Mh[:], in0=m0[:], scalar=ALPHA,
                                       in1=m0[:], op0=ALU.mult, op1=ALU.max)
        bias_cols = consts.tile([H, NBIAS], fp32)
        nc.vector.tensor_tensor(bias_cols[:, BB:BB + 1], f1max[:], Mh[:],
                                op=ALU.subtract)
        nc.vector.scalar_tensor_tensor(
            bias_cols[:, BD:BD + 1], in0=f1max[:], scalar=ALPHA, in1=Mh[:],
            op0=ALU.mult, op1=ALU.subtract)
        nc.vector.tensor_scalar_mul(bias_cols[:, BW:BW + 1], f1max[:], 0.8)
        bias_row = consts.tile([1, H, NBIAS], fp32)
        nc.scalar.dma_start(bias_row[:], bias_cols[:])
        pba = psA.tile([P, H * NBIAS], fp32, tag="ps")
        nc.tensor.matmul(pba[:], lhsT=onesf[:],
                         rhs=bias_row[:].rearrange("o h e -> o (h e)"),
                         start=True, stop=True)
        bias_all = consts.tile([P, H, NBIAS], fp32)
        nc.vector.tensor_copy(
            bias_all[:], pba[:].rearrange("p (h e) -> p h e", e=NBIAS))

        # B/D columns + f1 broadcasts + w broadcasts (layer 1)
        for h in range(H):
            nc.scalar.activation(bcol[:, 0, :, h], f2col[:, :, h], AF.Exp,
                                 bias=bias_all[:, h, BB:BB + 1], scale=1.0)
            nc.scalar.activation(dcol[:, 0, :, h], f2col[:, :, h], AF.Exp,
                                 bias=bias_all[:, h, BD:BD + 1], scale=ALPHA)
            pb = psA.tile([P, R], fp32, tag="ps")
            nc.tensor.matmul(pb[:], lhsT=onesb[:],
                             rhs=f1row_1[0:1, h, :], start=True, stop=True)
            nc.scalar.copy(f1b_all[:, h, :], pb[:])
            nc.scalar.activation(w_bc[:, h, :], f1b_all[:, h, :], AF.Exp,
                                 bias=bias_all[:, h, BW:BW + 1], scale=-0.8)

        # scale whaug into the stacked lhsT (broadcast B/D along e), plus a
        # (u,e)-contiguous copy of the D-half for the batched Q' matmuls
        # (matmul weight APs may only have ONE free dimension)
        def scale_bd(u0, u1):
            src = whaug[:, :, :, u0:u1]
            for col, dst0 in ((bcol, 0), (dcol, W2)):
                a0, a1 = broadcast_tensor_aps(src, col[:, 0:1, :, u0:u1])
                nc.vector.tensor_tensor(whBD[:, dst0:dst0 + E, :, u0:u1],
                                        a0, a1, op=ALU.mult)
            for u in range(u0, u1):
                a0, a1 = broadcast_tensor_aps(
                    whaug[:, :, :, u].rearrange("p e t -> p t e"),
                    dcol[:, 0, :, u:u + 1])
                nc.vector.tensor_tensor(whD2[:, :, u * E:(u + 1) * E],
                                        a0, a1, op=ALU.mult)
        scale_bd(0, H)

        # (adjthr transposes are issued right after the xT loads, top of
        # build; see below the const loads)

        # ---- Q'/r0 batched matmuls (groups: units 0:4, 4:8, [8]) ----
        GROUPS = ((0, 4, 0), (4, 8, 1), (8, U, 2))

        def emit_q(u0, u1, g):
            gp = (u1 - u0) * E
            pq = psQ.tile([gp, R], fp32, tag="psq")
            for jt in range(JT):
                nc.tensor.matmul(pq[:],
                                 lhsT=whD2[:, jt, u0 * E:u1 * E],
                                 rhs=adjT[:, jt, :],
                                 start=(jt == 0), stop=(jt == JT - 1))
            nc.scalar.copy(qsb[g][:], pq[:])

        # ---- attention emitter (one unit = one head or layer 2) ----
        def emit_att(u, patt):
            for b in range(NB):
                jt0 = b * JB
                X = sc_t.tile([P, JB, R], bf16, tag="xs")
                for q in range(JB):
                    jt = jt0 + q
                    if (u * JT + jt) in act_tiles:
                        nc.scalar.activation(X[:, q, :], f1b_all[:, u, :],
                                             AF.Sigmoid,
                                             bias=f2colK[:, jt, u:u + 1],
                                             scale=KSTEEP)
                    else:
                        nc.vector.tensor_scalar(X[:, q, :],
                                                f1b_all[:, u, :],
                                                f2col[:, jt, u:u + 1], 0.0,
                                                op0=ALU.add, op1=ALU.is_ge)
                eng = (nc.gpsimd if (u * NB + b) in pool_batches
                       else nc.vector)
                eng.tensor_tensor(X[:], X[:],
                                  adjT[:, jt0:jt0 + JB, :], op=ALU.mult)
                for q in range(JB):
                    jt = jt0 + q
                    nc.tensor.matmul(
                        patt[:], lhsT=whBD[:, 0:W2 + E, jt, u],
                        rhs=X[:, q, :],
                        start=(jt == 0), stop=(jt == JT - 1))

        def epilogue(u, patt, g, goff):
            """t = stB - w*(Q' + stD - 1.5 r0). All engine-op operands
            base-partition 0 (walrus same-start-partition rule); the
            base-32 D-half and base-32h' Q' slices are DMA-extracted."""
            st = work.tile([W2 + E, R], fp32, tag="st")
            nc.scalar.copy(st[:], patt[:])
            sd = wk1.tile([E, R], fp32, tag="sd")
            nc.sync.dma_start(sd[:], st[W2:W2 + E, :])
            if goff == 0:
                qh = qsb[g][0:E, :]
            else:
                qx = wk1.tile([E, R], fp32, tag="qx")
                nc.sync.dma_start(qx[:], qsb[g][goff:goff + E, :])
                qh = qx[:]
            x1 = wk1.tile([E, R], fp32, tag="x1")
            nc.vector.tensor_tensor(x1[:], qh, sd[:], op=ALU.subtract)
            v = wk1.tile([E, R], fp32, tag="v")
            nc.vector.tensor_tensor(v[:], x1[:], w_bc[0:E, u, :],
                                    op=ALU.mult)
            t = work.tile([E, R], fp32, tag="t")
            nc.vector.tensor_tensor(t[:], st[0:E, :], v[:], op=ALU.add)
            return t

        # ---- layer-1 attention + epilogues ----
        # Q' groups are emitted after the first two heads' attention so the
        # in-order PE queue is not blocked waiting for all 32 adjthr
        # transposes before any attention matmul can start.
        for h in range(H):
            patt = psATT.tile([W2 + E, R], fp32, tag="att")
            emit_att(h, patt)
            if h < 2:
                emit_q(*GROUPS[h][:2], GROUPS[h][2])
            g = 0 if h < 4 else 1
            t = epilogue(h, patt, g, (h % 4) * E)
            den = wk1.tile([1, R], fp32, tag="den")
            nc.sync.dma_start(den[:], t[D:E, :])
            rec = wk1.tile([1, R], fp32, tag="rec")
            nc.vector.reciprocal(rec[:], den[:])
            prb = psB.tile([D, R], fp32, tag="ep")
            nc.tensor.matmul(prb[:], lhsT=onesf[0:1, 0:D], rhs=rec[:],
                             start=True, stop=True)
            hph = wk1.tile([D, R], fp32, tag="hph")
            nc.vector.tensor_tensor(hph[:], t[0:D, :], prb[:], op=ALU.mult)
            nc.sync.dma_start(hpre[h * D:(h + 1) * D, :], hph[:])

        # ELU: elu(x) = max(x, min(exp(x)-1, 0))
        etile = wk1.tile([P, R], fp32, tag="etile")
        nc.scalar.activation(etile[:], hpre[:], AF.Exp, bias=0.0, scale=1.0)
        em = wk1.tile([P, R], fp32, tag="em")
        nc.vector.tensor_scalar(em[:], etile[:], 1.0, 0.0,
                                op0=ALU.subtract, op1=ALU.min)
        nc.vector.tensor_tensor(hT[:], hpre[:], em[:], op=ALU.max)

        # ---- layer 2 ----
        gsrc = dram.tile([R, C + 1], fp32)
        for icb in range(IC):
            pg = psB.tile([P, C + 1], fp32, tag="ep")
            nc.tensor.matmul(pg[:], lhsT=hT[:, icb * P:(icb + 1) * P],
                             rhs=WoA_sb[:], start=True, stop=True)
            gs = wk1.tile([P, C + 1], fp32, tag="gs")
            nc.vector.tensor_copy(gs[:], pg[:])
            nc.sync.dma_start(
                gsrc[:].rearrange("(c p) e -> p c e", p=P)[:, icb, :], gs[:])
        gdst = dram.tile([n, C + 1], fp32)
        if no_collective:
            # timing-sim stand-in (TimelineSim can't model collectives)
            for k in range(ncores):
                nc.sync.dma_start(gdst[k * R:(k + 1) * R, :], gsrc[:])
        else:
            nc.gpsimd.collective_compute(
                "AllGather", ALU.bypass,
                replica_groups=[list(range(ncores))],
                ins=[gsrc.opt()], outs=[gdst.opt()])

        g_sb = big.tile([P, JT, C + 1], fp32)
        nc.sync.dma_start(g_sb[:], gdst[:].rearrange("(t p) e -> p t e", p=P))
        nc.scalar.copy(whaug[:, 0:C, :, U - 1],
                       g_sb[:, :, 0:C].rearrange("p t e -> p e t"))
        nc.scalar.copy(f2col[:, :, U - 1], g_sb[:, :, C])
        nc.vector.tensor_scalar_mul(f2colK[:, :, U - 1:U],
                                    f2col[:, :, U - 1:U], KSTEEP)

        pf1o = psB.tile([1, R], fp32, tag="ep")
        nc.tensor.matmul(pf1o[:], lhsT=w1o_sb[:], rhs=hT[:],
                         start=True, stop=True)
        f1orow = consts.tile([1, R], fp32)
        nc.vector.tensor_copy(f1orow[:], pf1o[:])
        f1orow_bf = consts.tile([1, R], bf16)
        nc.vector.tensor_copy(f1orow_bf[:], f1orow[:])
        pf1ob = psB.tile([P, R], fp32, tag="ep")
        nc.tensor.matmul(pf1ob[:], lhsT=onesb[:], rhs=f1orow_bf[:],
                         start=True, stop=True)
        nc.scalar.copy(f1b_all[:, U - 1, :], pf1ob[:])

        f1omax = consts.tile([1, 1], fp32)
        nc.vector.tensor_reduce(f1omax[:], f1orow[:], axis=AX.X, op=ALU.max)
        f2ored = consts.tile([P, 1], fp32)
        nc.vector.tensor_reduce(f2ored[:], f2col[:, :, U - 1], axis=AX.X,
                                op=ALU.max)
        ptm2 = psB.tile([1, P], fp32, tag="ep")
        nc.tensor.transpose(ptm2[:], f2ored[:], identf_sb[:])
        f2omax = consts.tile([1, 1], fp32)
        nc.vector.tensor_reduce(f2omax[:], ptm2[:], axis=AX.X, op=ALU.max)
        m2 = consts.tile([1, 1], fp32)
        nc.vector.tensor_tensor(m2[:], f1omax[:], f2omax[:], op=ALU.add)
        M2 = consts.tile([1, 1], fp32)
        nc.vector.scalar_tensor_tensor(M2[:], in0=m2[:], scalar=ALPHA,
                                       in1=m2[:], op0=ALU.mult, op1=ALU.max)
        b2_cols = consts.tile([1, NBIAS], fp32)
        nc.vector.tensor_tensor(b2_cols[:, BB:BB + 1], f1omax[:], M2[:],
                                op=ALU.subtract)
        nc.vector.scalar_tensor_tensor(
            b2_cols[:, BD:BD + 1], in0=f1omax[:], scalar=ALPHA, in1=M2[:],
            op0=ALU.mult, op1=ALU.subtract)
        nc.vector.tensor_scalar_mul(b2_cols[:, BW:BW + 1], f1omax[:], 0.8)
        pb2 = psB.tile([P, NBIAS], fp32, tag="ep")
        nc.tensor.matmul(pb2[:], lhsT=onesf[:], rhs=b2_cols[:],
                         start=True, stop=True)
        bias2 = consts.tile([P, NBIAS], fp32)
        nc.vector.tensor_copy(bias2[:], pb2[:])

        nc.scalar.activation(bcol[:, 0, :, U - 1], f2col[:, :, U - 1], AF.Exp,
                             bias=bias2[:, BB:BB + 1], scale=1.0)
        nc.scalar.activation(dcol[:, 0, :, U - 1], f2col[:, :, U - 1], AF.Exp,
                             bias=bias2[:, BD:BD + 1], scale=ALPHA)
        nc.scalar.activation(w_bc[:, U - 1, :], f1b_all[:, U - 1, :], AF.Exp,
                             bias=bias2[:, BW:BW + 1], scale=-0.8)
        scale_bd(U - 1, U)
        emit_q(U - 1, U, 2)

        patt2 = psATT.tile([W2 + E, R], fp32, tag="att")
        emit_att(U - 1, patt2)
        t2 = epilogue(U - 1, patt2, 2, 0)

        # final: transpose (incl. denominator row), normalize, log_softmax
        # (stage-major across i-blocks: one Exp + one Ln total, no ACT
        # table thrashing; single output DMA)
        pos = wk1.tile([P, IC, C + 1], fp32, tag="pos")
        for icb in range(IC):
            po = psB.tile([P, C + 1], fp32, tag="ep")
            nc.tensor.transpose(po[:], t2[:, icb * P:(icb + 1) * P],
                                identf_sb[0:C + 1, 0:C + 1])
            nc.vector.tensor_copy(pos[:, icb, :], po[:])
        rc = wk1.tile([P, IC, 1], fp32, tag="rc")
        nc.vector.reciprocal(rc[:], pos[:, :, C:C + 1])
        z = wk1.tile([P, IC, C], fp32, tag="z")
        a0, a1 = broadcast_tensor_aps(pos[:, :, 0:C], rc[:])
        nc.vector.tensor_tensor(z[:], a0, a1, op=ALU.mult)
        negmx = wk1.tile([P, IC, 1], fp32, tag="negmx")
        nc.vector.tensor_reduce(negmx[:], z[:], axis=AX.X, op=ALU.max,
                                negate=True)
        zs = wk1.tile([P, IC, C], fp32, tag="zs")
        a0, a1 = broadcast_tensor_aps(z[:], negmx[:])
        nc.vector.tensor_tensor(zs[:], a0, a1, op=ALU.add)
        ez = wk1.tile([P, IC, C], fp32, tag="ez")
        nc.scalar.activation(ez[:], zs[:], AF.Exp, bias=0.0, scale=1.0)
        sume = wk1.tile([P, IC, 1], fp32, tag="sume")
        nc.vector.tensor_reduce(sume[:], ez[:], axis=AX.X, op=ALU.add)
        lns = wk1.tile([P, IC, 1], fp32, tag="lns")
        nc.scalar.activation(lns[:], sume[:], AF.Ln, bias=0.0, scale=1.0)
        zo = wk1.tile([P, IC, C], fp32, tag="zo")
        a0, a1 = broadcast_tensor_aps(zs[:], lns[:])
        nc.vector.tensor_tensor(zo[:], a0, a1, op=ALU.subtract)
        nc.sync.dma_start(out.rearrange("(c p) e -> p c e", p=P), zo[:])

    nc.compile()
    return nc


def prep_inputs(x, adj, W1, a1, Wout, a_out, n=4096, ncores=NCORES):
    """Host-side prep: slice + transpose + bf16 cast + weight folds."""
    R = n // ncores
    x = np.asarray(x, np.float32)
    adj = np.asarray(adj)
    W1 = np.asarray(W1, np.float32)
    a1 = np.asarray(a1, np.float32)
    Wout = np.asarray(Wout, np.float32)
    a_out = np.asarray(a_out, np.float32)

    xT = np.ascontiguousarray(x.T).astype(BF16)
    W1a = W1.transpose(1, 0, 2).reshape(F, H * D)
    w2c = np.einsum("hfd,hd->fh", W1, a1[:, D:])
    w1c = np.einsum("hfd,hd->fh", W1, a1[:, :D])
    w2o = Wout @ a_out[C:]
    w1o = Wout @ a_out[:C]
    Wo2 = np.ascontiguousarray(np.concatenate(
        [Wout, w2o[:, None], w1o[:, None]], axis=1)).astype(BF16)
    identf = np.eye(P, dtype=np.float32)

    adjb = adj.astype(np.float32).astype(BF16)
    in_maps = []
    for k in range(ncores):
        rows = slice(k * R, (k + 1) * R)
        xwk = np.concatenate([x[rows].T, W1a, w2c, w1c], axis=1)
        in_maps.append({
            "xT": xT,
            "xw": np.ascontiguousarray(xwk).astype(BF16),
            "adjm": np.ascontiguousarray(adjb[rows]),
            "Wo2": Wo2,
            "identf": identf,
        })
    return in_maps


_cached = {}


def kernel(x, adj, W1, a1, Wout, a_out):
    n = x.shape[0]
    if n not in _cached:
        _cached[n] = build_gat(n=n)
    nc = _cached[n]
    in_maps = prep_inputs(x, adj, W1, a1, Wout, a_out, n=n)
    res = run_bass_kernel_spmd(nc, in_maps, core_ids=list(range(NCORES)))
    outs = [res.results[k]["out"] for k in range(NCORES)]
    return np.concatenate(outs, axis=0)



# revision 32
# speedup vs baseline: 1.4483x; 1.4483x over previous
"""GAT (2-layer graph attention network) Trainium2 Bass kernel — v3.

N=4096 nodes, F=512 feats; layer1: 8 heads x 16 (ELU, concat); layer2:
1 head 128->16; log_softmax. Dense masked attention, row-parallel over
8 cores (core k owns rows [512k, 512k+512)).

Score restructure: leaky(s) = max(s, 0.2s) with s = f1[i]+f2[j] splits
softmax numerators into two rank-separable branches selected by
S = [s >= 0]:
  P = exp(leaky(s)-M)*adj = S*adj*A[i]*B[j] + (1-S)*adj*C[i]*D[j]
with A=exp(f1-f1max), B=exp(f2+f1max-M), C=exp(.2(f1-f1max)),
D=exp(.2 f2+.2 f1max-M), M = f1max + FSLACK.  The constant-slack M
(instead of f1max+f2max) needs no global f2 reduction, so the bias /
exp / scale chain leaves the critical path; the uniform exp(-FSLACK)
factor cancels in the softmax division.

Masks: a tile's 0/1 mask X = S*adj is a first op producing S
(DVE tensor_scalar 4x is_ge, or ACT steep Sigmoid) followed by a
batched multiply against adjT (DVE 2x or Pool), spread across the
three engines by a tunable per-batch pattern (kinds: A=ACT+DVE,
B=ACT+Pool, V=DVE+DVE, W=DVE+Pool).

Attention matmuls are FLIPPED: X[j, i-block] is the PE stationary
operand and the scaled factors whB/whD [j, 17] are the moving operand
(out free size 17, not 512); each (unit, i-block) accumulation chain
owns one PSUM bank, B/D halves share the chain.  Epilogue scalars
(w[i], 1/den) are per-partition; log_softmax needs no transposes.
The complement branch Q = sum_j adj*D*Wh uses lhsT=adjT directly.
Layer 2 gathers
the f2o column separately from the Wh2 block so unit-8 mask compute
overlaps the main gather.
"""

import os
import sys
import contextlib

for _p in ("/opt/trn_rl_repo",):
    if _p not in sys.path and os.path.isdir(_p):
        sys.path.insert(0, _p)

import numpy as np
import ml_dtypes

import concourse.bass as bass
import concourse.bacc as bacc
import concourse.tile as tile
from concourse import mybir
from concourse.bass import broadcast_tensor_aps
from concourse.bass_utils import run_bass_kernel_spmd

BF16 = ml_dtypes.bfloat16
ALPHA = 0.2

F = 512      # input features
H = 8        # heads (layer 1)
D = 16       # per-head dim
C = 16       # classes
P = 128      # partitions
NCORES = 8
E = D + 1    # Wh columns + ones column (17)
U = H + 1    # units: 8 layer-1 heads + 1 layer-2 head
HD = H * D   # 128

BB, BD, BW = 0, 1, 2   # bias cols: -FSLACK, -.8*f1max-FSLACK, .8*f1max
NBIAS = 3
KSTEEP = 1024.0        # sigmoid / score steepness
FSLACK = 30.0          # constant softmax shift slack (M = f1max + FSLACK)


def build_gat(n=4096, ncores=NCORES, dbg=False, no_collective=False,
              pat=("A", "V", "B", "A", "V", "V", "A", "V"),
              pat0=("V", "W", "V", "V", "V", "V", "W", "V"),
              pat2=("V", "V", "W", "V", "V", "V", "W", "V"),
              ttbufs=12, jb=4):
    """Build the SPMD Bass program for one core (row-parallel)."""
    R = n // ncores          # rows per core
    IC = R // P              # i-blocks per core
    JT = n // P              # j-tiles (partition tiles of full node dim)
    FC = F // P              # f chunks
    JB = jb                  # j-tiles per mask batch
    NB = JT // JB
    NCH = n // FC            # xT column-chunk width
    RHSW = H + HD            # fused f2|Wh matmul width (136)
    XWC = R + RHSW + H       # combined xTm|rhs|w1c load width
    PSW = max(R, RHSW)       # shared psA slot width
    assert R % P == 0 and JT % JB == 0

    fp32 = mybir.dt.float32
    bf16 = mybir.dt.bfloat16

    nc = bacc.Bacc("TRN2", target_bir_lowering=False, debug=dbg,
                   num_devices=ncores)

    xT = nc.dram_tensor("xT", [F, n], bf16, kind="ExternalInput").ap()
    xw = nc.dram_tensor("xw", [F, XWC], bf16, kind="ExternalInput").ap()
    adjt = nc.dram_tensor("adjt", [P, JT * R], bf16,
                          kind="ExternalInput").ap()
    Wo2 = nc.dram_tensor("Wo2", [HD, C + 2], bf16, kind="ExternalInput").ap()
    identf = nc.dram_tensor("identf", [P, P], fp32, kind="ExternalInput").ap()
    out = nc.dram_tensor("out", [R, C], fp32, kind="ExternalOutput").ap()

    AF = mybir.ActivationFunctionType
    ALU = mybir.AluOpType
    AX = mybir.AxisListType

    with tile.TileContext(nc) as tc, contextlib.ExitStack() as ctx:
        big = ctx.enter_context(tc.tile_pool(name="big", bufs=1))
        consts = ctx.enter_context(tc.tile_pool(name="consts", bufs=1))
        work = ctx.enter_context(tc.tile_pool(name="work", bufs=2))
        wk1 = ctx.enter_context(tc.tile_pool(name="wk1", bufs=1))
        sc_t = ctx.enter_context(tc.tile_pool(name="sc_t", bufs=ttbufs))
        psA = ctx.enter_context(tc.tile_pool(name="psA", bufs=2, space="PSUM"))
        psQ = ctx.enter_context(tc.tile_pool(name="psQ", bufs=1, space="PSUM"))
        psATT = ctx.enter_context(
            tc.tile_pool(name="psATT", bufs=4, space="PSUM"))
        psB = ctx.enter_context(tc.tile_pool(name="psB", bufs=1, space="PSUM"))
        dram = ctx.enter_context(tc.tile_pool(name="dram", bufs=1,
                                              space="DRAM"))

        # ---- const / persistent loads (spread across DMA queues).
        # xT is loaded in COLUMN chunks so pf12 chain jt can close as soon
        # as the chunk holding its columns lands.
        # big loads ride the SP + Pool DMA queues only; the ACT queue is
        # reserved for small latency-critical transfers (Wo2, f1row_1,
        # bias_row, the layer-2 gathers)
        bigq = [nc.sync, nc.gpsimd]
        qeng = [nc.sync, nc.scalar, nc.gpsimd]
        identf_sb = consts.tile([P, P], fp32)
        nc.gpsimd.dma_start(identf_sb[:], identf)
        xw_sb = consts.tile([P, FC, XWC], bf16)
        nc.sync.dma_start(xw_sb[:], xw.rearrange("(c p) n -> p c n", p=P))
        xT_sb = big.tile([P, FC, n], bf16, tag="xtslot")
        adjT = big.tile([P, JT, R], bf16)
        ACH = JT // 4
        Wo2_sb = consts.tile([P, C + 2], bf16)
        nc.scalar.dma_start(Wo2_sb[:], Wo2)
        # interleave xT column chunks with adjBG chunks: pf12 chain jt and
        # the mask tiles for jt both become runnable ~3us after their chunk
        for c in range(4):
            bigq[c % 2].dma_start(
                xT_sb[:, :, c * NCH:(c + 1) * NCH],
                xT.rearrange("(c p) n -> p c n", p=P)
                [:, :, c * NCH:(c + 1) * NCH])
            bigq[(c + 1) % 2].dma_start(
                adjT[:, c * ACH:(c + 1) * ACH, :],
                adjt[:, c * ACH * R:(c + 1) * ACH * R]
                .rearrange("p (t r) -> p t r", r=R))
        xTm_sb = xw_sb[:, :, 0:R]
        rhs_sb = xw_sb[:, :, R:R + RHSW]          # [w2c(8) | W1a(128)]
        w1c_sb = xw_sb[:, :, R + RHSW:XWC]
        w1o_sb = Wo2_sb[:, C + 1:C + 2]

        identb_sb = consts.tile([P, P], bf16)
        onesb = consts.tile([1, P], bf16)
        nc.vector.memset(onesb[:], 1.0)
        onesf = consts.tile([1, P], fp32)
        nc.vector.memset(onesf[:], 1.0)
        onescol = consts.tile([P, 1], bf16)
        nc.vector.memset(onescol[:], 1.0)

        # persistent intermediates
        whaug = big.tile([P, JT, HD], bf16)       # Wh columns per j
        whB = big.tile([P, JT, U, E], bf16)       # B_j*[Wh_u|1]
        whD2 = big.tile([P, JT, U * E], bf16)     # D_j*[Wh_u|1], (u,e)-contig
        f1b_all = big.tile([P, U, R], bf16)       # f1[i] bcast on partitions
        f2col = big.tile([P, JT, H], fp32)        # f2[j] (layer 1)
        f2colK = big.tile([P, JT, H], fp32)       # KSTEEP*f2[j]
        bcol = consts.tile([P, JT, H], bf16)
        dcol = consts.tile([P, JT, H], bf16)
        w_col = consts.tile([P, IC, U], fp32)     # exp(.8(f1max-f1[i]))
        hpre = big.tile([P, IC, HD], fp32)        # layer-1 out pre-ELU
        h_sb = big.tile([P, IC, HD], bf16)        # post-ELU
        hT_sb = big.tile([P, IC, P], bf16)        # transposed h blocks
        qsb = consts.tile([P, IC, H * E], fp32)   # Q for layer-1 units
        st_all = big.tile([P, IC, H, 2, E], fp32)  # drained attention psums
        f1colT = consts.tile([P, IC, H], fp32)    # f1[i] per-partition

        # ---- phase 1a: f1 row (own rows) ----
        pf1 = psB.tile([H, R], fp32, tag="ep")
        for fc in range(FC):
            nc.tensor.matmul(pf1[:], lhsT=w1c_sb[:, fc, :],
                             rhs=xTm_sb[:, fc, :],
                             start=(fc == 0), stop=(fc == FC - 1))
        f1row_sb = consts.tile([H, R], fp32)
        nc.vector.tensor_copy(f1row_sb[:], pf1[:])
        f1row_bf = consts.tile([H, R], bf16)
        nc.vector.tensor_copy(f1row_bf[:], f1row_sb[:])
        f1row_1 = consts.tile([1, H, R], bf16)
        nc.scalar.dma_start(f1row_1[:], f1row_bf[:])
        f1max = consts.tile([H, 1], fp32)
        nc.vector.tensor_reduce(f1max[:], f1row_sb[:], axis=AX.X, op=ALU.max)

        # f1 broadcast for head 0 first (gates the first mask tiles)
        def f1b_bcast(h):
            pbf = psA.tile([P, PSW], fp32, tag="ps", name="pbf")
            pb = pbf[:, 0:R]
            nc.tensor.matmul(pb[:], lhsT=onesb[:],
                             rhs=f1row_1[0:1, h, :], start=True, stop=True)
            nc.scalar.copy(f1b_all[:, h, :], pb[:])
        f1b_bcast(0)

        # ---- phase 1b: fused f2|Wh matmuls over all j ----
        for jt in range(JT):
            pf12f = psA.tile([P, PSW], fp32, tag="ps", name="pf12f")
            pf12 = pf12f[:, 0:RHSW]
            for fc in range(FC):
                nc.tensor.matmul(
                    pf12[:],
                    lhsT=xT_sb[:, fc, jt * P:(jt + 1) * P],
                    rhs=rhs_sb[:, fc, :],
                    start=(fc == 0), stop=(fc == FC - 1))
            nc.vector.tensor_copy(f2col[:, jt, :], pf12[:, 0:H])
            nc.scalar.copy(whaug[:, jt, :], pf12[:, H:RHSW])

        for h in range(1, H):
            f1b_bcast(h)

        # f1 per-partition columns (for epilogue w)
        nc.vector.tensor_copy(identb_sb[:], identf_sb[:])
        for ib in range(IC):
            pt = psB.tile([P, H], bf16, tag="ep", name="pt")
            nc.tensor.transpose(pt[:], f1row_bf[:, ib * P:(ib + 1) * P],
                                identb_sb[0:H, 0:H])
            nc.vector.tensor_copy(f1colT[:, ib, :], pt[:])

        # ---- mask emitter: per-batch engine pattern.
        # 'A': 4 ACT sigmoids + DVE is_gt; 'V': 4 DVE 4x tensor_scalar +
        # DVE is_gt; 'P': 4 Pool fused stt.
        bat_i = [0]

        def emit_masks(u, f2c, f2cK, patv):
            Xs = []
            for b in range(NB):
                jt0 = b * JB
                kind = patv[bat_i[0] % len(patv)]
                bat_i[0] += 1
                X = sc_t.tile([P, JB, R], bf16, tag="xs")
                for q in range(JB):
                    jt = jt0 + q
                    if kind in ("A", "B"):
                        nc.scalar.activation(
                            X[:, q, :], f1b_all[:, u, :], AF.Sigmoid,
                            bias=f2cK[:, jt:jt + 1], scale=KSTEEP)
                    else:
                        nc.vector.tensor_scalar(
                            X[:, q, :], f1b_all[:, u, :],
                            f2c[:, jt:jt + 1], 0.0,
                            op0=ALU.add, op1=ALU.is_ge)
                eng2 = nc.gpsimd if kind in ("B", "W") else nc.vector
                eng2.tensor_tensor(
                    X[:], X[:], adjT[:, jt0:jt0 + JB, :], op=ALU.mult)
                Xs.append(X)
            return Xs

        # attention matmuls (flipped): X is the stationary operand; each
        # (unit, iblock) chain owns one PSUM bank, B/D halves share it.
        def emit_mms(patts, Xs, whB_u, whD_u):
            for b in range(NB):
                X = Xs[b]
                for q in range(JB):
                    jt = b * JB + q
                    for ib in range(IC):
                        lw = X[:, q, ib * P:(ib + 1) * P]
                        nc.tensor.matmul(
                            patts[ib][:, 0, :], lhsT=lw, rhs=whB_u(jt),
                            start=(jt == 0), stop=False)
                        nc.tensor.matmul(
                            patts[ib][:, 1, :], lhsT=lw, rhs=whD_u(jt),
                            start=False, stop=(jt == JT - 1))

        # masks for unit 0 go out BEFORE the scale chain so DVE/ACT/Pool
        # fill buffers while whB/whD are being scaled
        Xs0 = emit_masks(0, f2col[:, :, 0], f2colK[:, :, 0], pat0)

        # biases off f1max only (constant-slack M): BD = -.8f1max-FSLACK,
        # BW = .8f1max; the B-column bias is the constant -FSLACK.
        bias_cols = consts.tile([H, NBIAS], fp32)
        nc.vector.memset(bias_cols[:, BB:BB + 1], -FSLACK)
        nc.vector.tensor_scalar(bias_cols[:, BD:BD + 1], f1max[:], -0.8,
                                -FSLACK, op0=ALU.mult, op1=ALU.add)
        nc.vector.tensor_scalar_mul(bias_cols[:, BW:BW + 1], f1max[:], 0.8)
        bias_row = consts.tile([1, H, NBIAS], fp32)
        nc.scalar.dma_start(bias_row[:], bias_cols[:])
        pba = psB.tile([P, H * NBIAS], fp32, tag="ep", name="pba")
        nc.tensor.matmul(pba[:], lhsT=onesf[:],
                         rhs=bias_row[:].rearrange("o h e -> o (h e)"),
                         start=True, stop=True)
        bias_all = consts.tile([P, H, NBIAS], fp32)
        nc.vector.tensor_copy(
            bias_all[:], pba[:].rearrange("p (h e) -> p h e", e=NBIAS))
        nc.vector.tensor_scalar_mul(f2colK[:], f2col[:], KSTEEP)

        # exps (ACT): B/D columns + per-partition w columns
        for h in range(H):
            nc.scalar.activation(bcol[:, :, h], f2col[:, :, h], AF.Exp,
                                 bias=bias_all[:, h, BB:BB + 1], scale=1.0)
            nc.scalar.activation(dcol[:, :, h], f2col[:, :, h], AF.Exp,
                                 bias=bias_all[:, h, BD:BD + 1], scale=ALPHA)
            nc.scalar.activation(w_col[:, :, h], f1colT[:, :, h], AF.Exp,
                                 bias=bias_all[:, h, BW:BW + 1], scale=-0.8)


        # scale whaug into whB / whD2 (broadcast B/D along e):
        # B-half on DVE, D-half on Pool
        whD2v = whD2[:].rearrange("p t (u e) -> p t u e", e=E)

        def scale_bd(u0, u1, src, bcs, dcs):
            for u in range(u0, u1):
                s = src(u - u0)
                a0, a1 = broadcast_tensor_aps(s, bcs[:, :, u - u0:u - u0 + 1])
                nc.vector.tensor_tensor(whB[:, :, u, 0:D], a0, a1,
                                        op=ALU.mult)
                a0, a1 = broadcast_tensor_aps(s, dcs[:, :, u - u0:u - u0 + 1])
                nc.gpsimd.tensor_tensor(whD2v[:, :, u, 0:D], a0, a1,
                                        op=ALU.mult)
            nc.vector.tensor_copy(
                whB[:, :, u0:u1, D:E],
                bcs[:].rearrange("p t (c o) -> p t c o", o=1))
            nc.vector.tensor_copy(
                whD2v[:, :, u0:u1, D:E],
                dcs[:].rearrange("p t (c o) -> p t c o", o=1))

        scale_bd(0, H, lambda c: whaug[:, :, c * D:(c + 1) * D], bcol, dcol)

        # ---- Q: complement branch, lhsT = adjT directly
        def emit_q(ib):
            pq = psQ.tile([P, H * E], fp32, tag="psq", name="pq")
            for jt in range(JT):
                nc.tensor.matmul(pq[:],
                                 lhsT=adjT[:, jt, ib * P:(ib + 1) * P],
                                 rhs=whD2[:, jt, 0:H * E],
                                 start=(jt == 0), stop=(jt == JT - 1))
            nc.scalar.copy(qsb[:, ib, :], pq[:])

        # Q = colsum - QBG/BIG; batched epilogue halves (first half is
        # emitted mid-loop so it overlaps units 5..7):
        # t = stB + w[i]*(Q - stD); h = t[:,0:16]/t[:,16]
        qv = qsb[:].rearrange("p c (u e) -> p c u e", e=E)
        hprev = hpre[:].rearrange("p c (u d) -> p c u d", d=D)


        def l1_epilogue(u0, u1, s):
            uu = u1 - u0
            x1a = wk1.tile([P, IC, uu, E], fp32, tag=f"x1a{s}")
            nc.vector.tensor_tensor(x1a[:], qv[:, :, u0:u1, :],
                                    st_all[:, :, u0:u1, 1, :],
                                    op=ALU.subtract)
            va = wk1.tile([P, IC, uu, E], fp32, tag=f"va{s}")
            a0, a1 = broadcast_tensor_aps(
                x1a[:],
                w_col[:, :, u0:u1].rearrange("p c (u o) -> p c u o", o=1))
            nc.vector.tensor_tensor(va[:], a0, a1, op=ALU.mult)
            ta = wk1.tile([P, IC, uu, E], fp32, tag=f"ta{s}")
            nc.vector.tensor_tensor(ta[:], st_all[:, :, u0:u1, 0, :], va[:],
                                    op=ALU.add)
            reca = wk1.tile([P, IC, uu, 1], fp32, tag=f"reca{s}")
            nc.vector.reciprocal(reca[:], ta[:, :, :, D:E])
            a0, a1 = broadcast_tensor_aps(ta[:, :, :, 0:D], reca[:])
            nc.vector.tensor_tensor(hprev[:, :, u0:u1, :], a0, a1,
                                    op=ALU.mult)


        # ---- layer-1 attention ----
        ep_eng = [0]
        for h in range(H):
            patts = [psATT.tile([P, 2, E], fp32, tag="att",
                                name=f"patt{h}_{ib}") for ib in range(IC)]
            Xs = Xs0 if h == 0 else emit_masks(h, f2col[:, :, h],
                                               f2colK[:, :, h], pat)
            emit_mms(patts,
                     Xs,
                     lambda jt, h=h: whB[:, jt, h, :],
                     lambda jt, h=h: whD2[:, jt, h * E:(h + 1) * E])
            for ib in range(IC):
                k = ep_eng[0] % 2
                ep_eng[0] += 1
                if k == 0:
                    nc.vector.tensor_copy(st_all[:, ib, h, :, :],
                                          patts[ib][:])
                else:
                    nc.scalar.copy(st_all[:, ib, h, :, :], patts[ib][:])
            if h < IC:
                emit_q(h)

        l1_epilogue(0, H // 2, 0)
        l1_epilogue(H // 2, H, 1)


        # ELU: elu(x) = max(x, min(exp(x)-1, 0))
        etile = wk1.tile([P, IC, HD], fp32, tag="etile")
        nc.scalar.activation(etile[:], hpre[:], AF.Exp, bias=0.0, scale=1.0)
        em = wk1.tile([P, IC, HD], fp32, tag="em")
        nc.vector.tensor_scalar(em[:], etile[:], 1.0, 0.0,
                                op0=ALU.subtract, op1=ALU.min)
        nc.vector.tensor_tensor(h_sb[:], hpre[:], em[:], op=ALU.max)

        # ---- layer 2 ----
        # transpose h blocks; g = h @ [Wout|w2o|w1o]
        g_loc = consts.tile([P, IC, C + 2], fp32)
        for ib in range(IC):
            pt = psB.tile([P, P], bf16, tag="ep", name="pth")
            nc.tensor.transpose(pt[:], h_sb[:, ib, :], identb_sb[:])
            if ib % 2 == 0:
                nc.scalar.copy(hT_sb[:, ib, :], pt[:])
            else:
                nc.vector.tensor_copy(hT_sb[:, ib, :], pt[:])
            pg = psB.tile([P, C + 2], fp32, tag="ep", name="pg")
            nc.tensor.matmul(pg[:], lhsT=hT_sb[:, ib, :], rhs=Wo2_sb[:],
                             start=True, stop=True)
            if ib % 2 == 0:
                nc.vector.tensor_copy(g_loc[:, ib, :], pg[:])
            else:
                nc.scalar.copy(g_loc[:, ib, :], pg[:])

        # gather g = [Wh2 | f2o] for all nodes
        gsrc = dram.tile([R, C + 1], fp32)
        nc.sync.dma_start(
            gsrc[:].rearrange("(c p) e -> p c e", p=P),
            g_loc[:, :, 0:C + 1])
        gdst = dram.tile([n, C + 1], fp32)
        if no_collective:
            # flat stand-in copies (same bytes as the AllGather), spread
            # over the three DMA queues
            for k in range(ncores):
                qeng[k % len(qeng)].dma_start(
                    gdst[k * R:(k + 1) * R, :], gsrc[:])
        else:
            nc.gpsimd.collective_compute(
                "AllGather", ALU.bypass,
                replica_groups=[list(range(ncores))],
                ins=[gsrc.opt()], outs=[gdst.opt()])

        g17_sb = consts.tile([P, JT, C + 1], fp32)
        for k in range(ncores):
            qeng[k % len(qeng)].dma_start(
                g17_sb[:, k * IC:(k + 1) * IC, :],
                gdst[k * R:(k + 1) * R, :]
                .rearrange("(t p) e -> p t e", p=P))
        g2_sb = g17_sb[:, :, 0:C]
        f2ocol = g17_sb[:, :, C:C + 1]

        # f1o row: w1o^T @ hT blocks
        pf1o = psB.tile([1, IC, P], fp32, tag="ep", name="pf1o")
        for ib in range(IC):
            nc.tensor.matmul(pf1o[:, ib, :], lhsT=w1o_sb[:],
                             rhs=hT_sb[:, ib, :], start=True, stop=True)
        f1orow = consts.tile([1, R], fp32)
        nc.vector.tensor_copy(f1orow[:],
                              pf1o[:].rearrange("o c p -> o (c p)"))
        f1orow_bf = consts.tile([1, R], bf16)
        nc.vector.tensor_copy(f1orow_bf[:], f1orow[:])
        pf1obf = psA.tile([P, PSW], fp32, tag="ps", name="pf1obf")
        pf1ob = pf1obf[:, 0:R]
        nc.tensor.matmul(pf1ob[:], lhsT=onesb[:], rhs=f1orow_bf[:],
                         start=True, stop=True)
        nc.scalar.copy(f1b_all[:, U - 1, :], pf1ob[:])

        f1omax = consts.tile([1, 1], fp32)
        nc.vector.tensor_reduce(f1omax[:], f1orow[:], axis=AX.X, op=ALU.max)
        b2_cols = consts.tile([1, NBIAS], fp32)
        nc.vector.memset(b2_cols[:, BB:BB + 1], -FSLACK)
        nc.vector.tensor_scalar(b2_cols[:, BD:BD + 1], f1omax[:], -0.8,
                                -FSLACK, op0=ALU.mult, op1=ALU.add)
        nc.vector.tensor_scalar_mul(b2_cols[:, BW:BW + 1], f1omax[:], 0.8)
        pb2 = psB.tile([P, NBIAS], fp32, tag="ep", name="pb2")
        nc.tensor.matmul(pb2[:], lhsT=onesf[:], rhs=b2_cols[:],
                         start=True, stop=True)
        bias2 = consts.tile([P, NBIAS], fp32)
        nc.vector.tensor_copy(bias2[:], pb2[:])

        bcol2 = consts.tile([P, JT, 1], bf16)
        dcol2 = consts.tile([P, JT, 1], bf16)
        f2oK = consts.tile([P, JT, 1], fp32)
        nc.scalar.activation(bcol2[:], f2ocol[:], AF.Exp,
                             bias=bias2[:, BB:BB + 1], scale=1.0)
        nc.scalar.activation(dcol2[:], f2ocol[:], AF.Exp,
                             bias=bias2[:, BD:BD + 1], scale=ALPHA)
        nc.scalar.activation(w_col[:, :, U - 1], g_loc[:, :, C + 1], AF.Exp,
                             bias=bias2[:, BW:BW + 1], scale=-0.8)
        nc.vector.tensor_scalar_mul(f2oK[:], f2ocol[:], KSTEEP)
        # dummy Ln: pulls the natural_log_exp table load (which also covers
        # the final Exp) off the critical softmax chain
        lnw = wk1.tile([1, 1], fp32, tag="lnw")
        nc.scalar.activation(lnw[:], onesf[0:1, 0:1], AF.Ln, bias=0.0,
                             scale=1.0)

        scale_bd(U - 1, U, lambda c: g2_sb[:], bcol2, dcol2)

        # Q for unit 8
        q2sb = consts.tile([P, IC, E], fp32)
        for ib in range(IC):
            pqf = psQ.tile([P, H * E], fp32, tag="psq", name="pqf")
            pq2 = pqf[:, 0:E]
            for jt in range(JT):
                nc.tensor.matmul(pq2[:],
                                 lhsT=adjT[:, jt, ib * P:(ib + 1) * P],
                                 rhs=whD2[:, jt, H * E:U * E],
                                 start=(jt == 0), stop=(jt == JT - 1))
            nc.vector.tensor_copy(q2sb[:, ib, :], pq2[:])

        patts2 = [psATT.tile([P, 2, E], fp32, tag="att",
                             name=f"patt2_{ib}") for ib in range(IC)]
        Xs2 = emit_masks(U - 1, f2ocol[:, :, 0], f2oK[:, :, 0], pat2)
        emit_mms(patts2, Xs2,
                 lambda jt: whB[:, jt, U - 1, :],
                 lambda jt: whD2[:, jt, (U - 1) * E:U * E])
        st2 = work.tile([P, IC, 2, E], fp32, tag="st2")
        for ib in range(IC):
            if ib % 2 == 0:
                nc.vector.tensor_copy(st2[:, ib, :, :], patts2[ib][:])
            else:
                nc.scalar.copy(st2[:, ib, :, :], patts2[ib][:])
        x2 = wk1.tile([P, IC, E], fp32, tag="x2")
        nc.vector.tensor_tensor(x2[:], q2sb[:], st2[:, :, 1, :],
                                op=ALU.subtract)
        v2 = wk1.tile([P, IC, E], fp32, tag="v2")
        a0, a1 = broadcast_tensor_aps(
            x2[:], w_col[:, :, U - 1].rearrange("p (c o) -> p c o", o=1))
        nc.vector.tensor_tensor(v2[:], a0, a1, op=ALU.mult)
        t2 = wk1.tile([P, IC, E], fp32, tag="t2")
        nc.vector.tensor_tensor(t2[:], st2[:, :, 0, :], v2[:], op=ALU.add)
        rec2 = wk1.tile([P, IC, 1], fp32, tag="rec2")
        nc.vector.reciprocal(rec2[:], t2[:, :, D:E])
        z = wk1.tile([P, IC, C], fp32, tag="z")
        a0, a1 = broadcast_tensor_aps(t2[:, :, 0:D], rec2[:])
        nc.vector.tensor_tensor(z[:], a0, a1, op=ALU.mult)

        # log_softmax along free dim (no transposes needed)
        negmx = wk1.tile([P, IC, 1], fp32, tag="negmx")
        nc.vector.tensor_reduce(negmx[:], z[:], axis=AX.X, op=ALU.max,
                                negate=True)
        zs = wk1.tile([P, IC, C], fp32, tag="zs")
        a0, a1 = broadcast_tensor_aps(z[:], negmx[:])
        nc.vector.tensor_tensor(zs[:], a0, a1, op=ALU.add)
        ez = wk1.tile([P, IC, C], fp32, tag="ez")
        nc.scalar.activation(ez[:], zs[:], AF.Exp, bias=0.0, scale=1.0)
        sume = wk1.tile([P, IC, 1], fp32, tag="sume")
        nc.vector.tensor_reduce(sume[:], ez[:], axis=AX.X, op=ALU.add)
        lns = wk1.tile([P, IC, 1], fp32, tag="lns")
        nc.scalar.activation(lns[:], sume[:], AF.Ln, bias=0.0, scale=1.0)
        zo = wk1.tile([P, IC, C], fp32, tag="zo")
        a0, a1 = broadcast_tensor_aps(zs[:], lns[:])
        nc.vector.tensor_tensor(zo[:], a0, a1, op=ALU.subtract)
        nc.sync.dma_start(out.rearrange("(c p) e -> p c e", p=P), zo[:])

    nc.compile()
    return nc


def prep_inputs(x, adj, W1, a1, Wout, a_out, n=4096, ncores=NCORES):
    """Host-side prep: slice + transpose + bf16 cast + weight folds."""
    R = n // ncores
    x = np.asarray(x, np.float32)
    adj = np.asarray(adj)
    W1 = np.asarray(W1, np.float32)
    a1 = np.asarray(a1, np.float32)
    Wout = np.asarray(Wout, np.float32)
    a_out = np.asarray(a_out, np.float32)

    xT = np.ascontiguousarray(x.T).astype(BF16)
    W1a = W1.transpose(1, 0, 2).reshape(F, H * D)
    w2c = np.einsum("hfd,hd->fh", W1, a1[:, D:])
    w1c = np.einsum("hfd,hd->fh", W1, a1[:, :D])
    w2o = Wout @ a_out[C:]
    w1o = Wout @ a_out[:C]
    Wo2 = np.ascontiguousarray(np.concatenate(
        [Wout, w2o[:, None], w1o[:, None]], axis=1)).astype(BF16)
    identf = np.eye(P, dtype=np.float32)

    adjf = adj.astype(np.float32)
    in_maps = []
    for k in range(ncores):
        rows = slice(k * R, (k + 1) * R)
        xwk = np.concatenate([x[rows].T, w2c, W1a, w1c], axis=1)
        JTl = n // P
        at = adjf[rows].astype(BF16).T                     # [n, R]
        at = np.ascontiguousarray(
            at.reshape(JTl, P, R).transpose(1, 0, 2)).reshape(P, JTl * R)
        in_maps.append({
            "xT": xT,
            "xw": np.ascontiguousarray(xwk).astype(BF16),
            "adjt": at,
            "Wo2": Wo2,
            "identf": identf,
        })
    return in_maps


_cached = {}


def kernel(x, adj, W1, a1, Wout, a_out):
    n = x.shape[0]
    if n not in _cached:
        _cached[n] = build_gat(n=n)
    nc = _cached[n]
    in_maps = prep_inputs(x, adj, W1, a1, Wout, a_out, n=n)
    res = run_bass_kernel_spmd(nc, in_maps, core_ids=list(range(NCORES)))
    outs = [res.results[k]["out"] for k in range(NCORES)]
    return np.concatenate(outs, axis=0)


# revision 33
# speedup vs baseline: 1.4640x; 1.0108x over previous
"""GAT (2-layer graph attention network) Trainium2 Bass kernel — v3.

N=4096 nodes, F=512 feats; layer1: 8 heads x 16 (ELU, concat); layer2:
1 head 128->16; log_softmax. Dense masked attention, row-parallel over
8 cores (core k owns rows [512k, 512k+512)).

Score restructure: leaky(s) = max(s, 0.2s) with s = f1[i]+f2[j] splits
softmax numerators into two rank-separable branches selected by
S = [s >= 0]:
  P = exp(leaky(s)-M)*adj = S*adj*A[i]*B[j] + (1-S)*adj*C[i]*D[j]
with A=exp(f1-f1max), B=exp(f2+f1max-M), C=exp(.2(f1-f1max)),
D=exp(.2 f2+.2 f1max-M), M = f1max + FSLACK.  The constant-slack M
(instead of f1max+f2max) needs no global f2 reduction, so the bias /
exp / scale chain leaves the critical path; the uniform exp(-FSLACK)
factor cancels in the softmax division.

Masks: a tile's 0/1 mask X = S*adj is a first op producing S
(DVE tensor_scalar 4x is_ge, or ACT steep Sigmoid) followed by a
batched multiply against adjT (DVE 2x or Pool), spread across the
three engines by a tunable per-batch pattern (kinds: A=ACT+DVE,
B=ACT+Pool, V=DVE+DVE, W=DVE+Pool).

Attention matmuls are FLIPPED: X[j, i-block] is the PE stationary
operand and the scaled factors whB/whD [j, 17] are the moving operand
(out free size 17, not 512); each (unit, i-block) accumulation chain
owns one PSUM bank, B/D halves share the chain.  Epilogue scalars
(w[i], 1/den) are per-partition; log_softmax needs no transposes.
The complement branch Q = sum_j adj*D*Wh uses lhsT=adjT directly.
Layer 2 gathers
the f2o column separately from the Wh2 block so unit-8 mask compute
overlaps the main gather.
"""

import os
import sys
import contextlib

for _p in ("/opt/trn_rl_repo",):
    if _p not in sys.path and os.path.isdir(_p):
        sys.path.insert(0, _p)

import numpy as np
import ml_dtypes

import concourse.bass as bass
import concourse.bacc as bacc
import concourse.tile as tile
from concourse import mybir
from concourse.bass import broadcast_tensor_aps
from concourse.bass_utils import run_bass_kernel_spmd

BF16 = ml_dtypes.bfloat16
ALPHA = 0.2

F = 512      # input features
H = 8        # heads (layer 1)
D = 16       # per-head dim
C = 16       # classes
P = 128      # partitions
NCORES = 8
E = D + 1    # Wh columns + ones column (17)
U = H + 1    # units: 8 layer-1 heads + 1 layer-2 head
HD = H * D   # 128

BB, BD, BW = 0, 1, 2   # bias cols: -FSLACK, -.8*f1max-FSLACK, .8*f1max
NBIAS = 3
KSTEEP = 1024.0        # sigmoid / score steepness
FSLACK = 30.0          # constant softmax shift slack (M = f1max + FSLACK)


def build_gat(n=4096, ncores=NCORES, dbg=False, no_collective=False,
              pat=("A", "V", "B", "A", "V", "V", "A", "V"),
              pat0=("V", "W", "V", "V", "V", "V", "W", "V"),
              pat2=("V", "V", "V", "W", "V", "V", "V", "V"),
              ttbufs=12, jb=4):
    """Build the SPMD Bass program for one core (row-parallel)."""
    R = n // ncores          # rows per core
    IC = R // P              # i-blocks per core
    JT = n // P              # j-tiles (partition tiles of full node dim)
    FC = F // P              # f chunks
    JB = jb                  # j-tiles per mask batch
    NB = JT // JB
    NCH = n // FC            # xT column-chunk width
    RHSW = H + HD            # fused f2|Wh matmul width (136)
    XWC = R + RHSW + H       # combined xTm|rhs|w1c load width
    PSW = max(R, RHSW)       # shared psA slot width
    assert R % P == 0 and JT % JB == 0

    fp32 = mybir.dt.float32
    bf16 = mybir.dt.bfloat16

    nc = bacc.Bacc("TRN2", target_bir_lowering=False, debug=dbg,
                   num_devices=ncores)

    xT = nc.dram_tensor("xT", [F, n], bf16, kind="ExternalInput").ap()
    xw = nc.dram_tensor("xw", [F, XWC], bf16, kind="ExternalInput").ap()
    adjt = nc.dram_tensor("adjt", [P, JT * R], bf16,
                          kind="ExternalInput").ap()
    Wo2 = nc.dram_tensor("Wo2", [HD, C + 2], bf16, kind="ExternalInput").ap()
    identf = nc.dram_tensor("identf", [P, P], fp32, kind="ExternalInput").ap()
    out = nc.dram_tensor("out", [R, C], fp32, kind="ExternalOutput").ap()

    AF = mybir.ActivationFunctionType
    ALU = mybir.AluOpType
    AX = mybir.AxisListType

    with tile.TileContext(nc) as tc, contextlib.ExitStack() as ctx:
        big = ctx.enter_context(tc.tile_pool(name="big", bufs=1))
        consts = ctx.enter_context(tc.tile_pool(name="consts", bufs=1))
        work = ctx.enter_context(tc.tile_pool(name="work", bufs=2))
        wk1 = ctx.enter_context(tc.tile_pool(name="wk1", bufs=1))
        sc_t = ctx.enter_context(tc.tile_pool(name="sc_t", bufs=ttbufs))
        psA = ctx.enter_context(tc.tile_pool(name="psA", bufs=2, space="PSUM"))
        psQ = ctx.enter_context(tc.tile_pool(name="psQ", bufs=1, space="PSUM"))
        psATT = ctx.enter_context(
            tc.tile_pool(name="psATT", bufs=4, space="PSUM"))
        psB = ctx.enter_context(tc.tile_pool(name="psB", bufs=1, space="PSUM"))
        dram = ctx.enter_context(tc.tile_pool(name="dram", bufs=1,
                                              space="DRAM"))

        # ---- const / persistent loads (spread across DMA queues).
        # xT is loaded in COLUMN chunks so pf12 chain jt can close as soon
        # as the chunk holding its columns lands.
        # big loads ride the SP + Pool DMA queues only; the ACT queue is
        # reserved for small latency-critical transfers (Wo2, f1row_1,
        # bias_row, the layer-2 gathers)
        bigq = [nc.sync, nc.gpsimd]
        qeng = [nc.sync, nc.scalar, nc.gpsimd]
        identf_sb = consts.tile([P, P], fp32)
        nc.gpsimd.dma_start(identf_sb[:], identf)
        xw_sb = consts.tile([P, FC, XWC], bf16)
        nc.sync.dma_start(xw_sb[:], xw.rearrange("(c p) n -> p c n", p=P))
        xT_sb = big.tile([P, FC, n], bf16, tag="xtslot")
        adjT = big.tile([P, JT, R], bf16)
        ACH = JT // 4
        Wo2_sb = consts.tile([P, C + 2], bf16)
        nc.scalar.dma_start(Wo2_sb[:], Wo2)
        # interleave xT column chunks with adjBG chunks: pf12 chain jt and
        # the mask tiles for jt both become runnable ~3us after their chunk
        for c in range(4):
            bigq[c % 2].dma_start(
                xT_sb[:, :, c * NCH:(c + 1) * NCH],
                xT.rearrange("(c p) n -> p c n", p=P)
                [:, :, c * NCH:(c + 1) * NCH])
            bigq[(c + 1) % 2].dma_start(
                adjT[:, c * ACH:(c + 1) * ACH, :],
                adjt[:, c * ACH * R:(c + 1) * ACH * R]
                .rearrange("p (t r) -> p t r", r=R))
        xTm_sb = xw_sb[:, :, 0:R]
        rhs_sb = xw_sb[:, :, R:R + RHSW]          # [w2c(8) | W1a(128)]
        w1c_sb = xw_sb[:, :, R + RHSW:XWC]
        w1o_sb = Wo2_sb[:, C + 1:C + 2]

        identb_sb = consts.tile([P, P], bf16)
        onesb = consts.tile([1, P], bf16)
        nc.vector.memset(onesb[:], 1.0)
        onesf = consts.tile([1, P], fp32)
        nc.vector.memset(onesf[:], 1.0)
        onescol = consts.tile([P, 1], bf16)
        nc.vector.memset(onescol[:], 1.0)

        # persistent intermediates
        whaug = big.tile([P, JT, HD], bf16)       # Wh columns per j
        whB = big.tile([P, JT, U, E], bf16)       # B_j*[Wh_u|1]
        whD2 = big.tile([P, JT, U * E], bf16)     # D_j*[Wh_u|1], (u,e)-contig
        f1b_all = big.tile([P, U, R], bf16)       # f1[i] bcast on partitions
        f2col = big.tile([P, JT, H], fp32)        # f2[j] (layer 1)
        f2colK = big.tile([P, JT, H], fp32)       # KSTEEP*f2[j]
        bcol = consts.tile([P, JT, H], bf16)
        dcol = consts.tile([P, JT, H], bf16)
        w_col = consts.tile([P, IC, U], fp32)     # exp(.8(f1max-f1[i]))
        hpre = big.tile([P, IC, HD], fp32)        # layer-1 out pre-ELU
        h_sb = big.tile([P, IC, HD], bf16)        # post-ELU
        hT_sb = big.tile([P, IC, P], bf16)        # transposed h blocks
        qsb = consts.tile([P, IC, H * E], fp32)   # Q for layer-1 units
        st_all = big.tile([P, IC, H, 2, E], fp32)  # drained attention psums
        f1colT = consts.tile([P, IC, H], fp32)    # f1[i] per-partition

        # ---- phase 1a: f1 row (own rows) ----
        pf1 = psB.tile([H, R], fp32, tag="ep")
        for fc in range(FC):
            nc.tensor.matmul(pf1[:], lhsT=w1c_sb[:, fc, :],
                             rhs=xTm_sb[:, fc, :],
                             start=(fc == 0), stop=(fc == FC - 1))
        f1row_sb = consts.tile([H, R], fp32)
        nc.vector.tensor_copy(f1row_sb[:], pf1[:])
        f1row_bf = consts.tile([H, R], bf16)
        nc.vector.tensor_copy(f1row_bf[:], f1row_sb[:])
        f1row_1 = consts.tile([1, H, R], bf16)
        nc.scalar.dma_start(f1row_1[:], f1row_bf[:])
        f1max = consts.tile([H, 1], fp32)
        nc.vector.tensor_reduce(f1max[:], f1row_sb[:], axis=AX.X, op=ALU.max)

        # f1 broadcast for head 0 first (gates the first mask tiles)
        def f1b_bcast(h):
            pbf = psA.tile([P, PSW], fp32, tag="ps", name="pbf")
            pb = pbf[:, 0:R]
            nc.tensor.matmul(pb[:], lhsT=onesb[:],
                             rhs=f1row_1[0:1, h, :], start=True, stop=True)
            nc.scalar.copy(f1b_all[:, h, :], pb[:])
        f1b_bcast(0)

        # ---- phase 1b: fused f2|Wh matmuls over all j ----
        for jt in range(JT):
            pf12f = psA.tile([P, PSW], fp32, tag="ps", name="pf12f")
            pf12 = pf12f[:, 0:RHSW]
            for fc in range(FC):
                nc.tensor.matmul(
                    pf12[:],
                    lhsT=xT_sb[:, fc, jt * P:(jt + 1) * P],
                    rhs=rhs_sb[:, fc, :],
                    start=(fc == 0), stop=(fc == FC - 1))
            nc.vector.tensor_copy(f2col[:, jt, :], pf12[:, 0:H])
            nc.scalar.copy(whaug[:, jt, :], pf12[:, H:RHSW])

        for h in range(1, H):
            f1b_bcast(h)

        # f1 per-partition columns (for epilogue w)
        nc.vector.tensor_copy(identb_sb[:], identf_sb[:])
        for ib in range(IC):
            pt = psB.tile([P, H], bf16, tag="ep", name="pt")
            nc.tensor.transpose(pt[:], f1row_bf[:, ib * P:(ib + 1) * P],
                                identb_sb[0:H, 0:H])
            nc.vector.tensor_copy(f1colT[:, ib, :], pt[:])

        # ---- mask emitter: per-batch engine pattern.
        # 'A': 4 ACT sigmoids + DVE is_gt; 'V': 4 DVE 4x tensor_scalar +
        # DVE is_gt; 'P': 4 Pool fused stt.
        bat_i = [0]

        def emit_masks(u, f2c, f2cK, patv):
            Xs = []
            for b in range(NB):
                jt0 = b * JB
                kind = patv[bat_i[0] % len(patv)]
                bat_i[0] += 1
                X = sc_t.tile([P, JB, R], bf16, tag="xs")
                for q in range(JB):
                    jt = jt0 + q
                    if kind in ("A", "B"):
                        nc.scalar.activation(
                            X[:, q, :], f1b_all[:, u, :], AF.Sigmoid,
                            bias=f2cK[:, jt:jt + 1], scale=KSTEEP)
                    else:
                        nc.vector.tensor_scalar(
                            X[:, q, :], f1b_all[:, u, :],
                            f2c[:, jt:jt + 1], 0.0,
                            op0=ALU.add, op1=ALU.is_ge)
                eng2 = nc.gpsimd if kind in ("B", "W") else nc.vector
                eng2.tensor_tensor(
                    X[:], X[:], adjT[:, jt0:jt0 + JB, :], op=ALU.mult)
                Xs.append(X)
            return Xs

        # attention matmuls (flipped): X is the stationary operand; each
        # (unit, iblock) chain owns one PSUM bank, B/D halves share it.
        def emit_mms(patts, Xs, whB_u, whD_u):
            for b in range(NB):
                X = Xs[b]
                for q in range(JB):
                    jt = b * JB + q
                    for ib in range(IC):
                        lw = X[:, q, ib * P:(ib + 1) * P]
                        nc.tensor.matmul(
                            patts[ib][:, 0, :], lhsT=lw, rhs=whB_u(jt),
                            start=(jt == 0), stop=False)
                        nc.tensor.matmul(
                            patts[ib][:, 1, :], lhsT=lw, rhs=whD_u(jt),
                            start=False, stop=(jt == JT - 1))

        # masks for unit 0 go out BEFORE the scale chain so DVE/ACT/Pool
        # fill buffers while whB/whD are being scaled
        Xs0 = emit_masks(0, f2col[:, :, 0], f2colK[:, :, 0], pat0)

        # biases off f1max only (constant-slack M): BD = -.8f1max-FSLACK,
        # BW = .8f1max; the B-column bias is the constant -FSLACK.
        bias_cols = consts.tile([H, NBIAS], fp32)
        nc.vector.memset(bias_cols[:, BB:BB + 1], -FSLACK)
        nc.vector.tensor_scalar(bias_cols[:, BD:BD + 1], f1max[:], -0.8,
                                -FSLACK, op0=ALU.mult, op1=ALU.add)
        nc.vector.tensor_scalar_mul(bias_cols[:, BW:BW + 1], f1max[:], 0.8)
        bias_row = consts.tile([1, H, NBIAS], fp32)
        nc.scalar.dma_start(bias_row[:], bias_cols[:])
        pba = psB.tile([P, H * NBIAS], fp32, tag="ep", name="pba")
        nc.tensor.matmul(pba[:], lhsT=onesf[:],
                         rhs=bias_row[:].rearrange("o h e -> o (h e)"),
                         start=True, stop=True)
        bias_all = consts.tile([P, H, NBIAS], fp32)
        nc.vector.tensor_copy(
            bias_all[:], pba[:].rearrange("p (h e) -> p h e", e=NBIAS))
        nc.vector.tensor_scalar_mul(f2colK[:], f2col[:], KSTEEP)

        # exps (ACT): B/D columns + per-partition w columns
        for h in range(H):
            nc.scalar.activation(bcol[:, :, h], f2col[:, :, h], AF.Exp,
                                 bias=bias_all[:, h, BB:BB + 1], scale=1.0)
            nc.scalar.activation(dcol[:, :, h], f2col[:, :, h], AF.Exp,
                                 bias=bias_all[:, h, BD:BD + 1], scale=ALPHA)
            nc.scalar.activation(w_col[:, :, h], f1colT[:, :, h], AF.Exp,
                                 bias=bias_all[:, h, BW:BW + 1], scale=-0.8)


        # scale whaug into whB / whD2 (broadcast B/D along e):
        # B-half on DVE, D-half on Pool
        whD2v = whD2[:].rearrange("p t (u e) -> p t u e", e=E)

        def scale_bd(u0, u1, src, bcs, dcs):
            for u in range(u0, u1):
                s = src(u - u0)
                a0, a1 = broadcast_tensor_aps(s, bcs[:, :, u - u0:u - u0 + 1])
                nc.vector.tensor_tensor(whB[:, :, u, 0:D], a0, a1,
                                        op=ALU.mult)
                a0, a1 = broadcast_tensor_aps(s, dcs[:, :, u - u0:u - u0 + 1])
                nc.gpsimd.tensor_tensor(whD2v[:, :, u, 0:D], a0, a1,
                                        op=ALU.mult)
            nc.vector.tensor_copy(
                whB[:, :, u0:u1, D:E],
                bcs[:].rearrange("p t (c o) -> p t c o", o=1))
            nc.vector.tensor_copy(
                whD2v[:, :, u0:u1, D:E],
                dcs[:].rearrange("p t (c o) -> p t c o", o=1))

        scale_bd(0, H, lambda c: whaug[:, :, c * D:(c + 1) * D], bcol, dcol)

        # ---- Q: complement branch, lhsT = adjT directly
        def emit_q(ib):
            pq = psQ.tile([P, H * E], fp32, tag="psq", name="pq")
            for jt in range(JT):
                nc.tensor.matmul(pq[:],
                                 lhsT=adjT[:, jt, ib * P:(ib + 1) * P],
                                 rhs=whD2[:, jt, 0:H * E],
                                 start=(jt == 0), stop=(jt == JT - 1))
            nc.scalar.copy(qsb[:, ib, :], pq[:])

        # Q = colsum - QBG/BIG; batched epilogue halves (first half is
        # emitted mid-loop so it overlaps units 5..7):
        # t = stB + w[i]*(Q - stD); h = t[:,0:16]/t[:,16]
        qv = qsb[:].rearrange("p c (u e) -> p c u e", e=E)
        hprev = hpre[:].rearrange("p c (u d) -> p c u d", d=D)


        def l1_epilogue(u0, u1, s):
            uu = u1 - u0
            x1a = wk1.tile([P, IC, uu, E], fp32, tag=f"x1a{s}")
            nc.vector.tensor_tensor(x1a[:], qv[:, :, u0:u1, :],
                                    st_all[:, :, u0:u1, 1, :],
                                    op=ALU.subtract)
            va = wk1.tile([P, IC, uu, E], fp32, tag=f"va{s}")
            a0, a1 = broadcast_tensor_aps(
                x1a[:],
                w_col[:, :, u0:u1].rearrange("p c (u o) -> p c u o", o=1))
            nc.vector.tensor_tensor(va[:], a0, a1, op=ALU.mult)
            ta = wk1.tile([P, IC, uu, E], fp32, tag=f"ta{s}")
            nc.vector.tensor_tensor(ta[:], st_all[:, :, u0:u1, 0, :], va[:],
                                    op=ALU.add)
            reca = wk1.tile([P, IC, uu, 1], fp32, tag=f"reca{s}")
            nc.vector.reciprocal(reca[:], ta[:, :, :, D:E])
            a0, a1 = broadcast_tensor_aps(ta[:, :, :, 0:D], reca[:])
            nc.vector.tensor_tensor(hprev[:, :, u0:u1, :], a0, a1,
                                    op=ALU.mult)


        # ---- layer-1 attention ----
        ep_eng = [0]
        for h in range(H):
            patts = [psATT.tile([P, 2, E], fp32, tag="att",
                                name=f"patt{h}_{ib}") for ib in range(IC)]
            Xs = Xs0 if h == 0 else emit_masks(h, f2col[:, :, h],
                                               f2colK[:, :, h], pat)
            emit_mms(patts,
                     Xs,
                     lambda jt, h=h: whB[:, jt, h, :],
                     lambda jt, h=h: whD2[:, jt, h * E:(h + 1) * E])
            for ib in range(IC):
                k = ep_eng[0] % 2
                ep_eng[0] += 1
                if k == 0:
                    nc.vector.tensor_copy(st_all[:, ib, h, :, :],
                                          patts[ib][:])
                else:
                    nc.scalar.copy(st_all[:, ib, h, :, :], patts[ib][:])
            if h < IC:
                emit_q(h)

        l1_epilogue(0, H // 2, 0)
        l1_epilogue(H // 2, H, 1)


        # ELU: elu(x) = max(x, min(exp(x)-1, 0))
        etile = wk1.tile([P, IC, HD], fp32, tag="etile")
        nc.scalar.activation(etile[:], hpre[:], AF.Exp, bias=0.0, scale=1.0)
        em = wk1.tile([P, IC, HD], fp32, tag="em")
        nc.vector.tensor_scalar(em[:], etile[:], 1.0, 0.0,
                                op0=ALU.subtract, op1=ALU.min)
        nc.vector.tensor_tensor(h_sb[:], hpre[:], em[:], op=ALU.max)

        # ---- layer 2 ----
        # transpose h blocks; g = h @ [Wout|w2o|w1o]
        g_loc = consts.tile([P, IC, C + 2], fp32)
        for ib in range(IC):
            pt = psB.tile([P, P], bf16, tag="ep", name="pth")
            nc.tensor.transpose(pt[:], h_sb[:, ib, :], identb_sb[:])
            if ib % 2 == 0:
                nc.scalar.copy(hT_sb[:, ib, :], pt[:])
            else:
                nc.vector.tensor_copy(hT_sb[:, ib, :], pt[:])
            pg = psB.tile([P, C + 2], fp32, tag="ep", name="pg")
            nc.tensor.matmul(pg[:], lhsT=hT_sb[:, ib, :], rhs=Wo2_sb[:],
                             start=True, stop=True)
            if ib % 2 == 0:
                nc.vector.tensor_copy(g_loc[:, ib, :], pg[:])
            else:
                nc.scalar.copy(g_loc[:, ib, :], pg[:])

        # gather g = [Wh2 | f2o] for all nodes
        gsrc = dram.tile([R, C + 1], fp32)
        nc.sync.dma_start(
            gsrc[:].rearrange("(c p) e -> p c e", p=P),
            g_loc[:, :, 0:C + 1])
        gdst = dram.tile([n, C + 1], fp32)
        if no_collective:
            # flat stand-in copies (same bytes as the AllGather), spread
            # over the three DMA queues
            for k in range(ncores):
                qeng[k % len(qeng)].dma_start(
                    gdst[k * R:(k + 1) * R, :], gsrc[:])
        else:
            nc.gpsimd.collective_compute(
                "AllGather", ALU.bypass,
                replica_groups=[list(range(ncores))],
                ins=[gsrc.opt()], outs=[gdst.opt()])

        g17_sb = consts.tile([P, JT, C + 1], fp32)
        for k in range(ncores):
            qeng[k % len(qeng)].dma_start(
                g17_sb[:, k * IC:(k + 1) * IC, :],
                gdst[k * R:(k + 1) * R, :]
                .rearrange("(t p) e -> p t e", p=P))
        g2_sb = g17_sb[:, :, 0:C]
        f2ocol = g17_sb[:, :, C:C + 1]

        # f1o row: w1o^T @ hT blocks
        pf1o = psB.tile([1, IC, P], fp32, tag="ep", name="pf1o")
        for ib in range(IC):
            nc.tensor.matmul(pf1o[:, ib, :], lhsT=w1o_sb[:],
                             rhs=hT_sb[:, ib, :], start=True, stop=True)
        f1orow = consts.tile([1, R], fp32)
        nc.vector.tensor_copy(f1orow[:],
                              pf1o[:].rearrange("o c p -> o (c p)"))
        f1orow_bf = consts.tile([1, R], bf16)
        nc.vector.tensor_copy(f1orow_bf[:], f1orow[:])
        pf1obf = psA.tile([P, PSW], fp32, tag="ps", name="pf1obf")
        pf1ob = pf1obf[:, 0:R]
        nc.tensor.matmul(pf1ob[:], lhsT=onesb[:], rhs=f1orow_bf[:],
                         start=True, stop=True)
        nc.scalar.copy(f1b_all[:, U - 1, :], pf1ob[:])

        f1omax = consts.tile([1, 1], fp32)
        nc.vector.tensor_reduce(f1omax[:], f1orow[:], axis=AX.X, op=ALU.max)
        b2_cols = consts.tile([1, NBIAS], fp32)
        nc.vector.memset(b2_cols[:, BB:BB + 1], -FSLACK)
        nc.vector.tensor_scalar(b2_cols[:, BD:BD + 1], f1omax[:], -0.8,
                                -FSLACK, op0=ALU.mult, op1=ALU.add)
        nc.vector.tensor_scalar_mul(b2_cols[:, BW:BW + 1], f1omax[:], 0.8)
        pb2 = psB.tile([P, NBIAS], fp32, tag="ep", name="pb2")
        nc.tensor.matmul(pb2[:], lhsT=onesf[:], rhs=b2_cols[:],
                         start=True, stop=True)
        bias2 = consts.tile([P, NBIAS], fp32)
        nc.vector.tensor_copy(bias2[:], pb2[:])

        bcol2 = consts.tile([P, JT, 1], bf16)
        dcol2 = consts.tile([P, JT, 1], bf16)
        f2oK = consts.tile([P, JT, 1], fp32)
        nc.scalar.activation(bcol2[:], f2ocol[:], AF.Exp,
                             bias=bias2[:, BB:BB + 1], scale=1.0)
        nc.scalar.activation(dcol2[:], f2ocol[:], AF.Exp,
                             bias=bias2[:, BD:BD + 1], scale=ALPHA)
        nc.scalar.activation(w_col[:, :, U - 1], g_loc[:, :, C + 1], AF.Exp,
                             bias=bias2[:, BW:BW + 1], scale=-0.8)
        nc.vector.tensor_scalar_mul(f2oK[:], f2ocol[:], KSTEEP)
        # dummy Ln: pulls the natural_log_exp table load (which also covers
        # the final Exp) off the critical softmax chain
        lnw = wk1.tile([1, 1], fp32, tag="lnw")
        nc.scalar.activation(lnw[:], onesf[0:1, 0:1], AF.Ln, bias=0.0,
                             scale=1.0)

        scale_bd(U - 1, U, lambda c: g2_sb[:], bcol2, dcol2)

        # Q for unit 8
        q2sb = consts.tile([P, IC, E], fp32)
        for ib in range(IC):
            pqf = psQ.tile([P, H * E], fp32, tag="psq", name="pqf")
            pq2 = pqf[:, 0:E]
            for jt in range(JT):
                nc.tensor.matmul(pq2[:],
                                 lhsT=adjT[:, jt, ib * P:(ib + 1) * P],
                                 rhs=whD2[:, jt, H * E:U * E],
                                 start=(jt == 0), stop=(jt == JT - 1))
            nc.vector.tensor_copy(q2sb[:, ib, :], pq2[:])

        patts2 = [psATT.tile([P, 2, E], fp32, tag="att",
                             name=f"patt2_{ib}") for ib in range(IC)]
        Xs2 = emit_masks(U - 1, f2ocol[:, :, 0], f2oK[:, :, 0], pat2)
        emit_mms(patts2, Xs2,
                 lambda jt: whB[:, jt, U - 1, :],
                 lambda jt: whD2[:, jt, (U - 1) * E:U * E])
        st2 = work.tile([P, IC, 2, E], fp32, tag="st2")
        for ib in range(IC):
            if ib % 2 == 0:
                nc.vector.tensor_copy(st2[:, ib, :, :], patts2[ib][:])
            else:
                nc.scalar.copy(st2[:, ib, :, :], patts2[ib][:])
        x2 = wk1.tile([P, IC, E], fp32, tag="x2")
        nc.vector.tensor_tensor(x2[:], q2sb[:], st2[:, :, 1, :],
                                op=ALU.subtract)
        v2 = wk1.tile([P, IC, E], fp32, tag="v2")
        a0, a1 = broadcast_tensor_aps(
            x2[:], w_col[:, :, U - 1].rearrange("p (c o) -> p c o", o=1))
        nc.vector.tensor_tensor(v2[:], a0, a1, op=ALU.mult)
        t2 = wk1.tile([P, IC, E], fp32, tag="t2")
        nc.vector.tensor_tensor(t2[:], st2[:, :, 0, :], v2[:], op=ALU.add)
        rec2 = wk1.tile([P, IC, 1], fp32, tag="rec2")
        nc.vector.reciprocal(rec2[:], t2[:, :, D:E])
        z = wk1.tile([P, IC, C], fp32, tag="z")
        a0, a1 = broadcast_tensor_aps(t2[:, :, 0:D], rec2[:])
        nc.vector.tensor_tensor(z[:], a0, a1, op=ALU.mult)

        # log_softmax along free dim (no transposes needed)
        negmx = wk1.tile([P, IC, 1], fp32, tag="negmx")
        nc.vector.tensor_reduce(negmx[:], z[:], axis=AX.X, op=ALU.max,
                                negate=True)
        zs = wk1.tile([P, IC, C], fp32, tag="zs")
        a0, a1 = broadcast_tensor_aps(z[:], negmx[:])
        nc.vector.tensor_tensor(zs[:], a0, a1, op=ALU.add)
        ez = wk1.tile([P, IC, C], fp32, tag="ez")
        nc.scalar.activation(ez[:], zs[:], AF.Exp, bias=0.0, scale=1.0)
        sume = wk1.tile([P, IC, 1], fp32, tag="sume")
        nc.vector.tensor_reduce(sume[:], ez[:], axis=AX.X, op=ALU.add)
        lns = wk1.tile([P, IC, 1], fp32, tag="lns")
        nc.scalar.activation(lns[:], sume[:], AF.Ln, bias=0.0, scale=1.0)
        zo = wk1.tile([P, IC, C], fp32, tag="zo")
        a0, a1 = broadcast_tensor_aps(zs[:], lns[:])
        nc.vector.tensor_tensor(zo[:], a0, a1, op=ALU.subtract)
        nc.sync.dma_start(out.rearrange("(c p) e -> p c e", p=P), zo[:])

    nc.compile()
    return nc


def prep_inputs(x, adj, W1, a1, Wout, a_out, n=4096, ncores=NCORES):
    """Host-side prep: slice + transpose + bf16 cast + weight folds."""
    R = n // ncores
    x = np.asarray(x, np.float32)
    adj = np.asarray(adj)
    W1 = np.asarray(W1, np.float32)
    a1 = np.asarray(a1, np.float32)
    Wout = np.asarray(Wout, np.float32)
    a_out = np.asarray(a_out, np.float32)

    xT = np.ascontiguousarray(x.T).astype(BF16)
    W1a = W1.transpose(1, 0, 2).reshape(F, H * D)
    w2c = np.einsum("hfd,hd->fh", W1, a1[:, D:])
    w1c = np.einsum("hfd,hd->fh", W1, a1[:, :D])
    w2o = Wout @ a_out[C:]
    w1o = Wout @ a_out[:C]
    Wo2 = np.ascontiguousarray(np.concatenate(
        [Wout, w2o[:, None], w1o[:, None]], axis=1)).astype(BF16)
    identf = np.eye(P, dtype=np.float32)

    adjf = adj.astype(np.float32)
    in_maps = []
    for k in range(ncores):
        rows = slice(k * R, (k + 1) * R)
        xwk = np.concatenate([x[rows].T, w2c, W1a, w1c], axis=1)
        JTl = n // P
        at = adjf[rows].astype(BF16).T                     # [n, R]
        at = np.ascontiguousarray(
            at.reshape(JTl, P, R).transpose(1, 0, 2)).reshape(P, JTl * R)
        in_maps.append({
            "xT": xT,
            "xw": np.ascontiguousarray(xwk).astype(BF16),
            "adjt": at,
            "Wo2": Wo2,
            "identf": identf,
        })
    return in_maps


_cached = {}


def kernel(x, adj, W1, a1, Wout, a_out):
    n = x.shape[0]
    if n not in _cached:
        _cached[n] = build_gat(n=n)
    nc = _cached[n]
    in_maps = prep_inputs(x, adj, W1, a1, Wout, a_out, n=n)
    res = run_bass_kernel_spmd(nc, in_maps, core_ids=list(range(NCORES)))
    outs = [res.results[k]["out"] for k in range(NCORES)]
    return np.concatenate(outs, axis=0)


# revision 39
# speedup vs baseline: 1.4983x; 1.0234x over previous
"""GAT (2-layer graph attention network) Trainium2 Bass kernel — v3.

N=4096 nodes, F=512 feats; layer1: 8 heads x 16 (ELU, concat); layer2:
1 head 128->16; log_softmax. Dense masked attention, row-parallel over
8 cores (core k owns rows [512k, 512k+512)).

Score restructure: leaky(s) = max(s, 0.2s) with s = f1[i]+f2[j] splits
softmax numerators into two rank-separable branches selected by
S = [s >= 0]:
  P = exp(leaky(s)-M)*adj = S*adj*A[i]*B[j] + (1-S)*adj*C[i]*D[j]
with A=exp(f1-f1max), B=exp(f2+f1max-M), C=exp(.2(f1-f1max)),
D=exp(.2 f2+.2 f1max-M), M = f1max + FSLACK.  The constant-slack M
(instead of f1max+f2max) needs no global f2 reduction, so the bias /
exp / scale chain leaves the critical path; the uniform exp(-FSLACK)
factor cancels in the softmax division.

Masks: a tile's 0/1 mask X = S*adj is a first op producing S
(DVE tensor_scalar 4x is_ge, or ACT steep Sigmoid) followed by a
batched multiply against adjT (DVE 2x or Pool), spread across the
three engines by a tunable per-batch pattern (kinds: A=ACT+DVE,
B=ACT+Pool, V=DVE+DVE, W=DVE+Pool).

Attention matmuls are FLIPPED: X[j, i-block] is the PE stationary
operand and the scaled factors whB/whD [j, 17] are the moving operand
(out free size 17, not 512); each (unit, i-block) accumulation chain
owns one PSUM bank, B/D halves share the chain.  Epilogue scalars
(w[i], 1/den) are per-partition; log_softmax needs no transposes.
The complement branch Q = sum_j adj*D*Wh uses lhsT=adjT directly.
Layer 2 gathers
the f2o column separately from the Wh2 block so unit-8 mask compute
overlaps the main gather.
"""

import os
import sys
import contextlib

for _p in ("/opt/trn_rl_repo",):
    if _p not in sys.path and os.path.isdir(_p):
        sys.path.insert(0, _p)

import numpy as np
import ml_dtypes

import concourse.bass as bass
import concourse.bacc as bacc
import concourse.tile as tile
from concourse import mybir
from concourse.bass import broadcast_tensor_aps
from concourse.bass_utils import run_bass_kernel_spmd

BF16 = ml_dtypes.bfloat16
ALPHA = 0.2

F = 512      # input features
H = 8        # heads (layer 1)
D = 16       # per-head dim
C = 16       # classes
P = 128      # partitions
NCORES = 8
E = D + 1    # Wh columns + ones column (17)
U = H + 1    # units: 8 layer-1 heads + 1 layer-2 head
HD = H * D   # 128

BB, BD, BW = 0, 1, 2   # bias cols: -FSLACK, -.8*f1max-FSLACK, .8*f1max
NBIAS = 3
KSTEEP = 1024.0        # sigmoid / score steepness
FSLACK = 30.0          # constant softmax shift slack (M = f1max + FSLACK)


def build_gat(n=4096, ncores=NCORES, dbg=False, no_collective=False,
              pat=("A", "V", "B", "A", "W", "V", "A", "V"),
              pat0=("V", "W", "V", "W", "V", "V", "W", "V"),
              pat2=("V", "A", "W", "V", "A", "V", "W", "A"),
              ttbufs=12, jb=4):
    """Build the SPMD Bass program for one core (row-parallel)."""
    R = n // ncores          # rows per core
    IC = R // P              # i-blocks per core
    JT = n // P              # j-tiles (partition tiles of full node dim)
    FC = F // P              # f chunks
    JB = jb                  # j-tiles per mask batch
    NB = JT // JB
    NCH = n // FC            # xT column-chunk width
    RHSW = H + HD            # fused f2|Wh matmul width (136)
    XWC = R + RHSW + H       # combined xTm|rhs|w1c load width
    PSW = max(R, RHSW)       # shared psA slot width
    assert R % P == 0 and JT % JB == 0

    fp32 = mybir.dt.float32
    bf16 = mybir.dt.bfloat16

    nc = bacc.Bacc("TRN2", target_bir_lowering=False, debug=dbg,
                   num_devices=ncores)

    xT = nc.dram_tensor("xT", [F, n], bf16, kind="ExternalInput").ap()
    xw = nc.dram_tensor("xw", [F, XWC], bf16, kind="ExternalInput").ap()
    adjt = nc.dram_tensor("adjt", [P, JT * R], bf16,
                          kind="ExternalInput").ap()
    Wo2 = nc.dram_tensor("Wo2", [HD, C + 2], bf16, kind="ExternalInput").ap()
    identf = nc.dram_tensor("identf", [P, P], fp32, kind="ExternalInput").ap()
    out = nc.dram_tensor("out", [R, C], fp32, kind="ExternalOutput").ap()

    AF = mybir.ActivationFunctionType
    ALU = mybir.AluOpType
    AX = mybir.AxisListType

    with tile.TileContext(nc) as tc, contextlib.ExitStack() as ctx:
        big = ctx.enter_context(tc.tile_pool(name="big", bufs=1))
        consts = ctx.enter_context(tc.tile_pool(name="consts", bufs=1))
        work = ctx.enter_context(tc.tile_pool(name="work", bufs=2))
        wk1 = ctx.enter_context(tc.tile_pool(name="wk1", bufs=1))
        sc_t = ctx.enter_context(tc.tile_pool(name="sc_t", bufs=ttbufs))
        psA = ctx.enter_context(tc.tile_pool(name="psA", bufs=2, space="PSUM"))
        psQ = ctx.enter_context(tc.tile_pool(name="psQ", bufs=1, space="PSUM"))
        psATT = ctx.enter_context(
            tc.tile_pool(name="psATT", bufs=4, space="PSUM"))
        psB = ctx.enter_context(tc.tile_pool(name="psB", bufs=1, space="PSUM"))
        dram = ctx.enter_context(tc.tile_pool(name="dram", bufs=1,
                                              space="DRAM"))

        # ---- const / persistent loads (spread across DMA queues).
        # xT is loaded in COLUMN chunks so pf12 chain jt can close as soon
        # as the chunk holding its columns lands.
        # big loads ride the SP + Pool DMA queues only; the ACT queue is
        # reserved for small latency-critical transfers (Wo2, f1row_1,
        # bias_row, the layer-2 gathers)
        bigq = [nc.sync, nc.gpsimd]
        qeng = [nc.sync, nc.scalar, nc.gpsimd]
        identf_sb = consts.tile([P, P], fp32)
        nc.gpsimd.dma_start(identf_sb[:], identf)
        xw_sb = consts.tile([P, FC, XWC], bf16)
        nc.sync.dma_start(xw_sb[:], xw.rearrange("(c p) n -> p c n", p=P))
        xT_sb = big.tile([P, FC, n], bf16, tag="xtslot")
        adjT = big.tile([P, JT, R], bf16)
        ACH = JT // 4
        Wo2_sb = consts.tile([P, C + 2], bf16)
        nc.scalar.dma_start(Wo2_sb[:], Wo2)
        # interleave xT column chunks with adjBG chunks: pf12 chain jt and
        # the mask tiles for jt both become runnable ~3us after their chunk
        for c in range(4):
            bigq[c % 2].dma_start(
                xT_sb[:, :, c * NCH:(c + 1) * NCH],
                xT.rearrange("(c p) n -> p c n", p=P)
                [:, :, c * NCH:(c + 1) * NCH])
            bigq[(c + 1) % 2].dma_start(
                adjT[:, c * ACH:(c + 1) * ACH, :],
                adjt[:, c * ACH * R:(c + 1) * ACH * R]
                .rearrange("p (t r) -> p t r", r=R))
        xTm_sb = xw_sb[:, :, 0:R]
        rhs_sb = xw_sb[:, :, R:R + RHSW]          # [w2c(8) | W1a(128)]
        w1c_sb = xw_sb[:, :, R + RHSW:XWC]
        w1o_sb = Wo2_sb[:, C + 1:C + 2]

        identb_sb = consts.tile([P, P], bf16)
        onesb = consts.tile([1, P], bf16)
        nc.vector.memset(onesb[:], 1.0)
        onesf = consts.tile([1, P], fp32)
        nc.vector.memset(onesf[:], 1.0)
        onescol = consts.tile([P, 1], bf16)
        nc.vector.memset(onescol[:], 1.0)

        # persistent intermediates
        whaug = big.tile([P, JT, HD], bf16)       # Wh columns per j
        whB = big.tile([P, JT, U, E], bf16)       # B_j*[Wh_u|1]
        whD2 = big.tile([P, JT, U * E], bf16)     # D_j*[Wh_u|1], (u,e)-contig
        f1b_all = big.tile([P, U, R], bf16)       # f1[i] bcast on partitions
        f2col = big.tile([P, JT, H], fp32)        # f2[j] (layer 1)
        f2colK = big.tile([P, JT, H], fp32)       # KSTEEP*f2[j]
        bcol = consts.tile([P, JT, H], bf16)
        dcol = consts.tile([P, JT, H], bf16)
        w_col = consts.tile([P, IC, U], fp32)     # exp(.8(f1max-f1[i]))
        hpre = big.tile([P, IC, HD], fp32)        # layer-1 out pre-ELU
        h_sb = big.tile([P, IC, HD], bf16)        # post-ELU
        hT_sb = big.tile([P, IC, P], bf16)        # transposed h blocks
        qsb = consts.tile([P, IC, H * E], fp32)   # Q for layer-1 units
        st_all = big.tile([P, IC, H, 2, E], fp32)  # drained attention psums
        f1colT = consts.tile([P, IC, H], fp32)    # f1[i] per-partition

        # ---- phase 1a: f1 row (own rows) ----
        pf1 = psB.tile([H, R], fp32, tag="ep")
        for fc in range(FC):
            nc.tensor.matmul(pf1[:], lhsT=w1c_sb[:, fc, :],
                             rhs=xTm_sb[:, fc, :],
                             start=(fc == 0), stop=(fc == FC - 1))
        f1row_sb = consts.tile([H, R], fp32)
        nc.vector.tensor_copy(f1row_sb[:], pf1[:])
        f1row_bf = consts.tile([H, R], bf16)
        nc.vector.tensor_copy(f1row_bf[:], f1row_sb[:])
        f1row_1 = consts.tile([1, H, R], bf16)
        nc.scalar.dma_start(f1row_1[:], f1row_bf[:])
        f1max = consts.tile([H, 1], fp32)
        nc.vector.tensor_reduce(f1max[:], f1row_sb[:], axis=AX.X, op=ALU.max)

        # f1 broadcast for head 0 first (gates the first mask tiles)
        def f1b_bcast(h):
            pbf = psA.tile([P, PSW], fp32, tag="ps", name="pbf")
            pb = pbf[:, 0:R]
            nc.tensor.matmul(pb[:], lhsT=onesb[:],
                             rhs=f1row_1[0:1, h, :], start=True, stop=True)
            nc.scalar.copy(f1b_all[:, h, :], pb[:])
        f1b_bcast(0)

        # ---- phase 1b: fused f2|Wh matmuls over all j ----
        for jt in range(JT):
            pf12f = psA.tile([P, PSW], fp32, tag="ps", name="pf12f")
            pf12 = pf12f[:, 0:RHSW]
            for fc in range(FC):
                nc.tensor.matmul(
                    pf12[:],
                    lhsT=xT_sb[:, fc, jt * P:(jt + 1) * P],
                    rhs=rhs_sb[:, fc, :],
                    start=(fc == 0), stop=(fc == FC - 1))
            nc.vector.tensor_copy(f2col[:, jt, :], pf12[:, 0:H])
            nc.scalar.copy(whaug[:, jt, :], pf12[:, H:RHSW])

        for h in range(1, H):
            f1b_bcast(h)

        # f1 per-partition columns (for epilogue w)
        nc.vector.tensor_copy(identb_sb[:], identf_sb[:])
        for ib in range(IC):
            pt = psB.tile([P, H], bf16, tag="ep", name="pt")
            nc.tensor.transpose(pt[:], f1row_bf[:, ib * P:(ib + 1) * P],
                                identb_sb[0:H, 0:H])
            nc.vector.tensor_copy(f1colT[:, ib, :], pt[:])

        # ---- mask emitter: per-batch engine pattern.
        # 'A': 4 ACT sigmoids + DVE is_gt; 'V': 4 DVE 4x tensor_scalar +
        # DVE is_gt; 'P': 4 Pool fused stt.
        bat_i = [0]

        def emit_masks(u, f2c, f2cK, patv):
            Xs = []
            for b in range(NB):
                jt0 = b * JB
                kind = patv[bat_i[0] % len(patv)]
                bat_i[0] += 1
                X = sc_t.tile([P, JB, R], bf16, tag="xs")
                for q in range(JB):
                    jt = jt0 + q
                    if kind in ("A", "B"):
                        nc.scalar.activation(
                            X[:, q, :], f1b_all[:, u, :], AF.Sigmoid,
                            bias=f2cK[:, jt:jt + 1], scale=KSTEEP)
                    else:
                        nc.vector.tensor_scalar(
                            X[:, q, :], f1b_all[:, u, :],
                            f2c[:, jt:jt + 1], 0.0,
                            op0=ALU.add, op1=ALU.is_ge)
                if kind in ("B", "W"):
                    # per-tile Pool multiplies: finer granularity releases
                    # each tile to the PE as soon as it is masked
                    for q in range(JB):
                        jt = jt0 + q
                        nc.gpsimd.tensor_tensor(
                            X[:, q, :], X[:, q, :], adjT[:, jt, :],
                            op=ALU.mult)
                else:
                    nc.vector.tensor_tensor(
                        X[:], X[:], adjT[:, jt0:jt0 + JB, :], op=ALU.mult)
                Xs.append(X)
            return Xs

        # attention matmuls (flipped): X is the stationary operand; each
        # (unit, iblock) chain owns one PSUM bank, B/D halves share it.
        def emit_mms(patts, Xs, whB_u, whD_u):
            for b in range(NB):
                X = Xs[b]
                for q in range(JB):
                    jt = b * JB + q
                    for ib in range(IC):
                        lw = X[:, q, ib * P:(ib + 1) * P]
                        nc.tensor.matmul(
                            patts[ib][:, 0, :], lhsT=lw, rhs=whB_u(jt),
                            start=(jt == 0), stop=False)
                        nc.tensor.matmul(
                            patts[ib][:, 1, :], lhsT=lw, rhs=whD_u(jt),
                            start=False, stop=(jt == JT - 1))

        # masks for unit 0 go out BEFORE the scale chain so DVE/ACT/Pool
        # fill buffers while whB/whD are being scaled
        Xs0 = emit_masks(0, f2col[:, :, 0], f2colK[:, :, 0], pat0)

        # biases off f1max only (constant-slack M): BD = -.8f1max-FSLACK,
        # BW = .8f1max; the B-column bias is the constant -FSLACK.
        bias_cols = consts.tile([H, NBIAS], fp32)
        nc.vector.memset(bias_cols[:, BB:BB + 1], -FSLACK)
        nc.vector.tensor_scalar(bias_cols[:, BD:BD + 1], f1max[:], -0.8,
                                -FSLACK, op0=ALU.mult, op1=ALU.add)
        nc.vector.tensor_scalar_mul(bias_cols[:, BW:BW + 1], f1max[:], 0.8)
        bias_row = consts.tile([1, H, NBIAS], fp32)
        nc.scalar.dma_start(bias_row[:], bias_cols[:])
        pba = psB.tile([P, H * NBIAS], fp32, tag="ep", name="pba")
        nc.tensor.matmul(pba[:], lhsT=onesf[:],
                         rhs=bias_row[:].rearrange("o h e -> o (h e)"),
                         start=True, stop=True)
        bias_all = consts.tile([P, H, NBIAS], fp32)
        nc.vector.tensor_copy(
            bias_all[:], pba[:].rearrange("p (h e) -> p h e", e=NBIAS))
        nc.vector.tensor_scalar_mul(f2colK[:], f2col[:], KSTEEP)

        # exps (ACT): B/D columns + per-partition w columns
        for h in range(H):
            nc.scalar.activation(bcol[:, :, h], f2col[:, :, h], AF.Exp,
                                 bias=bias_all[:, h, BB:BB + 1], scale=1.0)
            nc.scalar.activation(dcol[:, :, h], f2col[:, :, h], AF.Exp,
                                 bias=bias_all[:, h, BD:BD + 1], scale=ALPHA)
            nc.scalar.activation(w_col[:, :, h], f1colT[:, :, h], AF.Exp,
                                 bias=bias_all[:, h, BW:BW + 1], scale=-0.8)


        # scale whaug into whB / whD2 (broadcast B/D along e):
        # B-half on DVE, D-half on Pool
        whD2v = whD2[:].rearrange("p t (u e) -> p t u e", e=E)

        def scale_bd(u0, u1, src, bcs, dcs):
            for u in range(u0, u1):
                s = src(u - u0)
                a0, a1 = broadcast_tensor_aps(s, bcs[:, :, u - u0:u - u0 + 1])
                nc.vector.tensor_tensor(whB[:, :, u, 0:D], a0, a1,
                                        op=ALU.mult)
                a0, a1 = broadcast_tensor_aps(s, dcs[:, :, u - u0:u - u0 + 1])
                nc.gpsimd.tensor_tensor(whD2v[:, :, u, 0:D], a0, a1,
                                        op=ALU.mult)
            nc.vector.tensor_copy(
                whB[:, :, u0:u1, D:E],
                bcs[:].rearrange("p t (c o) -> p t c o", o=1))
            nc.vector.tensor_copy(
                whD2v[:, :, u0:u1, D:E],
                dcs[:].rearrange("p t (c o) -> p t c o", o=1))

        scale_bd(0, H, lambda c: whaug[:, :, c * D:(c + 1) * D], bcol, dcol)

        # ---- Q: complement branch, lhsT = adjT directly
        def emit_q(ib):
            pq = psQ.tile([P, H * E], fp32, tag="psq", name="pq")
            for jt in range(JT):
                nc.tensor.matmul(pq[:],
                                 lhsT=adjT[:, jt, ib * P:(ib + 1) * P],
                                 rhs=whD2[:, jt, 0:H * E],
                                 start=(jt == 0), stop=(jt == JT - 1))
            nc.scalar.copy(qsb[:, ib, :], pq[:])

        # Q = colsum - QBG/BIG; batched epilogue halves (first half is
        # emitted mid-loop so it overlaps units 5..7):
        # t = stB + w[i]*(Q - stD); h = t[:,0:16]/t[:,16]
        qv = qsb[:].rearrange("p c (u e) -> p c u e", e=E)
        hprev = hpre[:].rearrange("p c (u d) -> p c u d", d=D)


        def l1_epilogue(u0, u1, s):
            uu = u1 - u0
            x1a = wk1.tile([P, IC, uu, E], fp32, tag=f"x1a{s}")
            nc.vector.tensor_tensor(x1a[:], qv[:, :, u0:u1, :],
                                    st_all[:, :, u0:u1, 1, :],
                                    op=ALU.subtract)
            va = wk1.tile([P, IC, uu, E], fp32, tag=f"va{s}")
            a0, a1 = broadcast_tensor_aps(
                x1a[:],
                w_col[:, :, u0:u1].rearrange("p c (u o) -> p c u o", o=1))
            nc.vector.tensor_tensor(va[:], a0, a1, op=ALU.mult)
            ta = wk1.tile([P, IC, uu, E], fp32, tag=f"ta{s}")
            nc.vector.tensor_tensor(ta[:], st_all[:, :, u0:u1, 0, :], va[:],
                                    op=ALU.add)
            reca = wk1.tile([P, IC, uu, 1], fp32, tag=f"reca{s}")
            nc.vector.reciprocal(reca[:], ta[:, :, :, D:E])
            a0, a1 = broadcast_tensor_aps(ta[:, :, :, 0:D], reca[:])
            nc.vector.tensor_tensor(hprev[:, :, u0:u1, :], a0, a1,
                                    op=ALU.mult)


        # ---- layer-1 attention ----
        ep_eng = [0]
        for h in range(H):
            patts = [psATT.tile([P, 2, E], fp32, tag="att",
                                name=f"patt{h}_{ib}") for ib in range(IC)]
            Xs = Xs0 if h == 0 else emit_masks(h, f2col[:, :, h],
                                               f2colK[:, :, h], pat)
            emit_mms(patts,
                     Xs,
                     lambda jt, h=h: whB[:, jt, h, :],
                     lambda jt, h=h: whD2[:, jt, h * E:(h + 1) * E])
            for ib in range(IC):
                k = ep_eng[0] % 2
                ep_eng[0] += 1
                if k == 0:
                    nc.vector.tensor_copy(st_all[:, ib, h, :, :],
                                          patts[ib][:])
                else:
                    nc.scalar.copy(st_all[:, ib, h, :, :], patts[ib][:])
            if h < IC:
                emit_q(h)

        l1_epilogue(0, H // 2, 0)
        l1_epilogue(H // 2, H, 1)


        # ELU: elu(x) = max(x, min(exp(x)-1, 0))
        etile = wk1.tile([P, IC, HD], fp32, tag="etile")
        nc.scalar.activation(etile[:], hpre[:], AF.Exp, bias=0.0, scale=1.0)
        em = wk1.tile([P, IC, HD], fp32, tag="em")
        nc.vector.tensor_scalar(em[:], etile[:], 1.0, 0.0,
                                op0=ALU.subtract, op1=ALU.min)
        nc.vector.tensor_tensor(h_sb[:], hpre[:], em[:], op=ALU.max)

        # ---- layer 2 ----
        # transpose h blocks; g = h @ [Wout|w2o|w1o]
        g_loc = consts.tile([P, IC, C + 2], fp32)
        for ib in range(IC):
            pt = psB.tile([P, P], bf16, tag="ep", name="pth")
            nc.tensor.transpose(pt[:], h_sb[:, ib, :], identb_sb[:])
            if ib % 2 == 0:
                nc.scalar.copy(hT_sb[:, ib, :], pt[:])
            else:
                nc.vector.tensor_copy(hT_sb[:, ib, :], pt[:])
            pg = psB.tile([P, C + 2], fp32, tag="ep", name="pg")
            nc.tensor.matmul(pg[:], lhsT=hT_sb[:, ib, :], rhs=Wo2_sb[:],
                             start=True, stop=True)
            if ib % 2 == 0:
                nc.vector.tensor_copy(g_loc[:, ib, :], pg[:])
            else:
                nc.scalar.copy(g_loc[:, ib, :], pg[:])

        # gather g = [Wh2 | f2o] for all nodes
        gsrc = dram.tile([R, C + 1], fp32)
        nc.sync.dma_start(
            gsrc[:].rearrange("(c p) e -> p c e", p=P),
            g_loc[:, :, 0:C + 1])
        gdst = dram.tile([n, C + 1], fp32)
        if no_collective:
            # flat stand-in copies (same bytes as the AllGather), spread
            # over the three DMA queues
            for k in range(ncores):
                qeng[k % len(qeng)].dma_start(
                    gdst[k * R:(k + 1) * R, :], gsrc[:])
        else:
            nc.gpsimd.collective_compute(
                "AllGather", ALU.bypass,
                replica_groups=[list(range(ncores))],
                ins=[gsrc.opt()], outs=[gdst.opt()])

        g17_sb = consts.tile([P, JT, C + 1], fp32)
        for k in range(ncores):
            qeng[k % len(qeng)].dma_start(
                g17_sb[:, k * IC:(k + 1) * IC, :],
                gdst[k * R:(k + 1) * R, :]
                .rearrange("(t p) e -> p t e", p=P))
        g2_sb = g17_sb[:, :, 0:C]
        f2ocol = g17_sb[:, :, C:C + 1]

        # f1o row: w1o^T @ hT blocks
        pf1o = psB.tile([1, IC, P], fp32, tag="ep", name="pf1o")
        for ib in range(IC):
            nc.tensor.matmul(pf1o[:, ib, :], lhsT=w1o_sb[:],
                             rhs=hT_sb[:, ib, :], start=True, stop=True)
        f1orow = consts.tile([1, R], fp32)
        nc.vector.tensor_copy(f1orow[:],
                              pf1o[:].rearrange("o c p -> o (c p)"))
        f1orow_bf = consts.tile([1, R], bf16)
        nc.vector.tensor_copy(f1orow_bf[:], f1orow[:])
        pf1obf = psA.tile([P, PSW], fp32, tag="ps", name="pf1obf")
        pf1ob = pf1obf[:, 0:R]
        nc.tensor.matmul(pf1ob[:], lhsT=onesb[:], rhs=f1orow_bf[:],
                         start=True, stop=True)
        nc.scalar.copy(f1b_all[:, U - 1, :], pf1ob[:])

        f1omax = consts.tile([1, 1], fp32)
        nc.vector.tensor_reduce(f1omax[:], f1orow[:], axis=AX.X, op=ALU.max)
        b2_cols = consts.tile([1, NBIAS], fp32)
        nc.vector.memset(b2_cols[:, BB:BB + 1], -FSLACK)
        nc.vector.tensor_scalar(b2_cols[:, BD:BD + 1], f1omax[:], -0.8,
                                -FSLACK, op0=ALU.mult, op1=ALU.add)
        nc.vector.tensor_scalar_mul(b2_cols[:, BW:BW + 1], f1omax[:], 0.8)
        pb2 = psB.tile([P, NBIAS], fp32, tag="ep", name="pb2")
        nc.tensor.matmul(pb2[:], lhsT=onesf[:], rhs=b2_cols[:],
                         start=True, stop=True)
        bias2 = consts.tile([P, NBIAS], fp32)
        nc.vector.tensor_copy(bias2[:], pb2[:])

        bcol2 = consts.tile([P, JT, 1], bf16)
        dcol2 = consts.tile([P, JT, 1], bf16)
        f2oK = consts.tile([P, JT, 1], fp32)
        nc.scalar.activation(bcol2[:], f2ocol[:], AF.Exp,
                             bias=bias2[:, BB:BB + 1], scale=1.0)
        nc.scalar.activation(dcol2[:], f2ocol[:], AF.Exp,
                             bias=bias2[:, BD:BD + 1], scale=ALPHA)
        nc.scalar.activation(w_col[:, :, U - 1], g_loc[:, :, C + 1], AF.Exp,
                             bias=bias2[:, BW:BW + 1], scale=-0.8)
        nc.vector.tensor_scalar_mul(f2oK[:], f2ocol[:], KSTEEP)
        # dummy Ln: pulls the natural_log_exp table load (which also covers
        # the final Exp) off the critical softmax chain
        lnw = wk1.tile([1, 1], fp32, tag="lnw")
        nc.scalar.activation(lnw[:], onesf[0:1, 0:1], AF.Ln, bias=0.0,
                             scale=1.0)

        scale_bd(U - 1, U, lambda c: g2_sb[:], bcol2, dcol2)

        # Q for unit 8
        q2sb = consts.tile([P, IC, E], fp32)
        for ib in range(IC):
            pqf = psQ.tile([P, H * E], fp32, tag="psq", name="pqf")
            pq2 = pqf[:, 0:E]
            for jt in range(JT):
                nc.tensor.matmul(pq2[:],
                                 lhsT=adjT[:, jt, ib * P:(ib + 1) * P],
                                 rhs=whD2[:, jt, H * E:U * E],
                                 start=(jt == 0), stop=(jt == JT - 1))
            nc.vector.tensor_copy(q2sb[:, ib, :], pq2[:])

        patts2 = [psATT.tile([P, 2, E], fp32, tag="att",
                             name=f"patt2_{ib}") for ib in range(IC)]
        Xs2 = emit_masks(U - 1, f2ocol[:, :, 0], f2oK[:, :, 0], pat2)
        emit_mms(patts2, Xs2,
                 lambda jt: whB[:, jt, U - 1, :],
                 lambda jt: whD2[:, jt, (U - 1) * E:U * E])
        st2 = work.tile([P, IC, 2, E], fp32, tag="st2")
        for ib in range(IC):
            if ib % 2 == 0:
                nc.vector.tensor_copy(st2[:, ib, :, :], patts2[ib][:])
            else:
                nc.scalar.copy(st2[:, ib, :, :], patts2[ib][:])
        x2 = wk1.tile([P, IC, E], fp32, tag="x2")
        nc.vector.tensor_tensor(x2[:], q2sb[:], st2[:, :, 1, :],
                                op=ALU.subtract)
        v2 = wk1.tile([P, IC, E], fp32, tag="v2")
        a0, a1 = broadcast_tensor_aps(
            x2[:], w_col[:, :, U - 1].rearrange("p (c o) -> p c o", o=1))
        nc.vector.tensor_tensor(v2[:], a0, a1, op=ALU.mult)
        t2 = wk1.tile([P, IC, E], fp32, tag="t2")
        nc.vector.tensor_tensor(t2[:], st2[:, :, 0, :], v2[:], op=ALU.add)
        rec2 = wk1.tile([P, IC, 1], fp32, tag="rec2")
        nc.vector.reciprocal(rec2[:], t2[:, :, D:E])
        z = wk1.tile([P, IC, C], fp32, tag="z")
        a0, a1 = broadcast_tensor_aps(t2[:, :, 0:D], rec2[:])
        nc.vector.tensor_tensor(z[:], a0, a1, op=ALU.mult)

        # log_softmax along free dim (no transposes needed)
        negmx = wk1.tile([P, IC, 1], fp32, tag="negmx")
        nc.vector.tensor_reduce(negmx[:], z[:], axis=AX.X, op=ALU.max,
                                negate=True)
        zs = wk1.tile([P, IC, C], fp32, tag="zs")
        a0, a1 = broadcast_tensor_aps(z[:], negmx[:])
        nc.vector.tensor_tensor(zs[:], a0, a1, op=ALU.add)
        ez = wk1.tile([P, IC, C], fp32, tag="ez")
        nc.scalar.activation(ez[:], zs[:], AF.Exp, bias=0.0, scale=1.0)
        sume = wk1.tile([P, IC, 1], fp32, tag="sume")
        nc.vector.tensor_reduce(sume[:], ez[:], axis=AX.X, op=ALU.add)
        lns = wk1.tile([P, IC, 1], fp32, tag="lns")
        nc.scalar.activation(lns[:], sume[:], AF.Ln, bias=0.0, scale=1.0)
        zo = wk1.tile([P, IC, C], fp32, tag="zo")
        a0, a1 = broadcast_tensor_aps(zs[:], lns[:])
        nc.vector.tensor_tensor(zo[:], a0, a1, op=ALU.subtract)
        nc.sync.dma_start(out.rearrange("(c p) e -> p c e", p=P), zo[:])

    nc.compile()
    return nc


def prep_inputs(x, adj, W1, a1, Wout, a_out, n=4096, ncores=NCORES):
    """Host-side prep: slice + transpose + bf16 cast + weight folds."""
    R = n // ncores
    x = np.asarray(x, np.float32)
    adj = np.asarray(adj)
    W1 = np.asarray(W1, np.float32)
    a1 = np.asarray(a1, np.float32)
    Wout = np.asarray(Wout, np.float32)
    a_out = np.asarray(a_out, np.float32)

    xT = np.ascontiguousarray(x.T).astype(BF16)
    W1a = W1.transpose(1, 0, 2).reshape(F, H * D)
    w2c = np.einsum("hfd,hd->fh", W1, a1[:, D:])
    w1c = np.einsum("hfd,hd->fh", W1, a1[:, :D])
    w2o = Wout @ a_out[C:]
    w1o = Wout @ a_out[:C]
    Wo2 = np.ascontiguousarray(np.concatenate(
        [Wout, w2o[:, None], w1o[:, None]], axis=1)).astype(BF16)
    identf = np.eye(P, dtype=np.float32)

    adjf = adj.astype(np.float32)
    in_maps = []
    for k in range(ncores):
        rows = slice(k * R, (k + 1) * R)
        xwk = np.concatenate([x[rows].T, w2c, W1a, w1c], axis=1)
        JTl = n // P
        at = adjf[rows].astype(BF16).T                     # [n, R]
        at = np.ascontiguousarray(
            at.reshape(JTl, P, R).transpose(1, 0, 2)).reshape(P, JTl * R)
        in_maps.append({
            "xT": xT,
            "xw": np.ascontiguousarray(xwk).astype(BF16),
            "adjt": at,
            "Wo2": Wo2,
            "identf": identf,
        })
    return in_maps


_cached = {}


def kernel(x, adj, W1, a1, Wout, a_out):
    n = x.shape[0]
    if n not in _cached:
        _cached[n] = build_gat(n=n)
    nc = _cached[n]
    in_maps = prep_inputs(x, adj, W1, a1, Wout, a_out, n=n)
    res = run_bass_kernel_spmd(nc, in_maps, core_ids=list(range(NCORES)))
    outs = [res.results[k]["out"] for k in range(NCORES)]
    return np.concatenate(outs, axis=0)


# revision 40
# speedup vs baseline: 1.5137x; 1.0103x over previous
"""GAT (2-layer graph attention network) Trainium2 Bass kernel — v3.

N=4096 nodes, F=512 feats; layer1: 8 heads x 16 (ELU, concat); layer2:
1 head 128->16; log_softmax. Dense masked attention, row-parallel over
8 cores (core k owns rows [512k, 512k+512)).

Score restructure: leaky(s) = max(s, 0.2s) with s = f1[i]+f2[j] splits
softmax numerators into two rank-separable branches selected by
S = [s >= 0]:
  P = exp(leaky(s)-M)*adj = S*adj*A[i]*B[j] + (1-S)*adj*C[i]*D[j]
with A=exp(f1-f1max), B=exp(f2+f1max-M), C=exp(.2(f1-f1max)),
D=exp(.2 f2+.2 f1max-M), M = f1max + FSLACK.  The constant-slack M
(instead of f1max+f2max) needs no global f2 reduction, so the bias /
exp / scale chain leaves the critical path; the uniform exp(-FSLACK)
factor cancels in the softmax division.

Masks: a tile's 0/1 mask X = S*adj is a first op producing S
(DVE tensor_scalar 4x is_ge, or ACT steep Sigmoid) followed by a
batched multiply against adjT (DVE 2x or Pool), spread across the
three engines by a tunable per-batch pattern (kinds: A=ACT+DVE,
B=ACT+Pool, V=DVE+DVE, W=DVE+Pool).

Attention matmuls are FLIPPED: X[j, i-block] is the PE stationary
operand and the scaled factors whB/whD [j, 17] are the moving operand
(out free size 17, not 512); each (unit, i-block) accumulation chain
owns one PSUM bank, B/D halves share the chain.  Epilogue scalars
(w[i], 1/den) are per-partition; log_softmax needs no transposes.
The complement branch Q = sum_j adj*D*Wh uses lhsT=adjT directly.
Layer 2 gathers
the f2o column separately from the Wh2 block so unit-8 mask compute
overlaps the main gather.
"""

import os
import sys
import contextlib

for _p in ("/opt/trn_rl_repo",):
    if _p not in sys.path and os.path.isdir(_p):
        sys.path.insert(0, _p)

import numpy as np
import ml_dtypes

import concourse.bass as bass
import concourse.bacc as bacc
import concourse.tile as tile
from concourse import mybir
from concourse.bass import broadcast_tensor_aps
from concourse.bass_utils import run_bass_kernel_spmd

BF16 = ml_dtypes.bfloat16
ALPHA = 0.2

F = 512      # input features
H = 8        # heads (layer 1)
D = 16       # per-head dim
C = 16       # classes
P = 128      # partitions
NCORES = 8
E = D + 1    # Wh columns + ones column (17)
U = H + 1    # units: 8 layer-1 heads + 1 layer-2 head
HD = H * D   # 128

BB, BD, BW = 0, 1, 2   # bias cols: -FSLACK, -.8*f1max-FSLACK, .8*f1max
NBIAS = 3
KSTEEP = 1024.0        # sigmoid / score steepness
FSLACK = 30.0          # constant softmax shift slack (M = f1max + FSLACK)


def build_gat(n=4096, ncores=NCORES, dbg=False, no_collective=False,
              pat=("A", "V", "B", "A", "W", "V", "A", "V"),
              pat0=("V", "W", "V", "W", "V", "V", "W", "V"),
              pat2=("V", "A", "W", "V", "A", "W", "V", "A"),
              ttbufs=12, jb=4):
    """Build the SPMD Bass program for one core (row-parallel)."""
    R = n // ncores          # rows per core
    IC = R // P              # i-blocks per core
    JT = n // P              # j-tiles (partition tiles of full node dim)
    FC = F // P              # f chunks
    JB = jb                  # j-tiles per mask batch
    NB = JT // JB
    NCH = n // FC            # xT column-chunk width
    RHSW = H + HD            # fused f2|Wh matmul width (136)
    XWC = R + RHSW + H       # combined xTm|rhs|w1c load width
    PSW = max(R, RHSW)       # shared psA slot width
    assert R % P == 0 and JT % JB == 0

    fp32 = mybir.dt.float32
    bf16 = mybir.dt.bfloat16

    nc = bacc.Bacc("TRN2", target_bir_lowering=False, debug=dbg,
                   num_devices=ncores)

    xT = nc.dram_tensor("xT", [F, n], bf16, kind="ExternalInput").ap()
    xw = nc.dram_tensor("xw", [F, XWC], bf16, kind="ExternalInput").ap()
    adjt = nc.dram_tensor("adjt", [P, JT * R], bf16,
                          kind="ExternalInput").ap()
    Wo2 = nc.dram_tensor("Wo2", [HD, C + 2], bf16, kind="ExternalInput").ap()
    identf = nc.dram_tensor("identf", [P, P], fp32, kind="ExternalInput").ap()
    out = nc.dram_tensor("out", [R, C], fp32, kind="ExternalOutput").ap()

    AF = mybir.ActivationFunctionType
    ALU = mybir.AluOpType
    AX = mybir.AxisListType

    with tile.TileContext(nc) as tc, contextlib.ExitStack() as ctx:
        big = ctx.enter_context(tc.tile_pool(name="big", bufs=1))
        consts = ctx.enter_context(tc.tile_pool(name="consts", bufs=1))
        work = ctx.enter_context(tc.tile_pool(name="work", bufs=2))
        wk1 = ctx.enter_context(tc.tile_pool(name="wk1", bufs=1))
        sc_t = ctx.enter_context(tc.tile_pool(name="sc_t", bufs=ttbufs))
        psA = ctx.enter_context(tc.tile_pool(name="psA", bufs=2, space="PSUM"))
        psQ = ctx.enter_context(tc.tile_pool(name="psQ", bufs=1, space="PSUM"))
        psATT = ctx.enter_context(
            tc.tile_pool(name="psATT", bufs=4, space="PSUM"))
        psB = ctx.enter_context(tc.tile_pool(name="psB", bufs=1, space="PSUM"))
        dram = ctx.enter_context(tc.tile_pool(name="dram", bufs=1,
                                              space="DRAM"))

        # ---- const / persistent loads (spread across DMA queues).
        # xT is loaded in COLUMN chunks so pf12 chain jt can close as soon
        # as the chunk holding its columns lands.
        # big loads ride the SP + Pool DMA queues only; the ACT queue is
        # reserved for small latency-critical transfers (Wo2, f1row_1,
        # bias_row, the layer-2 gathers)
        bigq = [nc.sync, nc.gpsimd]
        qeng = [nc.sync, nc.scalar, nc.gpsimd]
        identf_sb = consts.tile([P, P], fp32)
        nc.gpsimd.dma_start(identf_sb[:], identf)
        xw_sb = consts.tile([P, FC, XWC], bf16)
        nc.sync.dma_start(xw_sb[:], xw.rearrange("(c p) n -> p c n", p=P))
        xT_sb = big.tile([P, FC, n], bf16, tag="xtslot")
        adjT = big.tile([P, JT, R], bf16)
        ACH = JT // 4
        Wo2_sb = consts.tile([P, C + 2], bf16)
        nc.scalar.dma_start(Wo2_sb[:], Wo2)
        # interleave xT column chunks with adjBG chunks: pf12 chain jt and
        # the mask tiles for jt both become runnable ~3us after their chunk
        for c in range(4):
            bigq[c % 2].dma_start(
                xT_sb[:, :, c * NCH:(c + 1) * NCH],
                xT.rearrange("(c p) n -> p c n", p=P)
                [:, :, c * NCH:(c + 1) * NCH])
            bigq[(c + 1) % 2].dma_start(
                adjT[:, c * ACH:(c + 1) * ACH, :],
                adjt[:, c * ACH * R:(c + 1) * ACH * R]
                .rearrange("p (t r) -> p t r", r=R))
        xTm_sb = xw_sb[:, :, 0:R]
        rhs_sb = xw_sb[:, :, R:R + RHSW]          # [w2c(8) | W1a(128)]
        w1c_sb = xw_sb[:, :, R + RHSW:XWC]
        w1o_sb = Wo2_sb[:, C + 1:C + 2]

        identb_sb = consts.tile([P, P], bf16)
        onesb = consts.tile([1, P], bf16)
        nc.vector.memset(onesb[:], 1.0)
        onesf = consts.tile([1, P], fp32)
        nc.vector.memset(onesf[:], 1.0)
        onescol = consts.tile([P, 1], bf16)
        nc.vector.memset(onescol[:], 1.0)

        # persistent intermediates
        whaug = big.tile([P, JT, HD], bf16)       # Wh columns per j
        whB = big.tile([P, JT, U, E], bf16)       # B_j*[Wh_u|1]
        whD2 = big.tile([P, JT, U * E], bf16)     # D_j*[Wh_u|1], (u,e)-contig
        f1b_all = big.tile([P, U, R], bf16)       # f1[i] bcast on partitions
        f2col = big.tile([P, JT, H], fp32)        # f2[j] (layer 1)
        f2colK = big.tile([P, JT, H], fp32)       # KSTEEP*f2[j]
        bcol = consts.tile([P, JT, H], bf16)
        dcol = consts.tile([P, JT, H], bf16)
        w_col = consts.tile([P, IC, U], fp32)     # exp(.8(f1max-f1[i]))
        hpre = big.tile([P, IC, HD], fp32)        # layer-1 out pre-ELU
        h_sb = big.tile([P, IC, HD], bf16)        # post-ELU
        hT_sb = big.tile([P, IC, P], bf16)        # transposed h blocks
        qsb = consts.tile([P, IC, H * E], fp32)   # Q for layer-1 units
        st_all = big.tile([P, IC, H, 2, E], fp32)  # drained attention psums
        f1colT = consts.tile([P, IC, H], fp32)    # f1[i] per-partition

        # ---- phase 1a: f1 row (own rows) ----
        pf1 = psB.tile([H, R], fp32, tag="ep")
        for fc in range(FC):
            nc.tensor.matmul(pf1[:], lhsT=w1c_sb[:, fc, :],
                             rhs=xTm_sb[:, fc, :],
                             start=(fc == 0), stop=(fc == FC - 1))
        f1row_sb = consts.tile([H, R], fp32)
        nc.vector.tensor_copy(f1row_sb[:], pf1[:])
        f1row_bf = consts.tile([H, R], bf16)
        nc.vector.tensor_copy(f1row_bf[:], f1row_sb[:])
        f1row_1 = consts.tile([1, H, R], bf16)
        nc.scalar.dma_start(f1row_1[:], f1row_bf[:])
        f1max = consts.tile([H, 1], fp32)
        nc.vector.tensor_reduce(f1max[:], f1row_sb[:], axis=AX.X, op=ALU.max)

        # f1 broadcast for head 0 first (gates the first mask tiles)
        def f1b_bcast(h):
            pbf = psA.tile([P, PSW], fp32, tag="ps", name="pbf")
            pb = pbf[:, 0:R]
            nc.tensor.matmul(pb[:], lhsT=onesb[:],
                             rhs=f1row_1[0:1, h, :], start=True, stop=True)
            nc.scalar.copy(f1b_all[:, h, :], pb[:])
        f1b_bcast(0)

        # ---- phase 1b: fused f2|Wh matmuls over all j ----
        for jt in range(JT):
            pf12f = psA.tile([P, PSW], fp32, tag="ps", name="pf12f")
            pf12 = pf12f[:, 0:RHSW]
            for fc in range(FC):
                nc.tensor.matmul(
                    pf12[:],
                    lhsT=xT_sb[:, fc, jt * P:(jt + 1) * P],
                    rhs=rhs_sb[:, fc, :],
                    start=(fc == 0), stop=(fc == FC - 1))
            nc.vector.tensor_copy(f2col[:, jt, :], pf12[:, 0:H])
            nc.scalar.copy(whaug[:, jt, :], pf12[:, H:RHSW])

        for h in range(1, H):
            f1b_bcast(h)

        # f1 per-partition columns (for epilogue w)
        nc.vector.tensor_copy(identb_sb[:], identf_sb[:])
        for ib in range(IC):
            pt = psB.tile([P, H], bf16, tag="ep", name="pt")
            nc.tensor.transpose(pt[:], f1row_bf[:, ib * P:(ib + 1) * P],
                                identb_sb[0:H, 0:H])
            nc.vector.tensor_copy(f1colT[:, ib, :], pt[:])

        # ---- mask emitter: per-batch engine pattern.
        # 'A': 4 ACT sigmoids + DVE is_gt; 'V': 4 DVE 4x tensor_scalar +
        # DVE is_gt; 'P': 4 Pool fused stt.
        bat_i = [0]

        def emit_masks(u, f2c, f2cK, patv):
            Xs = []
            for b in range(NB):
                jt0 = b * JB
                kind = patv[bat_i[0] % len(patv)]
                bat_i[0] += 1
                X = sc_t.tile([P, JB, R], bf16, tag="xs")
                for q in range(JB):
                    jt = jt0 + q
                    if kind in ("A", "B"):
                        nc.scalar.activation(
                            X[:, q, :], f1b_all[:, u, :], AF.Sigmoid,
                            bias=f2cK[:, jt:jt + 1], scale=KSTEEP)
                    else:
                        nc.vector.tensor_scalar(
                            X[:, q, :], f1b_all[:, u, :],
                            f2c[:, jt:jt + 1], 0.0,
                            op0=ALU.add, op1=ALU.is_ge)
                if kind in ("B", "W"):
                    # per-tile Pool multiplies: finer granularity releases
                    # each tile to the PE as soon as it is masked
                    for q in range(JB):
                        jt = jt0 + q
                        nc.gpsimd.tensor_tensor(
                            X[:, q, :], X[:, q, :], adjT[:, jt, :],
                            op=ALU.mult)
                else:
                    nc.vector.tensor_tensor(
                        X[:], X[:], adjT[:, jt0:jt0 + JB, :], op=ALU.mult)
                Xs.append(X)
            return Xs

        # attention matmuls (flipped): X is the stationary operand; each
        # (unit, iblock) chain owns one PSUM bank, B/D halves share it.
        def emit_mms(patts, Xs, whB_u, whD_u):
            for b in range(NB):
                X = Xs[b]
                for q in range(JB):
                    jt = b * JB + q
                    for ib in range(IC):
                        lw = X[:, q, ib * P:(ib + 1) * P]
                        nc.tensor.matmul(
                            patts[ib][:, 0, :], lhsT=lw, rhs=whB_u(jt),
                            start=(jt == 0), stop=False)
                        nc.tensor.matmul(
                            patts[ib][:, 1, :], lhsT=lw, rhs=whD_u(jt),
                            start=False, stop=(jt == JT - 1))

        # masks for unit 0 go out BEFORE the scale chain so DVE/ACT/Pool
        # fill buffers while whB/whD are being scaled
        Xs0 = emit_masks(0, f2col[:, :, 0], f2colK[:, :, 0], pat0)

        # biases off f1max only (constant-slack M): BD = -.8f1max-FSLACK,
        # BW = .8f1max; the B-column bias is the constant -FSLACK.
        bias_cols = consts.tile([H, NBIAS], fp32)
        nc.vector.memset(bias_cols[:, BB:BB + 1], -FSLACK)
        nc.vector.tensor_scalar(bias_cols[:, BD:BD + 1], f1max[:], -0.8,
                                -FSLACK, op0=ALU.mult, op1=ALU.add)
        nc.vector.tensor_scalar_mul(bias_cols[:, BW:BW + 1], f1max[:], 0.8)
        bias_row = consts.tile([1, H, NBIAS], fp32)
        nc.scalar.dma_start(bias_row[:], bias_cols[:])
        pba = psB.tile([P, H * NBIAS], fp32, tag="ep", name="pba")
        nc.tensor.matmul(pba[:], lhsT=onesf[:],
                         rhs=bias_row[:].rearrange("o h e -> o (h e)"),
                         start=True, stop=True)
        bias_all = consts.tile([P, H, NBIAS], fp32)
        nc.vector.tensor_copy(
            bias_all[:], pba[:].rearrange("p (h e) -> p h e", e=NBIAS))
        nc.vector.tensor_scalar_mul(f2colK[:], f2col[:], KSTEEP)

        # exps (ACT): B/D columns + per-partition w columns
        for h in range(H):
            nc.scalar.activation(bcol[:, :, h], f2col[:, :, h], AF.Exp,
                                 bias=bias_all[:, h, BB:BB + 1], scale=1.0)
            nc.scalar.activation(dcol[:, :, h], f2col[:, :, h], AF.Exp,
                                 bias=bias_all[:, h, BD:BD + 1], scale=ALPHA)
            nc.scalar.activation(w_col[:, :, h], f1colT[:, :, h], AF.Exp,
                                 bias=bias_all[:, h, BW:BW + 1], scale=-0.8)


        # scale whaug into whB / whD2 (broadcast B/D along e):
        # B-half on DVE, D-half on Pool
        whD2v = whD2[:].rearrange("p t (u e) -> p t u e", e=E)

        def scale_bd(u0, u1, src, bcs, dcs):
            for u in range(u0, u1):
                s = src(u - u0)
                a0, a1 = broadcast_tensor_aps(s, bcs[:, :, u - u0:u - u0 + 1])
                nc.vector.tensor_tensor(whB[:, :, u, 0:D], a0, a1,
                                        op=ALU.mult)
                a0, a1 = broadcast_tensor_aps(s, dcs[:, :, u - u0:u - u0 + 1])
                nc.gpsimd.tensor_tensor(whD2v[:, :, u, 0:D], a0, a1,
                                        op=ALU.mult)
            nc.vector.tensor_copy(
                whB[:, :, u0:u1, D:E],
                bcs[:].rearrange("p t (c o) -> p t c o", o=1))
            nc.vector.tensor_copy(
                whD2v[:, :, u0:u1, D:E],
                dcs[:].rearrange("p t (c o) -> p t c o", o=1))

        scale_bd(0, H, lambda c: whaug[:, :, c * D:(c + 1) * D], bcol, dcol)

        # ---- Q: complement branch, lhsT = adjT directly
        def emit_q(ib):
            pq = psQ.tile([P, H * E], fp32, tag="psq", name="pq")
            for jt in range(JT):
                nc.tensor.matmul(pq[:],
                                 lhsT=adjT[:, jt, ib * P:(ib + 1) * P],
                                 rhs=whD2[:, jt, 0:H * E],
                                 start=(jt == 0), stop=(jt == JT - 1))
            nc.scalar.copy(qsb[:, ib, :], pq[:])

        # Q = colsum - QBG/BIG; batched epilogue halves (first half is
        # emitted mid-loop so it overlaps units 5..7):
        # t = stB + w[i]*(Q - stD); h = t[:,0:16]/t[:,16]
        qv = qsb[:].rearrange("p c (u e) -> p c u e", e=E)
        hprev = hpre[:].rearrange("p c (u d) -> p c u d", d=D)


        def l1_epilogue(u0, u1, s):
            uu = u1 - u0
            x1a = wk1.tile([P, IC, uu, E], fp32, tag=f"x1a{s}")
            nc.vector.tensor_tensor(x1a[:], qv[:, :, u0:u1, :],
                                    st_all[:, :, u0:u1, 1, :],
                                    op=ALU.subtract)
            va = wk1.tile([P, IC, uu, E], fp32, tag=f"va{s}")
            a0, a1 = broadcast_tensor_aps(
                x1a[:],
                w_col[:, :, u0:u1].rearrange("p c (u o) -> p c u o", o=1))
            nc.vector.tensor_tensor(va[:], a0, a1, op=ALU.mult)
            ta = wk1.tile([P, IC, uu, E], fp32, tag=f"ta{s}")
            nc.vector.tensor_tensor(ta[:], st_all[:, :, u0:u1, 0, :], va[:],
                                    op=ALU.add)
            reca = wk1.tile([P, IC, uu, 1], fp32, tag=f"reca{s}")
            nc.vector.reciprocal(reca[:], ta[:, :, :, D:E])
            a0, a1 = broadcast_tensor_aps(ta[:, :, :, 0:D], reca[:])
            nc.vector.tensor_tensor(hprev[:, :, u0:u1, :], a0, a1,
                                    op=ALU.mult)


        # ---- layer-1 attention ----
        ep_eng = [0]
        for h in range(H):
            patts = [psATT.tile([P, 2, E], fp32, tag="att",
                                name=f"patt{h}_{ib}") for ib in range(IC)]
            Xs = Xs0 if h == 0 else emit_masks(h, f2col[:, :, h],
                                               f2colK[:, :, h], pat)
            emit_mms(patts,
                     Xs,
                     lambda jt, h=h: whB[:, jt, h, :],
                     lambda jt, h=h: whD2[:, jt, h * E:(h + 1) * E])
            for ib in range(IC):
                k = ep_eng[0] % 2
                ep_eng[0] += 1
                if k == 0:
                    nc.vector.tensor_copy(st_all[:, ib, h, :, :],
                                          patts[ib][:])
                else:
                    nc.scalar.copy(st_all[:, ib, h, :, :], patts[ib][:])
            if h < IC:
                emit_q(h)

        l1_epilogue(0, H // 2, 0)
        l1_epilogue(H // 2, H, 1)


        # ELU: elu(x) = max(x, min(exp(x)-1, 0))
        etile = wk1.tile([P, IC, HD], fp32, tag="etile")
        nc.scalar.activation(etile[:], hpre[:], AF.Exp, bias=0.0, scale=1.0)
        em = wk1.tile([P, IC, HD], fp32, tag="em")
        nc.vector.tensor_scalar(em[:], etile[:], 1.0, 0.0,
                                op0=ALU.subtract, op1=ALU.min)
        nc.vector.tensor_tensor(h_sb[:], hpre[:], em[:], op=ALU.max)

        # ---- layer 2 ----
        # transpose h blocks; g = h @ [Wout|w2o|w1o]
        g_loc = consts.tile([P, IC, C + 2], fp32)
        for ib in range(IC):
            pt = psB.tile([P, P], bf16, tag="ep", name="pth")
            nc.tensor.transpose(pt[:], h_sb[:, ib, :], identb_sb[:])
            if ib % 2 == 0:
                nc.scalar.copy(hT_sb[:, ib, :], pt[:])
            else:
                nc.vector.tensor_copy(hT_sb[:, ib, :], pt[:])
            pg = psB.tile([P, C + 2], fp32, tag="ep", name="pg")
            nc.tensor.matmul(pg[:], lhsT=hT_sb[:, ib, :], rhs=Wo2_sb[:],
                             start=True, stop=True)
            if ib % 2 == 0:
                nc.vector.tensor_copy(g_loc[:, ib, :], pg[:])
            else:
                nc.scalar.copy(g_loc[:, ib, :], pg[:])

        # gather g = [Wh2 | f2o] for all nodes
        gsrc = dram.tile([R, C + 1], fp32)
        nc.sync.dma_start(
            gsrc[:].rearrange("(c p) e -> p c e", p=P),
            g_loc[:, :, 0:C + 1])
        gdst = dram.tile([n, C + 1], fp32)
        if no_collective:
            # flat stand-in copies (same bytes as the AllGather), spread
            # over the three DMA queues
            for k in range(ncores):
                qeng[k % len(qeng)].dma_start(
                    gdst[k * R:(k + 1) * R, :], gsrc[:])
        else:
            nc.gpsimd.collective_compute(
                "AllGather", ALU.bypass,
                replica_groups=[list(range(ncores))],
                ins=[gsrc.opt()], outs=[gdst.opt()])

        g17_sb = consts.tile([P, JT, C + 1], fp32)
        for k in range(ncores):
            qeng[k % len(qeng)].dma_start(
                g17_sb[:, k * IC:(k + 1) * IC, :],
                gdst[k * R:(k + 1) * R, :]
                .rearrange("(t p) e -> p t e", p=P))
        g2_sb = g17_sb[:, :, 0:C]
        f2ocol = g17_sb[:, :, C:C + 1]

        # f1o row: w1o^T @ hT blocks
        pf1o = psB.tile([1, IC, P], fp32, tag="ep", name="pf1o")
        for ib in range(IC):
            nc.tensor.matmul(pf1o[:, ib, :], lhsT=w1o_sb[:],
                             rhs=hT_sb[:, ib, :], start=True, stop=True)
        f1orow = consts.tile([1, R], fp32)
        nc.vector.tensor_copy(f1orow[:],
                              pf1o[:].rearrange("o c p -> o (c p)"))
        f1orow_bf = consts.tile([1, R], bf16)
        nc.vector.tensor_copy(f1orow_bf[:], f1orow[:])
        pf1obf = psA.tile([P, PSW], fp32, tag="ps", name="pf1obf")
        pf1ob = pf1obf[:, 0:R]
        nc.tensor.matmul(pf1ob[:], lhsT=onesb[:], rhs=f1orow_bf[:],
                         start=True, stop=True)
        nc.scalar.copy(f1b_all[:, U - 1, :], pf1ob[:])

        f1omax = consts.tile([1, 1], fp32)
        nc.vector.tensor_reduce(f1omax[:], f1orow[:], axis=AX.X, op=ALU.max)
        b2_cols = consts.tile([1, NBIAS], fp32)
        nc.vector.memset(b2_cols[:, BB:BB + 1], -FSLACK)
        nc.vector.tensor_scalar(b2_cols[:, BD:BD + 1], f1omax[:], -0.8,
                                -FSLACK, op0=ALU.mult, op1=ALU.add)
        nc.vector.tensor_scalar_mul(b2_cols[:, BW:BW + 1], f1omax[:], 0.8)
        pb2 = psB.tile([P, NBIAS], fp32, tag="ep", name="pb2")
        nc.tensor.matmul(pb2[:], lhsT=onesf[:], rhs=b2_cols[:],
                         start=True, stop=True)
        bias2 = consts.tile([P, NBIAS], fp32)
        nc.vector.tensor_copy(bias2[:], pb2[:])

        bcol2 = consts.tile([P, JT, 1], bf16)
        dcol2 = consts.tile([P, JT, 1], bf16)
        f2oK = consts.tile([P, JT, 1], fp32)
        nc.scalar.activation(bcol2[:], f2ocol[:], AF.Exp,
                             bias=bias2[:, BB:BB + 1], scale=1.0)
        nc.scalar.activation(dcol2[:], f2ocol[:], AF.Exp,
                             bias=bias2[:, BD:BD + 1], scale=ALPHA)
        nc.scalar.activation(w_col[:, :, U - 1], g_loc[:, :, C + 1], AF.Exp,
                             bias=bias2[:, BW:BW + 1], scale=-0.8)
        nc.vector.tensor_scalar_mul(f2oK[:], f2ocol[:], KSTEEP)
        # dummy Ln: pulls the natural_log_exp table load (which also covers
        # the final Exp) off the critical softmax chain
        lnw = wk1.tile([1, 1], fp32, tag="lnw")
        nc.scalar.activation(lnw[:], onesf[0:1, 0:1], AF.Ln, bias=0.0,
                             scale=1.0)

        scale_bd(U - 1, U, lambda c: g2_sb[:], bcol2, dcol2)

        # Q for unit 8
        q2sb = consts.tile([P, IC, E], fp32)
        for ib in range(IC):
            pqf = psQ.tile([P, H * E], fp32, tag="psq", name="pqf")
            pq2 = pqf[:, 0:E]
            for jt in range(JT):
                nc.tensor.matmul(pq2[:],
                                 lhsT=adjT[:, jt, ib * P:(ib + 1) * P],
                                 rhs=whD2[:, jt, H * E:U * E],
                                 start=(jt == 0), stop=(jt == JT - 1))
            nc.vector.tensor_copy(q2sb[:, ib, :], pq2[:])

        patts2 = [psATT.tile([P, 2, E], fp32, tag="att",
                             name=f"patt2_{ib}") for ib in range(IC)]
        Xs2 = emit_masks(U - 1, f2ocol[:, :, 0], f2oK[:, :, 0], pat2)
        emit_mms(patts2, Xs2,
                 lambda jt: whB[:, jt, U - 1, :],
                 lambda jt: whD2[:, jt, (U - 1) * E:U * E])
        st2 = work.tile([P, IC, 2, E], fp32, tag="st2")
        for ib in range(IC):
            if ib % 2 == 0:
                nc.vector.tensor_copy(st2[:, ib, :, :], patts2[ib][:])
            else:
                nc.scalar.copy(st2[:, ib, :, :], patts2[ib][:])
        x2 = wk1.tile([P, IC, E], fp32, tag="x2")
        nc.vector.tensor_tensor(x2[:], q2sb[:], st2[:, :, 1, :],
                                op=ALU.subtract)
        v2 = wk1.tile([P, IC, E], fp32, tag="v2")
        a0, a1 = broadcast_tensor_aps(
            x2[:], w_col[:, :, U - 1].rearrange("p (c o) -> p c o", o=1))
        nc.vector.tensor_tensor(v2[:], a0, a1, op=ALU.mult)
        t2 = wk1.tile([P, IC, E], fp32, tag="t2")
        nc.vector.tensor_tensor(t2[:], st2[:, :, 0, :], v2[:], op=ALU.add)
        rec2 = wk1.tile([P, IC, 1], fp32, tag="rec2")
        nc.vector.reciprocal(rec2[:], t2[:, :, D:E])
        z = wk1.tile([P, IC, C], fp32, tag="z")
        a0, a1 = broadcast_tensor_aps(t2[:, :, 0:D], rec2[:])
        nc.vector.tensor_tensor(z[:], a0, a1, op=ALU.mult)

        # log_softmax along free dim (no transposes needed)
        negmx = wk1.tile([P, IC, 1], fp32, tag="negmx")
        nc.vector.tensor_reduce(negmx[:], z[:], axis=AX.X, op=ALU.max,
                                negate=True)
        zs = wk1.tile([P, IC, C], fp32, tag="zs")
        a0, a1 = broadcast_tensor_aps(z[:], negmx[:])
        nc.vector.tensor_tensor(zs[:], a0, a1, op=ALU.add)
        ez = wk1.tile([P, IC, C], fp32, tag="ez")
        nc.scalar.activation(ez[:], zs[:], AF.Exp, bias=0.0, scale=1.0)
        sume = wk1.tile([P, IC, 1], fp32, tag="sume")
        nc.vector.tensor_reduce(sume[:], ez[:], axis=AX.X, op=ALU.add)
        lns = wk1.tile([P, IC, 1], fp32, tag="lns")
        nc.scalar.activation(lns[:], sume[:], AF.Ln, bias=0.0, scale=1.0)
        zo = wk1.tile([P, IC, C], fp32, tag="zo")
        a0, a1 = broadcast_tensor_aps(zs[:], lns[:])
        nc.vector.tensor_tensor(zo[:], a0, a1, op=ALU.subtract)
        nc.sync.dma_start(out.rearrange("(c p) e -> p c e", p=P), zo[:])

    nc.compile()
    return nc


def prep_inputs(x, adj, W1, a1, Wout, a_out, n=4096, ncores=NCORES):
    """Host-side prep: slice + transpose + bf16 cast + weight folds."""
    R = n // ncores
    x = np.asarray(x, np.float32)
    adj = np.asarray(adj)
    W1 = np.asarray(W1, np.float32)
    a1 = np.asarray(a1, np.float32)
    Wout = np.asarray(Wout, np.float32)
    a_out = np.asarray(a_out, np.float32)

    xT = np.ascontiguousarray(x.T).astype(BF16)
    W1a = W1.transpose(1, 0, 2).reshape(F, H * D)
    w2c = np.einsum("hfd,hd->fh", W1, a1[:, D:])
    w1c = np.einsum("hfd,hd->fh", W1, a1[:, :D])
    w2o = Wout @ a_out[C:]
    w1o = Wout @ a_out[:C]
    Wo2 = np.ascontiguousarray(np.concatenate(
        [Wout, w2o[:, None], w1o[:, None]], axis=1)).astype(BF16)
    identf = np.eye(P, dtype=np.float32)

    adjf = adj.astype(np.float32)
    in_maps = []
    for k in range(ncores):
        rows = slice(k * R, (k + 1) * R)
        xwk = np.concatenate([x[rows].T, w2c, W1a, w1c], axis=1)
        JTl = n // P
        at = adjf[rows].astype(BF16).T                     # [n, R]
        at = np.ascontiguousarray(
            at.reshape(JTl, P, R).transpose(1, 0, 2)).reshape(P, JTl * R)
        in_maps.append({
            "xT": xT,
            "xw": np.ascontiguousarray(xwk).astype(BF16),
            "adjt": at,
            "Wo2": Wo2,
            "identf": identf,
        })
    return in_maps


_cached = {}


def kernel(x, adj, W1, a1, Wout, a_out):
    n = x.shape[0]
    if n not in _cached:
        _cached[n] = build_gat(n=n)
    nc = _cached[n]
    in_maps = prep_inputs(x, adj, W1, a1, Wout, a_out, n=n)
    res = run_bass_kernel_spmd(nc, in_maps, core_ids=list(range(NCORES)))
    outs = [res.results[k]["out"] for k in range(NCORES)]
    return np.concatenate(outs, axis=0)


# revision 41
# speedup vs baseline: 1.5632x; 1.0327x over previous
"""GAT (2-layer graph attention network) Trainium2 Bass kernel — v3.

N=4096 nodes, F=512 feats; layer1: 8 heads x 16 (ELU, concat); layer2:
1 head 128->16; log_softmax. Dense masked attention, row-parallel over
8 cores (core k owns rows [512k, 512k+512)).

Score restructure: leaky(s) = max(s, 0.2s) with s = f1[i]+f2[j] splits
softmax numerators into two rank-separable branches selected by
S = [s >= 0]:
  P = exp(leaky(s)-M)*adj = S*adj*A[i]*B[j] + (1-S)*adj*C[i]*D[j]
with A=exp(f1-f1max), B=exp(f2+f1max-M), C=exp(.2(f1-f1max)),
D=exp(.2 f2+.2 f1max-M), M = f1max + FSLACK.  The constant-slack M
(instead of f1max+f2max) needs no global f2 reduction, so the bias /
exp / scale chain leaves the critical path; the uniform exp(-FSLACK)
factor cancels in the softmax division.

Masks: a tile's 0/1 mask X = S*adj is a first op producing S
(DVE tensor_scalar 4x is_ge, or ACT steep Sigmoid) followed by a
batched multiply against adjT (DVE 2x or Pool), spread across the
three engines by a tunable per-batch pattern (kinds: A=ACT+DVE,
B=ACT+Pool, V=DVE+DVE, W=DVE+Pool).

Attention matmuls are FLIPPED: X[j, i-block] is the PE stationary
operand and the scaled factors whB/whD [j, 17] are the moving operand
(out free size 17, not 512); each (unit, i-block) accumulation chain
owns one PSUM bank, B/D halves share the chain.  Epilogue scalars
(w[i], 1/den) are per-partition; log_softmax needs no transposes.
The complement branch Q = sum_j adj*D*Wh uses lhsT=adjT directly.
Layer 2 gathers
the f2o column separately from the Wh2 block so unit-8 mask compute
overlaps the main gather.
"""

import os
import sys
import contextlib

for _p in ("/opt/trn_rl_repo",):
    if _p not in sys.path and os.path.isdir(_p):
        sys.path.insert(0, _p)

import numpy as np
import ml_dtypes

import concourse.bass as bass
import concourse.bacc as bacc
import concourse.tile as tile
from concourse import mybir
from concourse.bass import broadcast_tensor_aps
from concourse.bass_utils import run_bass_kernel_spmd

BF16 = ml_dtypes.bfloat16
ALPHA = 0.2

F = 512      # input features
H = 8        # heads (layer 1)
D = 16       # per-head dim
C = 16       # classes
P = 128      # partitions
NCORES = 8
E = D + 1    # Wh columns + ones column (17)
U = H + 1    # units: 8 layer-1 heads + 1 layer-2 head
HD = H * D   # 128

BB, BD, BW = 0, 1, 2   # bias cols: -FSLACK, -.8*f1max-FSLACK, .8*f1max
NBIAS = 3
KSTEEP = 1024.0        # sigmoid / score steepness
FSLACK = 30.0          # constant softmax shift slack (M = f1max + FSLACK)


def build_gat(n=4096, ncores=NCORES, dbg=False, no_collective=False,
              pat=("A", "V", "B", "A", "W", "V", "A", "V"),
              pat0=("V", "W", "V", "W", "V", "V", "W", "V"),
              pat2=("V", "A", "W", "V", "A", "W", "V", "A"),
              ttbufs=12, jb=4):
    """Build the SPMD Bass program for one core (row-parallel)."""
    R = n // ncores          # rows per core
    IC = R // P              # i-blocks per core
    JT = n // P              # j-tiles (partition tiles of full node dim)
    FC = F // P              # f chunks
    JB = jb                  # j-tiles per mask batch
    NB = JT // JB
    NCH = n // FC            # xT column-chunk width
    RHSW = H + HD            # fused f2|Wh matmul width (136)
    XWC = R + RHSW + H       # combined xTm|rhs|w1c load width
    PSW = max(R, RHSW)       # shared psA slot width
    assert R % P == 0 and JT % JB == 0

    fp32 = mybir.dt.float32
    bf16 = mybir.dt.bfloat16

    nc = bacc.Bacc("TRN2", target_bir_lowering=False, debug=dbg,
                   num_devices=ncores)

    xT = nc.dram_tensor("xT", [F, n], mybir.dt.float8e4,
                        kind="ExternalInput").ap()
    xw = nc.dram_tensor("xw", [F, XWC], bf16, kind="ExternalInput").ap()
    adjt = nc.dram_tensor("adjt", [P, JT * R], bf16,
                          kind="ExternalInput").ap()
    Wo2 = nc.dram_tensor("Wo2", [HD, C + 2], bf16, kind="ExternalInput").ap()
    identf = nc.dram_tensor("identf", [P, P], fp32, kind="ExternalInput").ap()
    out = nc.dram_tensor("out", [R, C], fp32, kind="ExternalOutput").ap()

    AF = mybir.ActivationFunctionType
    ALU = mybir.AluOpType
    AX = mybir.AxisListType

    with tile.TileContext(nc) as tc, contextlib.ExitStack() as ctx:
        big = ctx.enter_context(tc.tile_pool(name="big", bufs=1))
        consts = ctx.enter_context(tc.tile_pool(name="consts", bufs=1))
        work = ctx.enter_context(tc.tile_pool(name="work", bufs=2))
        wk1 = ctx.enter_context(tc.tile_pool(name="wk1", bufs=1))
        sc_t = ctx.enter_context(tc.tile_pool(name="sc_t", bufs=ttbufs))
        psA = ctx.enter_context(tc.tile_pool(name="psA", bufs=2, space="PSUM"))
        psQ = ctx.enter_context(tc.tile_pool(name="psQ", bufs=1, space="PSUM"))
        psATT = ctx.enter_context(
            tc.tile_pool(name="psATT", bufs=4, space="PSUM"))
        psB = ctx.enter_context(tc.tile_pool(name="psB", bufs=1, space="PSUM"))
        dram = ctx.enter_context(tc.tile_pool(name="dram", bufs=1,
                                              space="DRAM"))

        # ---- const / persistent loads (spread across DMA queues).
        # xT is loaded in COLUMN chunks so pf12 chain jt can close as soon
        # as the chunk holding its columns lands.
        # big loads ride the SP + Pool DMA queues only; the ACT queue is
        # reserved for small latency-critical transfers (Wo2, f1row_1,
        # bias_row, the layer-2 gathers)
        bigq = [nc.sync, nc.gpsimd]
        qeng = [nc.sync, nc.scalar, nc.gpsimd]
        identf_sb = consts.tile([P, P], fp32)
        nc.gpsimd.dma_start(identf_sb[:], identf)
        xw_sb = consts.tile([P, FC, XWC], bf16)
        nc.sync.dma_start(xw_sb[:], xw.rearrange("(c p) n -> p c n", p=P))
        xT_sb = big.tile([P, FC, n], mybir.dt.float8e4, tag="xtslot")
        adjT = big.tile([P, JT, R], bf16)
        ACH = JT // 4
        Wo2_sb = consts.tile([P, C + 2], bf16)
        nc.scalar.dma_start(Wo2_sb[:], Wo2)
        # interleave xT column chunks with adjBG chunks: pf12 chain jt and
        # the mask tiles for jt both become runnable ~3us after their chunk
        for c in range(4):
            bigq[c % 2].dma_start(
                xT_sb[:, :, c * NCH:(c + 1) * NCH],
                xT.rearrange("(c p) n -> p c n", p=P)
                [:, :, c * NCH:(c + 1) * NCH])
            bigq[(c + 1) % 2].dma_start(
                adjT[:, c * ACH:(c + 1) * ACH, :],
                adjt[:, c * ACH * R:(c + 1) * ACH * R]
                .rearrange("p (t r) -> p t r", r=R))
        xTm_sb = xw_sb[:, :, 0:R]
        rhs_sb = xw_sb[:, :, R:R + RHSW]          # [w2c(8) | W1a(128)]
        w1c_sb = xw_sb[:, :, R + RHSW:XWC]
        w1o_sb = Wo2_sb[:, C + 1:C + 2]

        identb_sb = consts.tile([P, P], bf16)
        onesb = consts.tile([1, P], bf16)
        nc.vector.memset(onesb[:], 1.0)
        onesf = consts.tile([1, P], fp32)
        nc.vector.memset(onesf[:], 1.0)
        onescol = consts.tile([P, 1], bf16)
        nc.vector.memset(onescol[:], 1.0)

        # persistent intermediates
        whaug = big.tile([P, JT, HD], bf16)       # Wh columns per j
        whB = big.tile([P, JT, U, E], bf16)       # B_j*[Wh_u|1]
        whD2 = big.tile([P, JT, U * E], bf16)     # D_j*[Wh_u|1], (u,e)-contig
        f1b_all = big.tile([P, U, R], bf16)       # f1[i] bcast on partitions
        f2col = big.tile([P, JT, H], fp32)        # f2[j] (layer 1)
        f2colK = big.tile([P, JT, H], fp32)       # KSTEEP*f2[j]
        bcol = consts.tile([P, JT, H], bf16)
        dcol = consts.tile([P, JT, H], bf16)
        w_col = consts.tile([P, IC, U], fp32)     # exp(.8(f1max-f1[i]))
        hpre = big.tile([P, IC, HD], fp32)        # layer-1 out pre-ELU
        h_sb = big.tile([P, IC, HD], bf16)        # post-ELU
        hT_sb = big.tile([P, IC, P], bf16)        # transposed h blocks
        qsb = consts.tile([P, IC, H * E], fp32)   # Q for layer-1 units
        st_all = big.tile([P, IC, H, 2, E], fp32)  # drained attention psums
        f1colT = consts.tile([P, IC, H], fp32)    # f1[i] per-partition

        # ---- phase 1a: f1 row (own rows) ----
        pf1 = psB.tile([H, R], fp32, tag="ep")
        for fc in range(FC):
            nc.tensor.matmul(pf1[:], lhsT=w1c_sb[:, fc, :],
                             rhs=xTm_sb[:, fc, :],
                             start=(fc == 0), stop=(fc == FC - 1))
        f1row_sb = consts.tile([H, R], fp32)
        nc.vector.tensor_copy(f1row_sb[:], pf1[:])
        f1row_bf = consts.tile([H, R], bf16)
        nc.vector.tensor_copy(f1row_bf[:], f1row_sb[:])
        f1row_1 = consts.tile([1, H, R], bf16)
        nc.scalar.dma_start(f1row_1[:], f1row_bf[:])
        f1max = consts.tile([H, 1], fp32)
        nc.vector.tensor_reduce(f1max[:], f1row_sb[:], axis=AX.X, op=ALU.max)

        # f1 broadcast for head 0 first (gates the first mask tiles)
        def f1b_bcast(h):
            pbf = psA.tile([P, PSW], fp32, tag="ps", name="pbf")
            pb = pbf[:, 0:R]
            nc.tensor.matmul(pb[:], lhsT=onesb[:],
                             rhs=f1row_1[0:1, h, :], start=True, stop=True)
            nc.scalar.copy(f1b_all[:, h, :], pb[:])
        f1b_bcast(0)

        # ---- phase 1b: fused f2|Wh matmuls over all j ----
        for jt in range(JT):
            pf12f = psA.tile([P, PSW], fp32, tag="ps", name="pf12f")
            pf12 = pf12f[:, 0:RHSW]
            for fc in range(FC):
                nc.tensor.matmul(
                    pf12[:],
                    lhsT=xT_sb[:, fc, jt * P:(jt + 1) * P],
                    rhs=rhs_sb[:, fc, :],
                    start=(fc == 0), stop=(fc == FC - 1))
            nc.vector.tensor_copy(f2col[:, jt, :], pf12[:, 0:H])
            nc.scalar.copy(whaug[:, jt, :], pf12[:, H:RHSW])

        for h in range(1, H):
            f1b_bcast(h)

        # f1 per-partition columns (for epilogue w)
        nc.vector.tensor_copy(identb_sb[:], identf_sb[:])
        for ib in range(IC):
            pt = psB.tile([P, H], bf16, tag="ep", name="pt")
            nc.tensor.transpose(pt[:], f1row_bf[:, ib * P:(ib + 1) * P],
                                identb_sb[0:H, 0:H])
            nc.vector.tensor_copy(f1colT[:, ib, :], pt[:])

        # ---- mask emitter: per-batch engine pattern.
        # 'A': 4 ACT sigmoids + DVE is_gt; 'V': 4 DVE 4x tensor_scalar +
        # DVE is_gt; 'P': 4 Pool fused stt.
        bat_i = [0]

        def emit_masks(u, f2c, f2cK, patv):
            Xs = []
            for b in range(NB):
                jt0 = b * JB
                kind = patv[bat_i[0] % len(patv)]
                bat_i[0] += 1
                X = sc_t.tile([P, JB, R], bf16, tag="xs")
                for q in range(JB):
                    jt = jt0 + q
                    if kind in ("A", "B"):
                        nc.scalar.activation(
                            X[:, q, :], f1b_all[:, u, :], AF.Sigmoid,
                            bias=f2cK[:, jt:jt + 1], scale=KSTEEP)
                    else:
                        nc.vector.tensor_scalar(
                            X[:, q, :], f1b_all[:, u, :],
                            f2c[:, jt:jt + 1], 0.0,
                            op0=ALU.add, op1=ALU.is_ge)
                if kind in ("B", "W"):
                    # per-tile Pool multiplies: finer granularity releases
                    # each tile to the PE as soon as it is masked
                    for q in range(JB):
                        jt = jt0 + q
                        nc.gpsimd.tensor_tensor(
                            X[:, q, :], X[:, q, :], adjT[:, jt, :],
                            op=ALU.mult)
                else:
                    nc.vector.tensor_tensor(
                        X[:], X[:], adjT[:, jt0:jt0 + JB, :], op=ALU.mult)
                Xs.append(X)
            return Xs

        # attention matmuls (flipped): X is the stationary operand; each
        # (unit, iblock) chain owns one PSUM bank, B/D halves share it.
        def emit_mms(patts, Xs, whB_u, whD_u):
            for b in range(NB):
                X = Xs[b]
                for q in range(JB):
                    jt = b * JB + q
                    for ib in range(IC):
                        lw = X[:, q, ib * P:(ib + 1) * P]
                        nc.tensor.matmul(
                            patts[ib][:, 0, :], lhsT=lw, rhs=whB_u(jt),
                            start=(jt == 0), stop=False)
                        nc.tensor.matmul(
                            patts[ib][:, 1, :], lhsT=lw, rhs=whD_u(jt),
                            start=False, stop=(jt == JT - 1))

        # masks for unit 0 go out BEFORE the scale chain so DVE/ACT/Pool
        # fill buffers while whB/whD are being scaled
        Xs0 = emit_masks(0, f2col[:, :, 0], f2colK[:, :, 0], pat0)

        # biases off f1max only (constant-slack M): BD = -.8f1max-FSLACK,
        # BW = .8f1max; the B-column bias is the constant -FSLACK.
        bias_cols = consts.tile([H, NBIAS], fp32)
        nc.vector.memset(bias_cols[:, BB:BB + 1], -FSLACK)
        nc.vector.tensor_scalar(bias_cols[:, BD:BD + 1], f1max[:], -0.8,
                                -FSLACK, op0=ALU.mult, op1=ALU.add)
        nc.vector.tensor_scalar_mul(bias_cols[:, BW:BW + 1], f1max[:], 0.8)
        bias_row = consts.tile([1, H, NBIAS], fp32)
        nc.scalar.dma_start(bias_row[:], bias_cols[:])
        pba = psB.tile([P, H * NBIAS], fp32, tag="ep", name="pba")
        nc.tensor.matmul(pba[:], lhsT=onesf[:],
                         rhs=bias_row[:].rearrange("o h e -> o (h e)"),
                         start=True, stop=True)
        bias_all = consts.tile([P, H, NBIAS], fp32)
        nc.vector.tensor_copy(
            bias_all[:], pba[:].rearrange("p (h e) -> p h e", e=NBIAS))
        nc.vector.tensor_scalar_mul(f2colK[:], f2col[:], KSTEEP)

        # exps (ACT): B/D columns + per-partition w columns
        for h in range(H):
            nc.scalar.activation(bcol[:, :, h], f2col[:, :, h], AF.Exp,
                                 bias=bias_all[:, h, BB:BB + 1], scale=1.0)
            nc.scalar.activation(dcol[:, :, h], f2col[:, :, h], AF.Exp,
                                 bias=bias_all[:, h, BD:BD + 1], scale=ALPHA)
            nc.scalar.activation(w_col[:, :, h], f1colT[:, :, h], AF.Exp,
                                 bias=bias_all[:, h, BW:BW + 1], scale=-0.8)


        # scale whaug into whB / whD2 (broadcast B/D along e):
        # B-half on DVE, D-half on Pool
        whD2v = whD2[:].rearrange("p t (u e) -> p t u e", e=E)

        def scale_bd(u0, u1, src, bcs, dcs):
            for u in range(u0, u1):
                s = src(u - u0)
                a0, a1 = broadcast_tensor_aps(s, bcs[:, :, u - u0:u - u0 + 1])
                nc.vector.tensor_tensor(whB[:, :, u, 0:D], a0, a1,
                                        op=ALU.mult)
                a0, a1 = broadcast_tensor_aps(s, dcs[:, :, u - u0:u - u0 + 1])
                nc.gpsimd.tensor_tensor(whD2v[:, :, u, 0:D], a0, a1,
                                        op=ALU.mult)
            nc.vector.tensor_copy(
                whB[:, :, u0:u1, D:E],
                bcs[:].rearrange("p t (c o) -> p t c o", o=1))
            nc.vector.tensor_copy(
                whD2v[:, :, u0:u1, D:E],
                dcs[:].rearrange("p t (c o) -> p t c o", o=1))

        scale_bd(0, H, lambda c: whaug[:, :, c * D:(c + 1) * D], bcol, dcol)

        # ---- Q: complement branch, lhsT = adjT directly
        def emit_q(ib):
            pq = psQ.tile([P, H * E], fp32, tag="psq", name="pq")
            for jt in range(JT):
                nc.tensor.matmul(pq[:],
                                 lhsT=adjT[:, jt, ib * P:(ib + 1) * P],
                                 rhs=whD2[:, jt, 0:H * E],
                                 start=(jt == 0), stop=(jt == JT - 1))
            nc.scalar.copy(qsb[:, ib, :], pq[:])

        # Q = colsum - QBG/BIG; batched epilogue halves (first half is
        # emitted mid-loop so it overlaps units 5..7):
        # t = stB + w[i]*(Q - stD); h = t[:,0:16]/t[:,16]
        qv = qsb[:].rearrange("p c (u e) -> p c u e", e=E)
        hprev = hpre[:].rearrange("p c (u d) -> p c u d", d=D)


        def l1_epilogue(u0, u1, s):
            uu = u1 - u0
            x1a = wk1.tile([P, IC, uu, E], fp32, tag=f"x1a{s}")
            nc.vector.tensor_tensor(x1a[:], qv[:, :, u0:u1, :],
                                    st_all[:, :, u0:u1, 1, :],
                                    op=ALU.subtract)
            va = wk1.tile([P, IC, uu, E], fp32, tag=f"va{s}")
            a0, a1 = broadcast_tensor_aps(
                x1a[:],
                w_col[:, :, u0:u1].rearrange("p c (u o) -> p c u o", o=1))
            nc.vector.tensor_tensor(va[:], a0, a1, op=ALU.mult)
            ta = wk1.tile([P, IC, uu, E], fp32, tag=f"ta{s}")
            nc.vector.tensor_tensor(ta[:], st_all[:, :, u0:u1, 0, :], va[:],
                                    op=ALU.add)
            reca = wk1.tile([P, IC, uu, 1], fp32, tag=f"reca{s}")
            nc.vector.reciprocal(reca[:], ta[:, :, :, D:E])
            a0, a1 = broadcast_tensor_aps(ta[:, :, :, 0:D], reca[:])
            nc.vector.tensor_tensor(hprev[:, :, u0:u1, :], a0, a1,
                                    op=ALU.mult)


        # ---- layer-1 attention ----
        ep_eng = [0]
        for h in range(H):
            patts = [psATT.tile([P, 2, E], fp32, tag="att",
                                name=f"patt{h}_{ib}") for ib in range(IC)]
            Xs = Xs0 if h == 0 else emit_masks(h, f2col[:, :, h],
                                               f2colK[:, :, h], pat)
            emit_mms(patts,
                     Xs,
                     lambda jt, h=h: whB[:, jt, h, :],
                     lambda jt, h=h: whD2[:, jt, h * E:(h + 1) * E])
            for ib in range(IC):
                k = ep_eng[0] % 2
                ep_eng[0] += 1
                if k == 0:
                    nc.vector.tensor_copy(st_all[:, ib, h, :, :],
                                          patts[ib][:])
                else:
                    nc.scalar.copy(st_all[:, ib, h, :, :], patts[ib][:])
            if h < IC:
                emit_q(h)

        l1_epilogue(0, H // 2, 0)
        l1_epilogue(H // 2, H, 1)


        # ELU: elu(x) = max(x, min(exp(x)-1, 0))
        etile = wk1.tile([P, IC, HD], fp32, tag="etile")
        nc.scalar.activation(etile[:], hpre[:], AF.Exp, bias=0.0, scale=1.0)
        em = wk1.tile([P, IC, HD], fp32, tag="em")
        nc.vector.tensor_scalar(em[:], etile[:], 1.0, 0.0,
                                op0=ALU.subtract, op1=ALU.min)
        nc.vector.tensor_tensor(h_sb[:], hpre[:], em[:], op=ALU.max)

        # ---- layer 2 ----
        # transpose h blocks; g = h @ [Wout|w2o|w1o]
        g_loc = consts.tile([P, IC, C + 2], fp32)
        for ib in range(IC):
            pt = psB.tile([P, P], bf16, tag="ep", name="pth")
            nc.tensor.transpose(pt[:], h_sb[:, ib, :], identb_sb[:])
            if ib % 2 == 0:
                nc.scalar.copy(hT_sb[:, ib, :], pt[:])
            else:
                nc.vector.tensor_copy(hT_sb[:, ib, :], pt[:])
            pg = psB.tile([P, C + 2], fp32, tag="ep", name="pg")
            nc.tensor.matmul(pg[:], lhsT=hT_sb[:, ib, :], rhs=Wo2_sb[:],
                             start=True, stop=True)
            if ib % 2 == 0:
                nc.vector.tensor_copy(g_loc[:, ib, :], pg[:])
            else:
                nc.scalar.copy(g_loc[:, ib, :], pg[:])

        # gather g = [Wh2 | f2o] for all nodes
        gsrc = dram.tile([R, C + 1], fp32)
        nc.sync.dma_start(
            gsrc[:].rearrange("(c p) e -> p c e", p=P),
            g_loc[:, :, 0:C + 1])
        gdst = dram.tile([n, C + 1], fp32)
        if no_collective:
            # flat stand-in copies (same bytes as the AllGather), spread
            # over the three DMA queues
            for k in range(ncores):
                qeng[k % len(qeng)].dma_start(
                    gdst[k * R:(k + 1) * R, :], gsrc[:])
        else:
            nc.gpsimd.collective_compute(
                "AllGather", ALU.bypass,
                replica_groups=[list(range(ncores))],
                ins=[gsrc.opt()], outs=[gdst.opt()])

        g17_sb = consts.tile([P, JT, C + 1], fp32)
        for k in range(ncores):
            qeng[k % len(qeng)].dma_start(
                g17_sb[:, k * IC:(k + 1) * IC, :],
                gdst[k * R:(k + 1) * R, :]
                .rearrange("(t p) e -> p t e", p=P))
        g2_sb = g17_sb[:, :, 0:C]
        f2ocol = g17_sb[:, :, C:C + 1]

        # f1o row: w1o^T @ hT blocks
        pf1o = psB.tile([1, IC, P], fp32, tag="ep", name="pf1o")
        for ib in range(IC):
            nc.tensor.matmul(pf1o[:, ib, :], lhsT=w1o_sb[:],
                             rhs=hT_sb[:, ib, :], start=True, stop=True)
        f1orow = consts.tile([1, R], fp32)
        nc.vector.tensor_copy(f1orow[:],
                              pf1o[:].rearrange("o c p -> o (c p)"))
        f1orow_bf = consts.tile([1, R], bf16)
        nc.vector.tensor_copy(f1orow_bf[:], f1orow[:])
        pf1obf = psA.tile([P, PSW], fp32, tag="ps", name="pf1obf")
        pf1ob = pf1obf[:, 0:R]
        nc.tensor.matmul(pf1ob[:], lhsT=onesb[:], rhs=f1orow_bf[:],
                         start=True, stop=True)
        nc.scalar.copy(f1b_all[:, U - 1, :], pf1ob[:])

        f1omax = consts.tile([1, 1], fp32)
        nc.vector.tensor_reduce(f1omax[:], f1orow[:], axis=AX.X, op=ALU.max)
        b2_cols = consts.tile([1, NBIAS], fp32)
        nc.vector.memset(b2_cols[:, BB:BB + 1], -FSLACK)
        nc.vector.tensor_scalar(b2_cols[:, BD:BD + 1], f1omax[:], -0.8,
                                -FSLACK, op0=ALU.mult, op1=ALU.add)
        nc.vector.tensor_scalar_mul(b2_cols[:, BW:BW + 1], f1omax[:], 0.8)
        pb2 = psB.tile([P, NBIAS], fp32, tag="ep", name="pb2")
        nc.tensor.matmul(pb2[:], lhsT=onesf[:], rhs=b2_cols[:],
                         start=True, stop=True)
        bias2 = consts.tile([P, NBIAS], fp32)
        nc.vector.tensor_copy(bias2[:], pb2[:])

        bcol2 = consts.tile([P, JT, 1], bf16)
        dcol2 = consts.tile([P, JT, 1], bf16)
        f2oK = consts.tile([P, JT, 1], fp32)
        nc.scalar.activation(bcol2[:], f2ocol[:], AF.Exp,
                             bias=bias2[:, BB:BB + 1], scale=1.0)
        nc.scalar.activation(dcol2[:], f2ocol[:], AF.Exp,
                             bias=bias2[:, BD:BD + 1], scale=ALPHA)
        nc.scalar.activation(w_col[:, :, U - 1], g_loc[:, :, C + 1], AF.Exp,
                             bias=bias2[:, BW:BW + 1], scale=-0.8)
        nc.vector.tensor_scalar_mul(f2oK[:], f2ocol[:], KSTEEP)
        # dummy Ln: pulls the natural_log_exp table load (which also covers
        # the final Exp) off the critical softmax chain
        lnw = wk1.tile([1, 1], fp32, tag="lnw")
        nc.scalar.activation(lnw[:], onesf[0:1, 0:1], AF.Ln, bias=0.0,
                             scale=1.0)

        scale_bd(U - 1, U, lambda c: g2_sb[:], bcol2, dcol2)

        # Q for unit 8
        q2sb = consts.tile([P, IC, E], fp32)
        for ib in range(IC):
            pqf = psQ.tile([P, H * E], fp32, tag="psq", name="pqf")
            pq2 = pqf[:, 0:E]
            for jt in range(JT):
                nc.tensor.matmul(pq2[:],
                                 lhsT=adjT[:, jt, ib * P:(ib + 1) * P],
                                 rhs=whD2[:, jt, H * E:U * E],
                                 start=(jt == 0), stop=(jt == JT - 1))
            nc.vector.tensor_copy(q2sb[:, ib, :], pq2[:])

        patts2 = [psATT.tile([P, 2, E], fp32, tag="att",
                             name=f"patt2_{ib}") for ib in range(IC)]
        Xs2 = emit_masks(U - 1, f2ocol[:, :, 0], f2oK[:, :, 0], pat2)
        emit_mms(patts2, Xs2,
                 lambda jt: whB[:, jt, U - 1, :],
                 lambda jt: whD2[:, jt, (U - 1) * E:U * E])
        st2 = work.tile([P, IC, 2, E], fp32, tag="st2")
        for ib in range(IC):
            if ib % 2 == 0:
                nc.vector.tensor_copy(st2[:, ib, :, :], patts2[ib][:])
            else:
                nc.scalar.copy(st2[:, ib, :, :], patts2[ib][:])
        x2 = wk1.tile([P, IC, E], fp32, tag="x2")
        nc.vector.tensor_tensor(x2[:], q2sb[:], st2[:, :, 1, :],
                                op=ALU.subtract)
        v2 = wk1.tile([P, IC, E], fp32, tag="v2")
        a0, a1 = broadcast_tensor_aps(
            x2[:], w_col[:, :, U - 1].rearrange("p (c o) -> p c o", o=1))
        nc.vector.tensor_tensor(v2[:], a0, a1, op=ALU.mult)
        t2 = wk1.tile([P, IC, E], fp32, tag="t2")
        nc.vector.tensor_tensor(t2[:], st2[:, :, 0, :], v2[:], op=ALU.add)
        rec2 = wk1.tile([P, IC, 1], fp32, tag="rec2")
        nc.vector.reciprocal(rec2[:], t2[:, :, D:E])
        z = wk1.tile([P, IC, C], fp32, tag="z")
        a0, a1 = broadcast_tensor_aps(t2[:, :, 0:D], rec2[:])
        nc.vector.tensor_tensor(z[:], a0, a1, op=ALU.mult)

        # log_softmax along free dim (no transposes needed)
        negmx = wk1.tile([P, IC, 1], fp32, tag="negmx")
        nc.vector.tensor_reduce(negmx[:], z[:], axis=AX.X, op=ALU.max,
                                negate=True)
        zs = wk1.tile([P, IC, C], fp32, tag="zs")
        a0, a1 = broadcast_tensor_aps(z[:], negmx[:])
        nc.vector.tensor_tensor(zs[:], a0, a1, op=ALU.add)
        ez = wk1.tile([P, IC, C], fp32, tag="ez")
        nc.scalar.activation(ez[:], zs[:], AF.Exp, bias=0.0, scale=1.0)
        sume = wk1.tile([P, IC, 1], fp32, tag="sume")
        nc.vector.tensor_reduce(sume[:], ez[:], axis=AX.X, op=ALU.add)
        lns = wk1.tile([P, IC, 1], fp32, tag="lns")
        nc.scalar.activation(lns[:], sume[:], AF.Ln, bias=0.0, scale=1.0)
        zo = wk1.tile([P, IC, C], fp32, tag="zo")
        a0, a1 = broadcast_tensor_aps(zs[:], lns[:])
        nc.vector.tensor_tensor(zo[:], a0, a1, op=ALU.subtract)
        nc.sync.dma_start(out.rearrange("(c p) e -> p c e", p=P), zo[:])

    nc.compile()
    return nc


def prep_inputs(x, adj, W1, a1, Wout, a_out, n=4096, ncores=NCORES):
    """Host-side prep: slice + transpose + bf16 cast + weight folds."""
    R = n // ncores
    x = np.asarray(x, np.float32)
    adj = np.asarray(adj)
    W1 = np.asarray(W1, np.float32)
    a1 = np.asarray(a1, np.float32)
    Wout = np.asarray(Wout, np.float32)
    a_out = np.asarray(a_out, np.float32)

    xT = np.ascontiguousarray(x.T).astype(ml_dtypes.float8_e4m3fn)
    W1a = W1.transpose(1, 0, 2).reshape(F, H * D)
    w2c = np.einsum("hfd,hd->fh", W1, a1[:, D:])
    w1c = np.einsum("hfd,hd->fh", W1, a1[:, :D])
    w2o = Wout @ a_out[C:]
    w1o = Wout @ a_out[:C]
    Wo2 = np.ascontiguousarray(np.concatenate(
        [Wout, w2o[:, None], w1o[:, None]], axis=1)).astype(BF16)
    identf = np.eye(P, dtype=np.float32)

    adjf = adj.astype(np.float32)
    in_maps = []
    for k in range(ncores):
        rows = slice(k * R, (k + 1) * R)
        xwk = np.concatenate([x[rows].T, w2c, W1a, w1c], axis=1)
        JTl = n // P
        at = adjf[rows].astype(BF16).T                     # [n, R]
        at = np.ascontiguousarray(
            at.reshape(JTl, P, R).transpose(1, 0, 2)).reshape(P, JTl * R)
        in_maps.append({
            "xT": xT,
            "xw": np.ascontiguousarray(xwk).astype(BF16),
            "adjt": at,
            "Wo2": Wo2,
            "identf": identf,
        })
    return in_maps


_cached = {}


def kernel(x, adj, W1, a1, Wout, a_out):
    n = x.shape[0]
    if n not in _cached:
        _cached[n] = build_gat(n=n)
    nc = _cached[n]
    in_maps = prep_inputs(x, adj, W1, a1, Wout, a_out, n=n)
    res = run_bass_kernel_spmd(nc, in_maps, core_ids=list(range(NCORES)))
    outs = [res.results[k]["out"] for k in range(NCORES)]
    return np.concatenate(outs, axis=0)


# revision 42
# speedup vs baseline: 1.5637x; 1.0003x over previous
"""GAT (2-layer graph attention network) Trainium2 Bass kernel — v3.

N=4096 nodes, F=512 feats; layer1: 8 heads x 16 (ELU, concat); layer2:
1 head 128->16; log_softmax. Dense masked attention, row-parallel over
8 cores (core k owns rows [512k, 512k+512)).

Score restructure: leaky(s) = max(s, 0.2s) with s = f1[i]+f2[j] splits
softmax numerators into two rank-separable branches selected by
S = [s >= 0]:
  P = exp(leaky(s)-M)*adj = S*adj*A[i]*B[j] + (1-S)*adj*C[i]*D[j]
with A=exp(f1-f1max), B=exp(f2+f1max-M), C=exp(.2(f1-f1max)),
D=exp(.2 f2+.2 f1max-M), M = f1max + FSLACK.  The constant-slack M
(instead of f1max+f2max) needs no global f2 reduction, so the bias /
exp / scale chain leaves the critical path; the uniform exp(-FSLACK)
factor cancels in the softmax division.

Masks: a tile's 0/1 mask X = S*adj is a first op producing S
(DVE tensor_scalar 4x is_ge, or ACT steep Sigmoid) followed by a
batched multiply against adjT (DVE 2x or Pool), spread across the
three engines by a tunable per-batch pattern (kinds: A=ACT+DVE,
B=ACT+Pool, V=DVE+DVE, W=DVE+Pool).

Attention matmuls are FLIPPED: X[j, i-block] is the PE stationary
operand and the scaled factors whB/whD [j, 17] are the moving operand
(out free size 17, not 512); each (unit, i-block) accumulation chain
owns one PSUM bank, B/D halves share the chain.  Epilogue scalars
(w[i], 1/den) are per-partition; log_softmax needs no transposes.
The complement branch Q = sum_j adj*D*Wh uses lhsT=adjT directly.
Layer 2 gathers
the f2o column separately from the Wh2 block so unit-8 mask compute
overlaps the main gather.
"""

import os
import sys
import contextlib

for _p in ("/opt/trn_rl_repo",):
    if _p not in sys.path and os.path.isdir(_p):
        sys.path.insert(0, _p)

import numpy as np
import ml_dtypes

import concourse.bass as bass
import concourse.bacc as bacc
import concourse.tile as tile
from concourse import mybir
from concourse.bass import broadcast_tensor_aps
from concourse.bass_utils import run_bass_kernel_spmd

BF16 = ml_dtypes.bfloat16
ALPHA = 0.2

F = 512      # input features
H = 8        # heads (layer 1)
D = 16       # per-head dim
C = 16       # classes
P = 128      # partitions
NCORES = 8
E = D + 1    # Wh columns + ones column (17)
U = H + 1    # units: 8 layer-1 heads + 1 layer-2 head
HD = H * D   # 128

BB, BD, BW = 0, 1, 2   # bias cols: -FSLACK, -.8*f1max-FSLACK, .8*f1max
NBIAS = 3
KSTEEP = 1024.0        # sigmoid / score steepness
FSLACK = 30.0          # constant softmax shift slack (M = f1max + FSLACK)


def build_gat(n=4096, ncores=NCORES, dbg=False, no_collective=False,
              pat=("A", "V", "B", "A", "W", "V", "A", "V"),
              pat0=("V", "W", "V", "W", "V", "V", "W", "V"),
              pat2=("V", "A", "W", "V", "A", "W", "V", "A"),
              ttbufs=10, jb=4):
    """Build the SPMD Bass program for one core (row-parallel)."""
    R = n // ncores          # rows per core
    IC = R // P              # i-blocks per core
    JT = n // P              # j-tiles (partition tiles of full node dim)
    FC = F // P              # f chunks
    JB = jb                  # j-tiles per mask batch
    NB = JT // JB
    NCH = n // FC            # xT column-chunk width
    RHSW = H + HD            # fused f2|Wh matmul width (136)
    XWC = R + RHSW + H       # combined xTm|rhs|w1c load width
    PSW = max(R, RHSW)       # shared psA slot width
    assert R % P == 0 and JT % JB == 0

    fp32 = mybir.dt.float32
    bf16 = mybir.dt.bfloat16

    nc = bacc.Bacc("TRN2", target_bir_lowering=False, debug=dbg,
                   num_devices=ncores)

    xT = nc.dram_tensor("xT", [F, n], mybir.dt.float8e4,
                        kind="ExternalInput").ap()
    xw = nc.dram_tensor("xw", [F, XWC], bf16, kind="ExternalInput").ap()
    adjt = nc.dram_tensor("adjt", [P, JT * R], bf16,
                          kind="ExternalInput").ap()
    Wo2 = nc.dram_tensor("Wo2", [HD, C + 2], bf16, kind="ExternalInput").ap()
    identf = nc.dram_tensor("identf", [P, P], fp32, kind="ExternalInput").ap()
    out = nc.dram_tensor("out", [R, C], fp32, kind="ExternalOutput").ap()

    AF = mybir.ActivationFunctionType
    ALU = mybir.AluOpType
    AX = mybir.AxisListType

    with tile.TileContext(nc) as tc, contextlib.ExitStack() as ctx:
        big = ctx.enter_context(tc.tile_pool(name="big", bufs=1))
        consts = ctx.enter_context(tc.tile_pool(name="consts", bufs=1))
        work = ctx.enter_context(tc.tile_pool(name="work", bufs=2))
        wk1 = ctx.enter_context(tc.tile_pool(name="wk1", bufs=1))
        sc_t = ctx.enter_context(tc.tile_pool(name="sc_t", bufs=ttbufs))
        psA = ctx.enter_context(tc.tile_pool(name="psA", bufs=2, space="PSUM"))
        psQ = ctx.enter_context(tc.tile_pool(name="psQ", bufs=1, space="PSUM"))
        psATT = ctx.enter_context(
            tc.tile_pool(name="psATT", bufs=4, space="PSUM"))
        psB = ctx.enter_context(tc.tile_pool(name="psB", bufs=1, space="PSUM"))
        dram = ctx.enter_context(tc.tile_pool(name="dram", bufs=1,
                                              space="DRAM"))

        # ---- const / persistent loads (spread across DMA queues).
        # xT is loaded in COLUMN chunks so pf12 chain jt can close as soon
        # as the chunk holding its columns lands.
        # big loads ride the SP + Pool DMA queues only; the ACT queue is
        # reserved for small latency-critical transfers (Wo2, f1row_1,
        # bias_row, the layer-2 gathers)
        bigq = [nc.sync, nc.gpsimd]
        qeng = [nc.sync, nc.scalar, nc.gpsimd]
        identf_sb = consts.tile([P, P], fp32)
        nc.gpsimd.dma_start(identf_sb[:], identf)
        xw_sb = consts.tile([P, FC, XWC], bf16)
        nc.sync.dma_start(xw_sb[:], xw.rearrange("(c p) n -> p c n", p=P))
        xT_sb = big.tile([P, FC, n], mybir.dt.float8e4, tag="xtslot")
        adjT = big.tile([P, JT, R], bf16)
        ACH = JT // 4
        Wo2_sb = consts.tile([P, C + 2], bf16)
        nc.scalar.dma_start(Wo2_sb[:], Wo2)
        # interleave xT column chunks with adjBG chunks: pf12 chain jt and
        # the mask tiles for jt both become runnable ~3us after their chunk
        for c in range(4):
            bigq[c % 2].dma_start(
                xT_sb[:, :, c * NCH:(c + 1) * NCH],
                xT.rearrange("(c p) n -> p c n", p=P)
                [:, :, c * NCH:(c + 1) * NCH])
            bigq[(c + 1) % 2].dma_start(
                adjT[:, c * ACH:(c + 1) * ACH, :],
                adjt[:, c * ACH * R:(c + 1) * ACH * R]
                .rearrange("p (t r) -> p t r", r=R))
        xTm_sb = xw_sb[:, :, 0:R]
        rhs_sb = xw_sb[:, :, R:R + RHSW]          # [w2c(8) | W1a(128)]
        w1c_sb = xw_sb[:, :, R + RHSW:XWC]
        w1o_sb = Wo2_sb[:, C + 1:C + 2]

        identb_sb = consts.tile([P, P], bf16)
        onesb = consts.tile([1, P], bf16)
        nc.vector.memset(onesb[:], 1.0)
        onesf = consts.tile([1, P], fp32)
        nc.vector.memset(onesf[:], 1.0)
        onescol = consts.tile([P, 1], bf16)
        nc.vector.memset(onescol[:], 1.0)

        # persistent intermediates
        whaug = big.tile([P, JT, HD], bf16)       # Wh columns per j
        whB = big.tile([P, JT, U, E], bf16)       # B_j*[Wh_u|1]
        whD2 = big.tile([P, JT, U * E], bf16)     # D_j*[Wh_u|1], (u,e)-contig
        f1b_all = big.tile([P, U, R], bf16)       # f1[i] bcast on partitions
        f2col = big.tile([P, JT, H], fp32)        # f2[j] (layer 1)
        f2colK = big.tile([P, JT, H], fp32)       # KSTEEP*f2[j]
        bcol = consts.tile([P, JT, H], bf16)
        dcol = consts.tile([P, JT, H], bf16)
        w_col = consts.tile([P, IC, U], fp32)     # exp(.8(f1max-f1[i]))
        hpre = big.tile([P, IC, HD], fp32)        # layer-1 out pre-ELU
        h_sb = big.tile([P, IC, HD], bf16)        # post-ELU
        hT_sb = big.tile([P, IC, P], bf16)        # transposed h blocks
        qsb = consts.tile([P, IC, H * E], fp32)   # Q for layer-1 units
        st_all = big.tile([P, IC, H, 2, E], fp32)  # drained attention psums
        f1colT = consts.tile([P, IC, H], fp32)    # f1[i] per-partition

        # ---- phase 1a: f1 row (own rows) ----
        pf1 = psB.tile([H, R], fp32, tag="ep")
        for fc in range(FC):
            nc.tensor.matmul(pf1[:], lhsT=w1c_sb[:, fc, :],
                             rhs=xTm_sb[:, fc, :],
                             start=(fc == 0), stop=(fc == FC - 1))
        f1row_sb = consts.tile([H, R], fp32)
        nc.vector.tensor_copy(f1row_sb[:], pf1[:])
        f1row_bf = consts.tile([H, R], bf16)
        nc.vector.tensor_copy(f1row_bf[:], f1row_sb[:])
        f1row_1 = consts.tile([1, H, R], bf16)
        nc.scalar.dma_start(f1row_1[:], f1row_bf[:])
        f1max = consts.tile([H, 1], fp32)
        nc.vector.tensor_reduce(f1max[:], f1row_sb[:], axis=AX.X, op=ALU.max)

        # f1 broadcast for head 0 first (gates the first mask tiles)
        def f1b_bcast(h):
            pbf = psA.tile([P, PSW], fp32, tag="ps", name="pbf")
            pb = pbf[:, 0:R]
            nc.tensor.matmul(pb[:], lhsT=onesb[:],
                             rhs=f1row_1[0:1, h, :], start=True, stop=True)
            nc.scalar.copy(f1b_all[:, h, :], pb[:])
        f1b_bcast(0)

        # ---- phase 1b: fused f2|Wh matmuls over all j ----
        for jt in range(JT):
            pf12f = psA.tile([P, PSW], fp32, tag="ps", name="pf12f")
            pf12 = pf12f[:, 0:RHSW]
            for fc in range(FC):
                nc.tensor.matmul(
                    pf12[:],
                    lhsT=xT_sb[:, fc, jt * P:(jt + 1) * P],
                    rhs=rhs_sb[:, fc, :],
                    start=(fc == 0), stop=(fc == FC - 1))
            nc.vector.tensor_copy(f2col[:, jt, :], pf12[:, 0:H])
            nc.scalar.copy(whaug[:, jt, :], pf12[:, H:RHSW])

        for h in range(1, H):
            f1b_bcast(h)

        # f1 per-partition columns (for epilogue w)
        nc.vector.tensor_copy(identb_sb[:], identf_sb[:])
        for ib in range(IC):
            pt = psB.tile([P, H], bf16, tag="ep", name="pt")
            nc.tensor.transpose(pt[:], f1row_bf[:, ib * P:(ib + 1) * P],
                                identb_sb[0:H, 0:H])
            nc.vector.tensor_copy(f1colT[:, ib, :], pt[:])

        # ---- mask emitter: per-batch engine pattern.
        # 'A': 4 ACT sigmoids + DVE is_gt; 'V': 4 DVE 4x tensor_scalar +
        # DVE is_gt; 'P': 4 Pool fused stt.
        bat_i = [0]

        def emit_masks(u, f2c, f2cK, patv):
            Xs = []
            for b in range(NB):
                jt0 = b * JB
                kind = patv[bat_i[0] % len(patv)]
                bat_i[0] += 1
                X = sc_t.tile([P, JB, R], bf16, tag="xs")
                for q in range(JB):
                    jt = jt0 + q
                    if kind in ("A", "B"):
                        nc.scalar.activation(
                            X[:, q, :], f1b_all[:, u, :], AF.Sigmoid,
                            bias=f2cK[:, jt:jt + 1], scale=KSTEEP)
                    else:
                        nc.vector.tensor_scalar(
                            X[:, q, :], f1b_all[:, u, :],
                            f2c[:, jt:jt + 1], 0.0,
                            op0=ALU.add, op1=ALU.is_ge)
                if kind in ("B", "W"):
                    # per-tile Pool multiplies: finer granularity releases
                    # each tile to the PE as soon as it is masked
                    for q in range(JB):
                        jt = jt0 + q
                        nc.gpsimd.tensor_tensor(
                            X[:, q, :], X[:, q, :], adjT[:, jt, :],
                            op=ALU.mult)
                else:
                    nc.vector.tensor_tensor(
                        X[:], X[:], adjT[:, jt0:jt0 + JB, :], op=ALU.mult)
                Xs.append(X)
            return Xs

        # attention matmuls (flipped): X is the stationary operand; each
        # (unit, iblock) chain owns one PSUM bank, B/D halves share it.
        def emit_mms(patts, Xs, whB_u, whD_u):
            for b in range(NB):
                X = Xs[b]
                for q in range(JB):
                    jt = b * JB + q
                    for ib in range(IC):
                        lw = X[:, q, ib * P:(ib + 1) * P]
                        nc.tensor.matmul(
                            patts[ib][:, 0, :], lhsT=lw, rhs=whB_u(jt),
                            start=(jt == 0), stop=False)
                        nc.tensor.matmul(
                            patts[ib][:, 1, :], lhsT=lw, rhs=whD_u(jt),
                            start=False, stop=(jt == JT - 1))

        # masks for unit 0 go out BEFORE the scale chain so DVE/ACT/Pool
        # fill buffers while whB/whD are being scaled
        Xs0 = emit_masks(0, f2col[:, :, 0], f2colK[:, :, 0], pat0)

        # biases off f1max only (constant-slack M): BD = -.8f1max-FSLACK,
        # BW = .8f1max; the B-column bias is the constant -FSLACK.
        bias_cols = consts.tile([H, NBIAS], fp32)
        nc.vector.memset(bias_cols[:, BB:BB + 1], -FSLACK)
        nc.vector.tensor_scalar(bias_cols[:, BD:BD + 1], f1max[:], -0.8,
                                -FSLACK, op0=ALU.mult, op1=ALU.add)
        nc.vector.tensor_scalar_mul(bias_cols[:, BW:BW + 1], f1max[:], 0.8)
        bias_row = consts.tile([1, H, NBIAS], fp32)
        nc.scalar.dma_start(bias_row[:], bias_cols[:])
        pba = psB.tile([P, H * NBIAS], fp32, tag="ep", name="pba")
        nc.tensor.matmul(pba[:], lhsT=onesf[:],
                         rhs=bias_row[:].rearrange("o h e -> o (h e)"),
                         start=True, stop=True)
        bias_all = consts.tile([P, H, NBIAS], fp32)
        nc.vector.tensor_copy(
            bias_all[:], pba[:].rearrange("p (h e) -> p h e", e=NBIAS))
        nc.vector.tensor_scalar_mul(f2colK[:], f2col[:], KSTEEP)

        # exps (ACT): B/D columns + per-partition w columns
        for h in range(H):
            nc.scalar.activation(bcol[:, :, h], f2col[:, :, h], AF.Exp,
                                 bias=bias_all[:, h, BB:BB + 1], scale=1.0)
            nc.scalar.activation(dcol[:, :, h], f2col[:, :, h], AF.Exp,
                                 bias=bias_all[:, h, BD:BD + 1], scale=ALPHA)
            nc.scalar.activation(w_col[:, :, h], f1colT[:, :, h], AF.Exp,
                                 bias=bias_all[:, h, BW:BW + 1], scale=-0.8)


        # scale whaug into whB / whD2 (broadcast B/D along e):
        # B-half on DVE, D-half on Pool
        whD2v = whD2[:].rearrange("p t (u e) -> p t u e", e=E)

        def scale_bd(u0, u1, src, bcs, dcs):
            for u in range(u0, u1):
                s = src(u - u0)
                a0, a1 = broadcast_tensor_aps(s, bcs[:, :, u - u0:u - u0 + 1])
                nc.vector.tensor_tensor(whB[:, :, u, 0:D], a0, a1,
                                        op=ALU.mult)
                a0, a1 = broadcast_tensor_aps(s, dcs[:, :, u - u0:u - u0 + 1])
                nc.gpsimd.tensor_tensor(whD2v[:, :, u, 0:D], a0, a1,
                                        op=ALU.mult)
            nc.vector.tensor_copy(
                whB[:, :, u0:u1, D:E],
                bcs[:].rearrange("p t (c o) -> p t c o", o=1))
            nc.vector.tensor_copy(
                whD2v[:, :, u0:u1, D:E],
                dcs[:].rearrange("p t (c o) -> p t c o", o=1))

        scale_bd(0, H, lambda c: whaug[:, :, c * D:(c + 1) * D], bcol, dcol)

        # ---- Q: complement branch, lhsT = adjT directly
        def emit_q(ib):
            pq = psQ.tile([P, H * E], fp32, tag="psq", name="pq")
            for jt in range(JT):
                nc.tensor.matmul(pq[:],
                                 lhsT=adjT[:, jt, ib * P:(ib + 1) * P],
                                 rhs=whD2[:, jt, 0:H * E],
                                 start=(jt == 0), stop=(jt == JT - 1))
            nc.scalar.copy(qsb[:, ib, :], pq[:])

        # Q = colsum - QBG/BIG; batched epilogue halves (first half is
        # emitted mid-loop so it overlaps units 5..7):
        # t = stB + w[i]*(Q - stD); h = t[:,0:16]/t[:,16]
        qv = qsb[:].rearrange("p c (u e) -> p c u e", e=E)
        hprev = hpre[:].rearrange("p c (u d) -> p c u d", d=D)


        def l1_epilogue(u0, u1, s):
            uu = u1 - u0
            x1a = wk1.tile([P, IC, uu, E], fp32, tag=f"x1a{s}")
            nc.vector.tensor_tensor(x1a[:], qv[:, :, u0:u1, :],
                                    st_all[:, :, u0:u1, 1, :],
                                    op=ALU.subtract)
            va = wk1.tile([P, IC, uu, E], fp32, tag=f"va{s}")
            a0, a1 = broadcast_tensor_aps(
                x1a[:],
                w_col[:, :, u0:u1].rearrange("p c (u o) -> p c u o", o=1))
            nc.vector.tensor_tensor(va[:], a0, a1, op=ALU.mult)
            ta = wk1.tile([P, IC, uu, E], fp32, tag=f"ta{s}")
            nc.vector.tensor_tensor(ta[:], st_all[:, :, u0:u1, 0, :], va[:],
                                    op=ALU.add)
            reca = wk1.tile([P, IC, uu, 1], fp32, tag=f"reca{s}")
            nc.vector.reciprocal(reca[:], ta[:, :, :, D:E])
            a0, a1 = broadcast_tensor_aps(ta[:, :, :, 0:D], reca[:])
            nc.vector.tensor_tensor(hprev[:, :, u0:u1, :], a0, a1,
                                    op=ALU.mult)


        # ---- layer-1 attention ----
        ep_eng = [0]
        for h in range(H):
            patts = [psATT.tile([P, 2, E], fp32, tag="att",
                                name=f"patt{h}_{ib}") for ib in range(IC)]
            Xs = Xs0 if h == 0 else emit_masks(h, f2col[:, :, h],
                                               f2colK[:, :, h], pat)
            emit_mms(patts,
                     Xs,
                     lambda jt, h=h: whB[:, jt, h, :],
                     lambda jt, h=h: whD2[:, jt, h * E:(h + 1) * E])
            for ib in range(IC):
                k = ep_eng[0] % 2
                ep_eng[0] += 1
                if k == 0:
                    nc.vector.tensor_copy(st_all[:, ib, h, :, :],
                                          patts[ib][:])
                else:
                    nc.scalar.copy(st_all[:, ib, h, :, :], patts[ib][:])
            if h < IC:
                emit_q(h)

        l1_epilogue(0, H // 2, 0)
        l1_epilogue(H // 2, H, 1)


        # ELU: elu(x) = max(x, min(exp(x)-1, 0))
        etile = wk1.tile([P, IC, HD], fp32, tag="etile")
        nc.scalar.activation(etile[:], hpre[:], AF.Exp, bias=0.0, scale=1.0)
        em = wk1.tile([P, IC, HD], fp32, tag="em")
        nc.vector.tensor_scalar(em[:], etile[:], 1.0, 0.0,
                                op0=ALU.subtract, op1=ALU.min)
        nc.vector.tensor_tensor(h_sb[:], hpre[:], em[:], op=ALU.max)

        # ---- layer 2 ----
        # transpose h blocks; g = h @ [Wout|w2o|w1o]
        g_loc = consts.tile([P, IC, C + 2], fp32)
        for ib in range(IC):
            pt = psB.tile([P, P], bf16, tag="ep", name="pth")
            nc.tensor.transpose(pt[:], h_sb[:, ib, :], identb_sb[:])
            if ib % 2 == 0:
                nc.scalar.copy(hT_sb[:, ib, :], pt[:])
            else:
                nc.vector.tensor_copy(hT_sb[:, ib, :], pt[:])
            pg = psB.tile([P, C + 2], fp32, tag="ep", name="pg")
            nc.tensor.matmul(pg[:], lhsT=hT_sb[:, ib, :], rhs=Wo2_sb[:],
                             start=True, stop=True)
            if ib % 2 == 0:
                nc.vector.tensor_copy(g_loc[:, ib, :], pg[:])
            else:
                nc.scalar.copy(g_loc[:, ib, :], pg[:])

        # gather g = [Wh2 | f2o] for all nodes
        gsrc = dram.tile([R, C + 1], fp32)
        nc.sync.dma_start(
            gsrc[:].rearrange("(c p) e -> p c e", p=P),
            g_loc[:, :, 0:C + 1])
        gdst = dram.tile([n, C + 1], fp32)
        if no_collective:
            # flat stand-in copies (same bytes as the AllGather), spread
            # over the three DMA queues
            for k in range(ncores):
                qeng[k % len(qeng)].dma_start(
                    gdst[k * R:(k + 1) * R, :], gsrc[:])
        else:
            nc.gpsimd.collective_compute(
                "AllGather", ALU.bypass,
                replica_groups=[list(range(ncores))],
                ins=[gsrc.opt()], outs=[gdst.opt()])

        g17_sb = consts.tile([P, JT, C + 1], fp32)
        for k in range(ncores):
            qeng[k % len(qeng)].dma_start(
                g17_sb[:, k * IC:(k + 1) * IC, :],
                gdst[k * R:(k + 1) * R, :]
                .rearrange("(t p) e -> p t e", p=P))
        g2_sb = g17_sb[:, :, 0:C]
        f2ocol = g17_sb[:, :, C:C + 1]

        # f1o row: w1o^T @ hT blocks
        pf1o = psB.tile([1, IC, P], fp32, tag="ep", name="pf1o")
        for ib in range(IC):
            nc.tensor.matmul(pf1o[:, ib, :], lhsT=w1o_sb[:],
                             rhs=hT_sb[:, ib, :], start=True, stop=True)
        f1orow = consts.tile([1, R], fp32)
        nc.vector.tensor_copy(f1orow[:],
                              pf1o[:].rearrange("o c p -> o (c p)"))
        f1orow_bf = consts.tile([1, R], bf16)
        nc.vector.tensor_copy(f1orow_bf[:], f1orow[:])
        pf1obf = psA.tile([P, PSW], fp32, tag="ps", name="pf1obf")
        pf1ob = pf1obf[:, 0:R]
        nc.tensor.matmul(pf1ob[:], lhsT=onesb[:], rhs=f1orow_bf[:],
                         start=True, stop=True)
        nc.scalar.copy(f1b_all[:, U - 1, :], pf1ob[:])

        f1omax = consts.tile([1, 1], fp32)
        nc.vector.tensor_reduce(f1omax[:], f1orow[:], axis=AX.X, op=ALU.max)
        b2_cols = consts.tile([1, NBIAS], fp32)
        nc.vector.memset(b2_cols[:, BB:BB + 1], -FSLACK)
        nc.vector.tensor_scalar(b2_cols[:, BD:BD + 1], f1omax[:], -0.8,
                                -FSLACK, op0=ALU.mult, op1=ALU.add)
        nc.vector.tensor_scalar_mul(b2_cols[:, BW:BW + 1], f1omax[:], 0.8)
        pb2 = psB.tile([P, NBIAS], fp32, tag="ep", name="pb2")
        nc.tensor.matmul(pb2[:], lhsT=onesf[:], rhs=b2_cols[:],
                         start=True, stop=True)
        bias2 = consts.tile([P, NBIAS], fp32)
        nc.vector.tensor_copy(bias2[:], pb2[:])

        bcol2 = consts.tile([P, JT, 1], bf16)
        dcol2 = consts.tile([P, JT, 1], bf16)
        f2oK = consts.tile([P, JT, 1], fp32)
        nc.scalar.activation(bcol2[:], f2ocol[:], AF.Exp,
                             bias=bias2[:, BB:BB + 1], scale=1.0)
        nc.scalar.activation(dcol2[:], f2ocol[:], AF.Exp,
                             bias=bias2[:, BD:BD + 1], scale=ALPHA)
        nc.scalar.activation(w_col[:, :, U - 1], g_loc[:, :, C + 1], AF.Exp,
                             bias=bias2[:, BW:BW + 1], scale=-0.8)
        nc.vector.tensor_scalar_mul(f2oK[:], f2ocol[:], KSTEEP)
        # dummy Ln: pulls the natural_log_exp table load (which also covers
        # the final Exp) off the critical softmax chain
        lnw = wk1.tile([1, 1], fp32, tag="lnw")
        nc.scalar.activation(lnw[:], onesf[0:1, 0:1], AF.Ln, bias=0.0,
                             scale=1.0)

        scale_bd(U - 1, U, lambda c: g2_sb[:], bcol2, dcol2)

        # Q for unit 8
        q2sb = consts.tile([P, IC, E], fp32)
        for ib in range(IC):
            pqf = psQ.tile([P, H * E], fp32, tag="psq", name="pqf")
            pq2 = pqf[:, 0:E]
            for jt in range(JT):
                nc.tensor.matmul(pq2[:],
                                 lhsT=adjT[:, jt, ib * P:(ib + 1) * P],
                                 rhs=whD2[:, jt, H * E:U * E],
                                 start=(jt == 0), stop=(jt == JT - 1))
            nc.vector.tensor_copy(q2sb[:, ib, :], pq2[:])

        patts2 = [psATT.tile([P, 2, E], fp32, tag="att",
                             name=f"patt2_{ib}") for ib in range(IC)]
        Xs2 = emit_masks(U - 1, f2ocol[:, :, 0], f2oK[:, :, 0], pat2)
        emit_mms(patts2, Xs2,
                 lambda jt: whB[:, jt, U - 1, :],
                 lambda jt: whD2[:, jt, (U - 1) * E:U * E])
        st2 = work.tile([P, IC, 2, E], fp32, tag="st2")
        for ib in range(IC):
            if ib % 2 == 0:
                nc.vector.tensor_copy(st2[:, ib, :, :], patts2[ib][:])
            else:
                nc.scalar.copy(st2[:, ib, :, :], patts2[ib][:])
        x2 = wk1.tile([P, IC, E], fp32, tag="x2")
        nc.vector.tensor_tensor(x2[:], q2sb[:], st2[:, :, 1, :],
                                op=ALU.subtract)
        v2 = wk1.tile([P, IC, E], fp32, tag="v2")
        a0, a1 = broadcast_tensor_aps(
            x2[:], w_col[:, :, U - 1].rearrange("p (c o) -> p c o", o=1))
        nc.vector.tensor_tensor(v2[:], a0, a1, op=ALU.mult)
        t2 = wk1.tile([P, IC, E], fp32, tag="t2")
        nc.vector.tensor_tensor(t2[:], st2[:, :, 0, :], v2[:], op=ALU.add)
        rec2 = wk1.tile([P, IC, 1], fp32, tag="rec2")
        nc.vector.reciprocal(rec2[:], t2[:, :, D:E])
        z = wk1.tile([P, IC, C], fp32, tag="z")
        a0, a1 = broadcast_tensor_aps(t2[:, :, 0:D], rec2[:])
        nc.vector.tensor_tensor(z[:], a0, a1, op=ALU.mult)

        # log_softmax along free dim (no transposes needed)
        negmx = wk1.tile([P, IC, 1], fp32, tag="negmx")
        nc.vector.tensor_reduce(negmx[:], z[:], axis=AX.X, op=ALU.max,
                                negate=True)
        zs = wk1.tile([P, IC, C], fp32, tag="zs")
        a0, a1 = broadcast_tensor_aps(z[:], negmx[:])
        nc.vector.tensor_tensor(zs[:], a0, a1, op=ALU.add)
        ez = wk1.tile([P, IC, C], fp32, tag="ez")
        nc.scalar.activation(ez[:], zs[:], AF.Exp, bias=0.0, scale=1.0)
        sume = wk1.tile([P, IC, 1], fp32, tag="sume")
        nc.vector.tensor_reduce(sume[:], ez[:], axis=AX.X, op=ALU.add)
        lns = wk1.tile([P, IC, 1], fp32, tag="lns")
        nc.scalar.activation(lns[:], sume[:], AF.Ln, bias=0.0, scale=1.0)
        zo = wk1.tile([P, IC, C], fp32, tag="zo")
        a0, a1 = broadcast_tensor_aps(zs[:], lns[:])
        nc.vector.tensor_tensor(zo[:], a0, a1, op=ALU.subtract)
        nc.sync.dma_start(out.rearrange("(c p) e -> p c e", p=P), zo[:])

    nc.compile()
    return nc


def prep_inputs(x, adj, W1, a1, Wout, a_out, n=4096, ncores=NCORES):
    """Host-side prep: slice + transpose + bf16 cast + weight folds."""
    R = n // ncores
    x = np.asarray(x, np.float32)
    adj = np.asarray(adj)
    W1 = np.asarray(W1, np.float32)
    a1 = np.asarray(a1, np.float32)
    Wout = np.asarray(Wout, np.float32)
    a_out = np.asarray(a_out, np.float32)

    xT = np.ascontiguousarray(x.T).astype(ml_dtypes.float8_e4m3fn)
    W1a = W1.transpose(1, 0, 2).reshape(F, H * D)
    w2c = np.einsum("hfd,hd->fh", W1, a1[:, D:])
    w1c = np.einsum("hfd,hd->fh", W1, a1[:, :D])
    w2o = Wout @ a_out[C:]
    w1o = Wout @ a_out[:C]
    Wo2 = np.ascontiguousarray(np.concatenate(
        [Wout, w2o[:, None], w1o[:, None]], axis=1)).astype(BF16)
    identf = np.eye(P, dtype=np.float32)

    adjf = adj.astype(np.float32)
    in_maps = []
    for k in range(ncores):
        rows = slice(k * R, (k + 1) * R)
        xwk = np.concatenate([x[rows].T, w2c, W1a, w1c], axis=1)
        JTl = n // P
        at = adjf[rows].astype(BF16).T                     # [n, R]
        at = np.ascontiguousarray(
            at.reshape(JTl, P, R).transpose(1, 0, 2)).reshape(P, JTl * R)
        in_maps.append({
            "xT": xT,
            "xw": np.ascontiguousarray(xwk).astype(BF16),
            "adjt": at,
            "Wo2": Wo2,
            "identf": identf,
        })
    return in_maps


_cached = {}


def kernel(x, adj, W1, a1, Wout, a_out):
    n = x.shape[0]
    if n not in _cached:
        _cached[n] = build_gat(n=n)
    nc = _cached[n]
    in_maps = prep_inputs(x, adj, W1, a1, Wout, a_out, n=n)
    res = run_bass_kernel_spmd(nc, in_maps, core_ids=list(range(NCORES)))
    outs = [res.results[k]["out"] for k in range(NCORES)]
    return np.concatenate(outs, axis=0)
